# revision 1
# baseline (speedup 1.0000x reference)
"""Trainium2 Bass kernel for nn_CogAgentDecoderLayer (8-core SPMD).

Feature-major activations [feat, tok] in permuted token order
(vis-only | both | lang-only | neither). TP: QKV/dense by heads (2/core),
MLP by intermediate slice (688/core), cross-attn token-parallel
(256 tok/core). Device collectives: ReduceScatter after self-attn dense,
AllGather of normed h2 before MLP; final partial-sum reduce on host.
Self-attention runs in permuted order with a host-built causal mask.
bf16 for h/qkv/attn-probs/cross-kv, fp32r for dense/cq/cdense/MLP.
"""
import os
import numpy as np
from contextlib import ExitStack
from concourse import bacc, tile, mybir
from concourse.bass_utils import run_bass_kernel_spmd

NC_ = 8
S, E, H, NH, HD = 2048, 2048, 2048, 16, 128
CH, CC, CHD = 1024, 1024, 64
I = 5504
IS = I // NC_          # 688
ISP = 768              # padded to 6*128
EPS = 1e-5
ROPE_BASE = 10000.0
F32 = mybir.dt.float32
F32R = mybir.dt.float32r
BF16 = mybir.dt.bfloat16
DVE_F32R = True        # DVE may write fp32r tiles directly


def _segs(lo, hi, b0, b1, b2):
    pts = sorted({lo, hi, *[b for b in (b0, b1, b2) if lo < b < hi]})
    out = []
    for s, e in zip(pts, pts[1:]):
        ex = []
        if s < b1:
            ex.append(0)
        if b0 <= s < b2:
            ex.append(1)
        out.append((s, e, ex))
    return out


def _chunks(lo, hi, w):
    out = []
    while lo < hi:
        out.append((lo, min(lo + w, hi)))
        lo += w
    return out


def build_kernel(b0, b1, b2):
    nc = bacc.Bacc("TRN2", target_bir_lowering=False, debug=False,
                   num_devices=NC_)
    din = lambda n, sh, dt: nc.dram_tensor(n, sh, dt, kind="ExternalInput")
    hT = din("hT", [H, S], BF16)
    wqkv0 = din("wqkv0", [H, 768], BF16)
    wqkv1 = din("wqkv1", [H, 768], BF16)
    wd0 = din("wd0", [256, H], F32R)
    wd1 = din("wd1", [256, H], F32R)
    cos2 = din("cos2", [128, S], BF16)
    sin2 = din("sin2", [128, S], BF16)
    rotT = din("rotT", [128, 128], BF16)
    onesr = din("onesr", [128, 128], F32R)
    onesb = din("onesb", [128, 128], BF16)
    zeros = din("zeros", [128, 512], F32R)
    maskneg = din("maskneg", [S, S], BF16)
    resid = din("resid", [H, 256], F32R)
    encT = din("encT", [CH, E], BF16)
    wk = din("wk", [CH, CC], BF16)
    wvv = din("wvv", [CH, CC], BF16)
    wcq = din("wcq", [H, CC], F32R)
    wcd = din("wcd", [CC, H], F32R)
    wgu0 = din("wgu0", [H, 2 * IS], BF16)
    wgu1 = din("wgu1", [H, 2 * IS], BF16)
    wdn0 = din("wdn0", [ISP, H], BF16)
    wdn1 = din("wdn1", [ISP, H], BF16)
    y = nc.dram_tensor("y", [H, S], F32, kind="ExternalOutput")

    SC = 1.0 / float(np.sqrt(HD))
    CSC = 1.0 / float(np.sqrt(CHD))
    EXP = mybir.ActivationFunctionType.Exp
    SQ = mybir.ActivationFunctionType.Square
    SQRT = mybir.ActivationFunctionType.Sqrt
    SILU = mybir.ActivationFunctionType.Silu
    r128 = lambda ap: ap.rearrange("(c p) n -> p c n", p=128)

    with tile.TileContext(nc) as tc, ExitStack() as top:
        const = top.enter_context(tc.tile_pool(name="const", bufs=1))
        ones_sb = const.tile([128, 128], F32R)
        nc.sync.dma_start(ones_sb[:], onesr.ap()[:])
        ones_bf = const.tile([128, 128], BF16)
        nc.sync.dma_start(ones_bf[:], onesb.ap()[:])
        rot_sb = const.tile([128, 128], BF16)
        nc.sync.dma_start(rot_sb[:], rotT.ap()[:])
        from concourse.masks import make_identity
        ident = const.tile([128, 128], BF16)
        make_identity(nc, ident[:])
        cos_sb = const.tile([128, S], BF16)
        nc.sync.dma_start(cos_sb[:], cos2.ap()[:])
        sin_sb = const.tile([128, S], BF16)
        nc.sync.dma_start(sin_sb[:], sin2.ap()[:])
        zer_sb = const.tile([128, 512], F32R)
        nc.sync.dma_start(zer_sb[:], zeros.ap()[:])
        eps_sb = const.tile([128, 1], F32)
        nc.vector.memset(eps_sb[:], EPS)

        dram = top.enter_context(tc.tile_pool(name="dram", bufs=1, space="DRAM"))
        bounce = dram.tile([NC_ * H, 256], F32)
        rs_out = dram.tile([H, 256], F32)
        h2n_bnc = dram.tile([H, 256], BF16)
        h2n_all = dram.tile([NC_ * H, 256], BF16, addr_space="Shared")
        h2out = nc.dram_tensor("h2out", [H, 256], F32, kind="ExternalOutput")

        scrp = top.enter_context(tc.tile_pool(name="scr", bufs=2))

        def vwrite(op, dst, a, bb):
            if DVE_F32R:
                op(dst, a, bb)
            else:
                scr = scrp.tile([dst.shape[0], dst.shape[-1]], F32,
                                name="vscr", tag="vscr")
                op(scr[:], a, bb)
                nc.scalar.copy(dst, scr[:])

        pABC = top.enter_context(ExitStack())
        qkp = pABC.enter_context(tc.tile_pool(name="qkp", bufs=1))
        qkv_sb = qkp.tile([128, 6, S], BF16)      # q0 q1 k0 k1 v0 v1
        v_sb = qkp.tile([128, 16, 256], BF16)     # token-major v
        ctxp = pABC.enter_context(tc.tile_pool(name="ctxp", bufs=1))
        ctx_sb = ctxp.tile([128, 2, S], F32R)

        # ===== phase A: h load + rmsnorm + QKV + rope + vT =====
        with ExitStack() as pA:
            hp = pA.enter_context(tc.tile_pool(name="hp", bufs=1))
            h_sb = hp.tile([128, 16, S], BF16)
            nc.sync.dma_start(h_sb[:], r128(hT.ap()))
            with ExitStack() as pA1:
                nrm = pA1.enter_context(tc.tile_pool(name="nrm", bufs=2))
                nps = pA1.enter_context(tc.tile_pool(name="nps", bufs=2,
                                                     space="PSUM"))
                for t0, t1 in _chunks(0, S, 512):
                    pss = nps.tile([128, 512], F32, name="pss", tag="pss")
                    for kc in range(16):
                        sq = nrm.tile([128, 512], F32R, name="sq", tag="sq")
                        nc.scalar.activation(sq[:], h_sb[:, kc, t0:t1], SQ)
                        nc.tensor.matmul(pss[:], ones_sb[:], sq[:],
                                         start=(kc == 0), stop=(kc == 15))
                    rms = nrm.tile([128, 512], F32, name="rms", tag="rms")
                    nc.scalar.activation(rms[:], pss[:], SQRT,
                                         scale=1.0 / H, bias=eps_sb[:])
                    rinv = nrm.tile([128, 512], F32, name="rinv", tag="rinv")
                    nc.vector.reciprocal(rinv[:], rms[:])
                    for kc in range(16):
                        nc.vector.tensor_mul(h_sb[:, kc, t0:t1],
                                             h_sb[:, kc, t0:t1], rinv[:])
            with ExitStack() as pA2:
                wp = pA2.enter_context(tc.tile_pool(name="wp", bufs=3))
                mps = pA2.enter_context(tc.tile_pool(name="mps", bufs=2,
                                                     space="PSUM"))
                for slot in range(6):
                    wts = []
                    for ex, wsrc in ((0, wqkv0), (1, wqkv1)):
                        wt = wp.tile([128, 16, 128], BF16,
                                     name=f"wq{ex}{slot}", tag=f"wq{ex}")
                        nc.sync.dma_start(
                            wt[:], r128(wsrc.ap()[:, slot * 128:slot * 128 + 128]))
                        wts.append(wt)
                    for t0, t1 in _chunks(0, S, 512):
                        sg = [x for x in _segs(t0, t1, b0, b1, b2) if x[2]]
                        if not sg:
                            continue
                        need = sorted({x for _, _, ex in sg for x in ex})
                        pss_ = {}
                        for x in need:
                            ps = mps.tile([128, 512], F32, name=f"qps{x}",
                                          tag=f"qps{x}")
                            for kc in range(16):
                                nc.tensor.matmul(ps[:], wts[x][:, kc, :],
                                                 h_sb[:, kc, t0:t1],
                                                 start=(kc == 0), stop=(kc == 15))
                            pss_[x] = ps
                        for s, e, ex in sg:
                            if len(ex) == 1:
                                nc.vector.tensor_copy(qkv_sb[:, slot, s:e],
                                                      pss_[ex[0]][:, s - t0:e - t0])
                            else:
                                nc.vector.tensor_add(qkv_sb[:, slot, s:e],
                                                     pss_[0][:, s - t0:e - t0],
                                                     pss_[1][:, s - t0:e - t0])
                    if b2 < S:
                        nc.vector.memset(qkv_sb[:, slot, b2:S], 0.0)
                # rope on q,k
                for slot in range(4):
                    for t0, t1 in _chunks(0, S, 512):
                        rp = mps.tile([128, 512], F32, name="rps", tag="qps")
                        nc.tensor.matmul(rp[:], rot_sb[:],
                                         qkv_sb[:, slot, t0:t1],
                                         start=True, stop=True)
                        c1 = scrp.tile([128, 512], F32, name="ropec", tag="ropec")
                        nc.vector.tensor_mul(c1[:], qkv_sb[:, slot, t0:t1],
                                             cos_sb[:, t0:t1])
                        s1 = scrp.tile([128, 512], F32, name="ropes", tag="ropes")
                        nc.vector.tensor_mul(s1[:], rp[:], sin_sb[:, t0:t1])
                        nc.vector.tensor_add(qkv_sb[:, slot, t0:t1],
                                             c1[:], s1[:])
                # v -> token-major via PE transpose
                for hh in range(2):
                    for tt in range(16):
                        tp = mps.tile([128, 512], BF16, name="tps", tag="qps")
                        nc.tensor.transpose(
                            tp[:, :128],
                            qkv_sb[:, 4 + hh, tt * 128:tt * 128 + 128],
                            ident[:])
                        nc.vector.tensor_copy(v_sb[:, tt, hh * 128:hh * 128 + 128],
                                       tp[:, :128])
        # ===== phase B: self-attention (perm order) =====
        with ExitStack() as pB:
            ap_ = pB.enter_context(tc.tile_pool(name="ap", bufs=3))
            aps = pB.enter_context(tc.tile_pool(name="aps", bufs=2, space="PSUM"))
            accp = pB.enter_context(tc.tile_pool(name="accp", bufs=1, space="PSUM"))
            for t0, t1 in _chunks(0, S, 512):
                pss_ = [accp.tile([128, 512], F32, name=f"pssum{h}", tag=f"pssum{h}")
                        for h in range(2)]
                psc_ = [accp.tile([128, 512], F32, name=f"psctx{h}", tag=f"psctx{h}")
                        for h in range(2)]
                for kt in range(16):
                    mt_ = ap_.tile([128, 512], BF16, name="mt", tag="mt")
                    nc.sync.dma_start(
                        mt_[:], maskneg.ap()[kt * 128:kt * 128 + 128, t0:t1])
                    for hh in range(2):
                        sc = aps.tile([128, 512], F32, name="sc", tag="sc")
                        nc.tensor.matmul(
                            sc[:], qkv_sb[:, 2 + hh, kt * 128:kt * 128 + 128],
                            qkv_sb[:, hh, t0:t1], start=True, stop=True)
                        nc.vector.tensor_add(sc[:], sc[:], mt_[:])
                        pr = ap_.tile([128, 512], BF16, name="pr", tag="pr")
                        nc.scalar.activation(pr[:], sc[:], EXP, scale=SC)
                        nc.tensor.matmul(pss_[hh][:], ones_bf[:], pr[:],
                                         start=(kt == 0), stop=(kt == 15))
                        nc.tensor.matmul(
                            psc_[hh][:], v_sb[:, kt, hh * 128:hh * 128 + 128],
                            pr[:], start=(kt == 0), stop=(kt == 15))
                for hh in range(2):
                    rc = ap_.tile([128, 512], F32, name="rc", tag="rc")
                    nc.vector.reciprocal(rc[:], pss_[hh][:])
                    vwrite(nc.vector.tensor_mul, ctx_sb[:, hh, t0:t1],
                           psc_[hh][:], rc[:])
        # ===== phase C: dense (routed) -> bounce -> RS =====
        with ExitStack() as pC:
            dwp = pC.enter_context(tc.tile_pool(name="dwp", bufs=1))
            dps = pC.enter_context(tc.tile_pool(name="dps", bufs=2, space="PSUM"))
            dop = pC.enter_context(tc.tile_pool(name="dop", bufs=4))
            dwts = []
            for ex, wsrc in ((0, wd0), (1, wd1)):
                dwt = dwp.tile([128, 2, H], F32R, name=f"dw{ex}", tag=f"dw{ex}")
                nc.sync.dma_start(dwt[:], r128(wsrc.ap()))
                dwts.append(dwt)
            for tt in range(8):
                t0, t1 = tt * 256, tt * 256 + 256
                sg = _segs(t0, t1, b0, b1, b2)
                live = [x for x in sg if x[2]]
                for mt in range(16):
                    ot = dop.tile([128, 256], F32, name="dot", tag="dot")
                    if live:
                        need = sorted({x for _, _, ex in live for x in ex})
                        pss_ = {}
                        for x in need:
                            ps = dps.tile([128, 256], F32, name=f"dpst{x}",
                                          tag=f"dpst{x}")
                            for kc in range(2):
                                nc.tensor.matmul(
                                    ps[:],
                                    dwts[x][:, kc, mt * 128:mt * 128 + 128],
                                    ctx_sb[:, kc, t0:t1],
                                    start=(kc == 0), stop=(kc == 1))
                            pss_[x] = ps
                        for s, e, ex in sg:
                            if len(ex) == 2:
                                nc.vector.tensor_add(ot[:, s - t0:e - t0],
                                                     pss_[0][:, s - t0:e - t0],
                                                     pss_[1][:, s - t0:e - t0])
                            elif ex:
                                nc.vector.tensor_copy(ot[:, s - t0:e - t0],
                                                      pss_[ex[0]][:, s - t0:e - t0])
                            else:
                                nc.vector.memset(ot[:, s - t0:e - t0], 0.0)
                    else:
                        nc.vector.memset(ot[:], 0.0)
                    nc.sync.dma_start(
                        bounce[tt * H + mt * 128: tt * H + mt * 128 + 128, :],
                        ot[:])
        pABC.close()
        nc.gpsimd.collective_compute(
            "ReduceScatter", mybir.AluOpType.add,
            replica_groups=[list(range(NC_))],
            ins=[bounce.opt()], outs=[rs_out.opt()])

        # ===== phase D: cross attention (token-parallel) =====
        with ExitStack() as pD:
            dp = pD.enter_context(tc.tile_pool(name="dp", bufs=1))
            dps2 = pD.enter_context(tc.tile_pool(name="dps2", bufs=2, space="PSUM"))
            h1_sb = dp.tile([128, 16, 256], F32R)
            cq_sb = dp.tile([128, 8, 256], BF16)
            cctx_sb = dp.tile([128, 8, 256], F32R)
            with ExitStack() as pD1:
                d1 = pD1.enter_context(tc.tile_pool(name="d1", bufs=1))
                rs_sb = d1.tile([128, 16, 256], F32)
                nc.sync.dma_start(rs_sb[:], r128(rs_out[:]))
                re_sb = d1.tile([128, 16, 256], F32R)
                nc.sync.dma_start(re_sb[:], r128(resid.ap()))
                for kc in range(16):
                    vwrite(nc.vector.tensor_add, h1_sb[:, kc, :],
                           rs_sb[:, kc, :], re_sb[:, kc, :].bitcast(F32))
                pss = dps2.tile([128, 256], F32, name="psd", tag="psd")
                for kc in range(16):
                    sq = scrp.tile([128, 256], F32R, name="sqd", tag="sqd")
                    nc.scalar.activation(sq[:], h1_sb[:, kc, :].bitcast(F32), SQ)
                    nc.tensor.matmul(pss[:], ones_sb[:], sq[:],
                                     start=(kc == 0), stop=(kc == 15))
                rms = scrp.tile([128, 256], F32, name="rmsd", tag="rmsd")
                nc.scalar.activation(rms[:], pss[:], SQRT, scale=1.0 / H, bias=eps_sb[:])
                rinv = d1.tile([128, 256], F32)
                nc.vector.reciprocal(rinv[:], rms[:])
                h1n_sb = d1.tile([128, 16, 256], F32R)
                for kc in range(16):
                    vwrite(nc.vector.tensor_mul, h1n_sb[:, kc, :],
                           h1_sb[:, kc, :].bitcast(F32), rinv[:])
                for mt in range(8):
                    wcq_t = d1.tile([128, 16, 128], F32R, name="wcqt", tag="wcqt",
                                    bufs=2)
                    nc.sync.dma_start(
                        wcq_t[:], r128(wcq.ap()[:, mt * 128:mt * 128 + 128]))
                    ps = dps2.tile([128, 256], F32, name="cqp", tag="psd")
                    for kc in range(16):
                        nc.tensor.matmul(ps[:],
                                         wcq_t[:, kc, :],
                                         h1n_sb[:, kc, :],
                                         start=(kc == 0), stop=(kc == 15))
                    nc.vector.tensor_copy(cq_sb[:, mt, :], ps[:])
            with ExitStack() as pD2:
                kp = pD2.enter_context(tc.tile_pool(name="kp", bufs=1))
                k_sb = kp.tile([128, 8, E], BF16)
                v_sb2 = kp.tile([128, 16, CC], BF16)
                with ExitStack() as pD2e:
                    ep = pD2e.enter_context(tc.tile_pool(name="ep", bufs=1))
                    enc_sb = ep.tile([128, 8, E], BF16)
                    nc.sync.dma_start(enc_sb[:], r128(encT.ap()))
                    wk_sb = ep.tile([128, 8, CC], BF16)
                    nc.sync.dma_start(wk_sb[:], r128(wk.ap()))
                    wv_sb = ep.tile([128, 8, CC], BF16)
                    nc.sync.dma_start(wv_sb[:], r128(wvv.ap()))
                    for mt in range(8):
                        for n0, n1 in _chunks(0, E, 512):
                            ps = dps2.tile([128, 512], F32, name="kps", tag="kps")
                            for kc in range(8):
                                nc.tensor.matmul(
                                    ps[:], wk_sb[:, kc, mt * 128:mt * 128 + 128],
                                    enc_sb[:, kc, n0:n1],
                                    start=(kc == 0), stop=(kc == 7))
                            nc.vector.tensor_copy(k_sb[:, mt, n0:n1], ps[:])
                    for tt in range(16):
                        for n0, n1 in _chunks(0, CC, 512):
                            ps = dps2.tile([128, 512], F32, name="vps", tag="kps")
                            for kc in range(8):
                                nc.tensor.matmul(
                                    ps[:], enc_sb[:, kc, tt * 128:tt * 128 + 128],
                                    wv_sb[:, kc, n0:n1],
                                    start=(kc == 0), stop=(kc == 7))
                            nc.vector.tensor_copy(v_sb2[:, tt, n0:n1], ps[:])
                with ExitStack() as pD3:
                    cap = pD3.enter_context(tc.tile_pool(name="cap", bufs=3))
                    caps = pD3.enter_context(tc.tile_pool(name="caps", bufs=2,
                                                          space="PSUM"))
                    cacc = pD3.enter_context(tc.tile_pool(name="cacc", bufs=1,
                                                          space="PSUM"))
                    for h in range(NH):
                        kch, koff = h // 2, 64 * (h % 2)
                        pssum = cacc.tile([128, 256], F32, name="cps", tag="cps")
                        psctx = cacc.tile([64, 256], F32, name="cpc", tag="cpc")
                        for kt in range(16):
                            sc = caps.tile([128, 256], F32, name="csc", tag="csc")
                            nc.tensor.matmul(
                                sc[:],
                                k_sb[koff:koff + 64, kch, kt * 128:kt * 128 + 128],
                                cq_sb[koff:koff + 64, kch, :],
                                start=True, stop=True)
                            pr = cap.tile([128, 256], BF16, name="cpr", tag="cpr")
                            nc.scalar.activation(pr[:], sc[:], EXP, scale=CSC)
                            nc.tensor.matmul(pssum[:], ones_bf[:], pr[:],
                                             start=(kt == 0), stop=(kt == 15))
                            nc.tensor.matmul(psctx[:],
                                             v_sb2[:, kt, 64 * h:64 * h + 64],
                                             pr[:], start=(kt == 0), stop=(kt == 15))
                        rc = cap.tile([64, 256], F32, name="crc", tag="crc")
                        nc.vector.reciprocal(rc[:], pssum[:64, :])
                        vwrite(nc.vector.tensor_mul,
                               cctx_sb[koff:koff + 64, kch, :], psctx[:], rc[:])
            # cdense + residual -> h2, rmsnorm -> h2n -> AG
            with ExitStack() as pD4:
                d4 = pD4.enter_context(tc.tile_pool(name="d4", bufs=1))
                h2_sb = d4.tile([128, 16, 256], F32)
                h2n_sb = d4.tile([128, 16, 256], BF16)
                wcd_sb = d4.tile([128, 8, H], F32R)
                nc.sync.dma_start(wcd_sb[:], r128(wcd.ap()))
                for mt in range(16):
                    ps = dps2.tile([128, 256], F32, name="cdp", tag="psd")
                    for kc in range(8):
                        nc.tensor.matmul(ps[:],
                                         wcd_sb[:, kc, mt * 128:mt * 128 + 128],
                                         cctx_sb[:, kc, :],
                                         start=(kc == 0), stop=(kc == 7))
                    nc.vector.tensor_add(h2_sb[:, mt, :], ps[:],
                                         h1_sb[:, mt, :].bitcast(F32))
                pss2 = dps2.tile([128, 256], F32, name="psd2", tag="psd")
                for kc in range(16):
                    sq = scrp.tile([128, 256], F32R, name="sqd2", tag="sqd")
                    nc.scalar.activation(sq[:], h2_sb[:, kc, :], SQ)
                    nc.tensor.matmul(pss2[:], ones_sb[:], sq[:],
                                     start=(kc == 0), stop=(kc == 15))
                rms2 = scrp.tile([128, 256], F32, name="rmsd2", tag="rmsd")
                nc.scalar.activation(rms2[:], pss2[:], SQRT,
                                     scale=1.0 / H, bias=eps_sb[:])
                rinv2 = d4.tile([128, 256], F32)
                nc.vector.reciprocal(rinv2[:], rms2[:])
                for kc in range(16):
                    nc.vector.tensor_mul(h2n_sb[:, kc, :],
                                         h2_sb[:, kc, :], rinv2[:])
                nc.sync.dma_start(r128(h2n_bnc[:]), h2n_sb[:])
                nc.sync.dma_start(r128(h2out.ap()), h2_sb[:])
            nc.gpsimd.collective_compute(
                "AllGather", mybir.AluOpType.bypass,
                replica_groups=[list(range(NC_))],
                ins=[h2n_bnc.opt()], outs=[h2n_all.opt()])
        # ===== phase F: MLP (routed by expert ranges, bf16) =====
        with ExitStack() as pF:
            fp = pF.enter_context(tc.tile_pool(name="fp", bufs=1))
            hn_sb = fp.tile([128, 16, S], BF16)
            for r in range(NC_):
                nc.sync.dma_start(hn_sb[:, :, r * 256:r * 256 + 256],
                                  r128(h2n_all[r * H:(r + 1) * H, :]))
            fw = pF.enter_context(tc.tile_pool(name="fw", bufs=1))
            fps = pF.enter_context(tc.tile_pool(name="fps", bufs=1, space="PSUM"))
            fpd = pF.enter_context(tc.tile_pool(name="fpd", bufs=2, space="PSUM"))
            fac = pF.enter_context(tc.tile_pool(name="fac", bufs=2))
            fout = pF.enter_context(tc.tile_pool(name="fout", bufs=4))
            for ex, (lo, hi) in ((0, (0, b1)), (1, (b1, S))):
                gsrc = (wgu0, wgu1)[ex]
                dsrc = (wdn0, wdn1)[ex]
                dn_t = fw.tile([128, 6, H], BF16, name=f"dn{ex}", tag="dn")
                nc.sync.dma_start(dn_t[:], r128(dsrc.ap()))
                gwts = []
                for pi in range(6):
                    gw = 128 if pi < 5 else 48
                    gwt = fw.tile([128, 16, 256], BF16,
                                  name=f"guw{ex}{pi}", tag=f"guw{pi}")
                    nc.sync.dma_start(
                        gwt[:, :, :2 * gw],
                        r128(gsrc.ap()[:, pi * 256:pi * 256 + 2 * gw]))
                    gwts.append(gwt)
                for a0 in range(0, S, 512):
                    c0, c1 = max(a0, lo), min(a0 + 512, hi)
                    if c0 >= c1:
                        continue
                    t0_, W = a0, 512
                    eo, ew = c0 - a0, c1 - c0
                    act = fac.tile([128, 6, 512], BF16, name="act", tag="act")
                    for pi in range(6):
                        gw = 128 if pi < 5 else 48
                        gwt = gwts[pi]
                        pg = fps.tile([128, 512], F32, name="pg", tag="pg")
                        pu = fps.tile([128, 512], F32, name="pu", tag="pu")
                        for kc in range(16):
                            nc.tensor.matmul(pg[:gw, :W], gwt[:, kc, :gw],
                                             hn_sb[:, kc, t0_:t0_ + 512],
                                             start=(kc == 0), stop=(kc == 15))
                            nc.tensor.matmul(pu[:gw, :W], gwt[:, kc, gw:2 * gw],
                                             hn_sb[:, kc, t0_:t0_ + 512],
                                             start=(kc == 0), stop=(kc == 15))
                        gs = scrp.tile([128, 512], F32, name="gs", tag="gs")
                        nc.scalar.activation(gs[:gw, :W], pg[:gw, :W], SILU)
                        nc.vector.tensor_mul(act[:gw, pi, :W],
                                             gs[:gw, :W], pu[:gw, :W])
                    for mt in range(16):
                        pd = fpd.tile([128, 512], F32, name="pd", tag="pd")
                        for pi in range(6):
                            kw = 128 if pi < 5 else 48
                            nc.tensor.matmul(
                                pd[:, :W],
                                dn_t[:kw, pi, mt * 128:mt * 128 + 128],
                                act[:kw, pi, :W],
                                start=(pi == 0), stop=(pi == 5))
                        ot = fout.tile([128, 512], F32, name="fot", tag="fot")
                        nc.vector.tensor_copy(ot[:, eo:eo + ew], pd[:, eo:eo + ew])
                        nc.sync.dma_start(
                            y.ap()[mt * 128:mt * 128 + 128, c0:c1],
                            ot[:, eo:eo + ew])
    nc.compile()
    return nc


_CACHE = {}


def kernel(**inputs):
    import ml_dtypes
    vm = np.asarray(inputs["vision_token_ids"]).astype(bool)
    lm = np.asarray(inputs["language_token_ids"]).astype(bool)
    g0 = np.where(vm & ~lm)[0]; g1 = np.where(vm & lm)[0]
    g2 = np.where(~vm & lm)[0]; g3 = np.where(~vm & ~lm)[0]
    perm = np.concatenate([g0, g1, g2, g3])
    b0 = len(g0); b1 = b0 + len(g1); b2 = b1 + len(g2)

    f32 = lambda x: np.ascontiguousarray(np.asarray(x, np.float32))
    bf = lambda x: np.ascontiguousarray(np.asarray(x).astype(ml_dtypes.bfloat16))
    pos = np.asarray(inputs["positions"]).astype(np.float32)
    half = HD // 2
    inv_freq = 1.0 / (ROPE_BASE ** (np.arange(half, dtype=np.float32) / half))
    fr = pos[:, None] * inv_freq[None, :]
    cos2 = np.concatenate([np.cos(fr)] * 2, 1).T[:, perm]
    sin2 = np.concatenate([np.sin(fr)] * 2, 1).T[:, perm]
    rot = np.zeros((HD, HD), np.float32)
    rot[np.arange(half), np.arange(half) + half] = -1.0
    rot[np.arange(half) + half, np.arange(half)] = 1.0
    op = np.asarray(inputs["positions"])[perm]
    maskneg = np.where(op[None, :] >= op[:, None], 0.0, -30000.0)

    wln_in = f32(inputs["w_ln_in"])[:, None]
    wln_pa = f32(inputs["w_ln_post_attn"])[:, None]
    wln_pc = f32(inputs["w_ln_post_cross"])[:, None]
    wqkv = [f32(inputs["w_vis_qkv"]) * wln_in, f32(inputs["w_lang_qkv"]) * wln_in]
    wd = [f32(inputs["w_vis_dense"]), f32(inputs["w_lang_dense"])]
    wgu = [f32(inputs["w_vis_gate_up"]) * wln_pc,
           f32(inputs["w_lang_gate_up"]) * wln_pc]
    wdn = [f32(inputs["w_vis_down"]), f32(inputs["w_lang_down"])]
    wkvf = f32(inputs["w_cross_kv"])
    hTp = f32(inputs["hidden_states"]).T[:, perm].copy()

    def interleave(w):  # w [H, 2*IS] = [gate | up]
        cols = []
        for i in range(5):
            cols.append(w[:, 128 * i:128 * i + 128])
            cols.append(w[:, IS + 128 * i:IS + 128 * i + 128])
        cols.append(w[:, 640:IS]); cols.append(w[:, IS + 640:2 * IS])
        return np.ascontiguousarray(np.concatenate(cols, 1))

    key = (b0, b1, b2)
    if key not in _CACHE:
        _CACHE.clear()
        _CACHE[key] = build_kernel(b0, b1, b2)
    nc = _CACHE[key]

    in_maps = []
    for c in range(NC_):
        qs = slice(256 * c, 256 * c + 256)
        m = dict(
            hT=bf(hTp),
            wqkv0=bf(np.concatenate([wqkv[0][:, qs], wqkv[0][:, H:][:, qs],
                                     wqkv[0][:, 2 * H:][:, qs]], 1)),
            wqkv1=bf(np.concatenate([wqkv[1][:, qs], wqkv[1][:, H:][:, qs],
                                     wqkv[1][:, 2 * H:][:, qs]], 1)),
            wd0=wd[0][qs].copy(), wd1=wd[1][qs].copy(),
            cos2=bf(cos2), sin2=bf(sin2), rotT=bf(rot.T),
            onesr=np.ones((128, 128), np.float32),
            onesb=np.ones((128, 128), ml_dtypes.bfloat16),
            zeros=np.zeros((128, 512), np.float32),
            maskneg=bf(maskneg), resid=hTp[:, qs].copy(),
            encT=bf(f32(inputs["encoder_embeds"]).T),
            wk=bf(wkvf[:, :CC]), wvv=bf(wkvf[:, CC:]),
            wcq=(f32(inputs["w_cross_q"]) * wln_pa).copy(),
            wcd=f32(inputs["w_cross_dense"]),
            wgu0=bf(interleave(np.concatenate(
                [wgu[0][:, IS * c:IS * c + IS],
                 wgu[0][:, I + IS * c:I + IS * c + IS]], 1))),
            wgu1=bf(interleave(np.concatenate(
                [wgu[1][:, IS * c:IS * c + IS],
                 wgu[1][:, I + IS * c:I + IS * c + IS]], 1))),
            wdn0=bf(np.concatenate([wdn[0][IS * c:IS * c + IS],
                                    np.zeros((ISP - IS, H), np.float32)], 0)),
            wdn1=bf(np.concatenate([wdn[1][IS * c:IS * c + IS],
                                    np.zeros((ISP - IS, H), np.float32)], 0)),
        )
        in_maps.append(m)

    trace = bool(int(os.environ.get("KTRACE", "0")))
    res = run_bass_kernel_spmd(nc, in_maps, core_ids=list(range(NC_)),
                               trace=trace)
    kernel.last_exec_ns = res.exec_time_ns
    tot = res.results[0]["y"].astype(np.float64)
    for c in range(1, NC_):
        tot += res.results[c]["y"]
    for c in range(NC_):
        tot[:, 256 * c:256 * c + 256] += res.results[c]["h2out"]
    out = np.empty((S, H), np.float32)
    out[perm, :] = tot.T.astype(np.float32)
    return out



# revision 5
# speedup vs baseline: 1.5138x; 1.5138x over previous
"""Trainium2 Bass kernel for nn_CogAgentDecoderLayer (8-core SPMD).

Fast path (disjoint vis/lang masks, expert boundary % 256 == 0):
feature-major activations [feat, tok] in permuted token order.
Self-attn head-parallel (2 heads/core, block-sparse causal), then an
AllToAll re-shards ctx token-parallel (256 tok/core); dense, cross-attn
and MLP all run token-parallel with full weights streamed from HBM
(each core uses one expert's weights, chosen host-side). Cross-attn KV
is computed enc-token-sharded and AllGathered early (overlapped with
self-attention). No post-MLP collective: each core emits its final
[H, 256] f32 output slice. bf16 everywhere except residual trunk (f32).

General fallback (any masks): original head/intermediate-parallel
kernel with ReduceScatter + AllGather.
"""
import os
import numpy as np
from contextlib import ExitStack
from concourse import bacc, tile, mybir
from concourse.bass_utils import run_bass_kernel_spmd

NC_ = 8
S, E, H, NH, HD = 2048, 2048, 2048, 16, 128
CH, CC, CHD = 1024, 1024, 64
I = 5504
IS = I // NC_          # 688
ISP = 768              # padded to 6*128 (general path)
NPI = I // 128         # 43 (fast path)
EPS = 1e-5
ROPE_BASE = 10000.0
F32 = mybir.dt.float32
F32R = mybir.dt.float32r
BF16 = mybir.dt.bfloat16
DVE_F32R = True        # DVE may write fp32r tiles directly


def _segs(lo, hi, b0, b1, b2):
    pts = sorted({lo, hi, *[b for b in (b0, b1, b2) if lo < b < hi]})
    out = []
    for s, e in zip(pts, pts[1:]):
        ex = []
        if s < b1:
            ex.append(0)
        if b0 <= s < b2:
            ex.append(1)
        out.append((s, e, ex))
    return out


def _chunks(lo, hi, w):
    out = []
    while lo < hi:
        out.append((lo, min(lo + w, hi)))
        lo += w
    return out


def build_fast(b0, b1, b2, pattern, nslot):
    """pattern: per 512-chunk tuple of (kt, slot) with slot=-1 for
    fully-visible key tiles, else index into maskm."""
    nc = bacc.Bacc("TRN2", target_bir_lowering=False, debug=False,
                   num_devices=NC_)
    din = lambda n, sh, dt: nc.dram_tensor(n, sh, dt, kind="ExternalInput")
    hT = din("hT", [H, S], BF16)
    resid = din("resid", [H, 256], F32)
    wqkv0 = din("wqkv0", [H, 768], BF16)
    wqkv1 = din("wqkv1", [H, 768], BF16)
    cos2 = din("cos2", [128, S], BF16)
    sin2 = din("sin2", [128, S], BF16)
    rotT = din("rotT", [128, 128], BF16)
    onesr = din("onesr", [128, 128], F32R)
    onesb = din("onesb", [128, 128], BF16)
    maskm = din("maskm", [128, max(nslot, 1), 1024], BF16)
    encsl = din("encsl", [CH, 256], BF16)
    wk = din("wk", [CH, CC], BF16)
    wvv = din("wvv", [CH, CC], BF16)
    wdT = din("wdT", [16, H, 128], BF16)
    wcqT = din("wcqT", [8, H, 128], BF16)
    wcdT = din("wcdT", [16, CC, 128], BF16)
    wguI = din("wguI", [NPI, H, 256], BF16)
    wdnT = din("wdnT", [16, I, 128], BF16)
    y = nc.dram_tensor("y", [H, 256], F32, kind="ExternalOutput")

    SC = 1.0 / float(np.sqrt(HD))
    CSC = 1.0 / float(np.sqrt(CHD))
    EXP = mybir.ActivationFunctionType.Exp
    SQ = mybir.ActivationFunctionType.Square
    SQRT = mybir.ActivationFunctionType.Sqrt
    SILU = mybir.ActivationFunctionType.Silu
    r128 = lambda ap: ap.rearrange("(c p) n -> p c n", p=128)

    with tile.TileContext(nc) as tc, ExitStack() as top:
        const = top.enter_context(tc.tile_pool(name="const", bufs=1))
        ones_sb = const.tile([128, 128], F32R)
        nc.sync.dma_start(ones_sb[:], onesr.ap()[:])
        ones_bf = const.tile([128, 128], BF16)
        nc.sync.dma_start(ones_bf[:], onesb.ap()[:])
        rot_sb = const.tile([128, 128], BF16)
        nc.sync.dma_start(rot_sb[:], rotT.ap()[:])
        from concourse.masks import make_identity
        ident = const.tile([128, 128], BF16)
        make_identity(nc, ident[:])
        cos_sb = const.tile([128, S], BF16)
        nc.sync.dma_start(cos_sb[:], cos2.ap()[:])
        sin_sb = const.tile([128, S], BF16)
        nc.sync.dma_start(sin_sb[:], sin2.ap()[:])
        eps_sb = const.tile([128, 1], F32)
        nc.vector.memset(eps_sb[:], EPS)

        dram = top.enter_context(tc.tile_pool(name="dram", bufs=1, space="DRAM"))
        kv_bnc = dram.tile([2 * CC, 256], BF16)
        kv_all = dram.tile([NC_ * 2 * CC, 256], BF16, addr_space="Shared")
        ctx_bnc = dram.tile([NC_ * 256, 256], BF16)
        ctx_all = dram.tile([NC_ * 256, 256], BF16)

        scrp = top.enter_context(tc.tile_pool(name="scr", bufs=2))

        # ===== phase 0: cross KV for this core's 256 enc tokens -> AG ====
        with ExitStack() as p0:
            ep = p0.enter_context(tc.tile_pool(name="ep", bufs=1))
            enc_sb = ep.tile([128, 8, 256], BF16)
            nc.sync.dma_start(enc_sb[:], r128(encsl.ap()))
            wk_sb = ep.tile([128, 8, CC], BF16)
            nc.sync.dma_start(wk_sb[:], r128(wk.ap()))
            wv_sb = ep.tile([128, 8, CC], BF16)
            nc.sync.dma_start(wv_sb[:], r128(wvv.ap()))
            kps = p0.enter_context(tc.tile_pool(name="kps", bufs=2,
                                                space="PSUM"))
            kout = p0.enter_context(tc.tile_pool(name="kout", bufs=3))
            for mt in range(8):
                ps = kps.tile([128, 256], F32, name="kp", tag="kp")
                for kc in range(8):
                    nc.tensor.matmul(ps[:], wk_sb[:, kc, mt * 128:mt * 128 + 128],
                                     enc_sb[:, kc, :],
                                     start=(kc == 0), stop=(kc == 7))
                ko = kout.tile([128, 256], BF16, name="ko", tag="ko")
                nc.vector.tensor_copy(ko[:], ps[:])
                nc.sync.dma_start(kv_bnc[mt * 128:mt * 128 + 128, :], ko[:])
            vdst = kv_bnc[CC:2 * CC, :].rearrange("(p x) n -> p (x n)", p=256)
            for et in range(2):
                for hf in range(2):
                    ps = kps.tile([128, 512], F32, name="vp", tag="kp")
                    for kc in range(8):
                        nc.tensor.matmul(
                            ps[:], enc_sb[:, kc, et * 128:et * 128 + 128],
                            wv_sb[:, kc, hf * 512:hf * 512 + 512],
                            start=(kc == 0), stop=(kc == 7))
                    vo = kout.tile([128, 512], BF16, name="vo", tag="ko")
                    nc.vector.tensor_copy(vo[:], ps[:])
                    nc.sync.dma_start(
                        vdst[et * 128:et * 128 + 128, hf * 512:hf * 512 + 512],
                        vo[:])
        nc.gpsimd.collective_compute(
            "AllGather", mybir.AluOpType.bypass,
            replica_groups=[list(range(NC_))],
            ins=[kv_bnc.opt()], outs=[kv_all.opt()])

        pAB = top.enter_context(ExitStack())
        qkp = pAB.enter_context(tc.tile_pool(name="qkp", bufs=1))
        qkv_sb = qkp.tile([128, 6, S], BF16)      # q0 q1 k0 k1 v0 v1
        v_sb = qkp.tile([128, 16, 256], BF16)     # token-major v

        # ===== phase A: h load + rmsnorm + QKV + rope + vT =====
        with ExitStack() as pA:
            hp = pA.enter_context(tc.tile_pool(name="hp", bufs=1))
            h_sb = hp.tile([128, 16, S], BF16)
            nc.sync.dma_start(h_sb[:], r128(hT.ap()))
            with ExitStack() as pA1:
                nrm = pA1.enter_context(tc.tile_pool(name="nrm", bufs=2))
                nps = pA1.enter_context(tc.tile_pool(name="nps", bufs=2,
                                                     space="PSUM"))
                for t0, t1 in _chunks(0, S, 512):
                    pss = nps.tile([128, 512], F32, name="pss", tag="pss")
                    for kc in range(16):
                        sq = nrm.tile([128, 512], F32R, name="sq", tag="sq")
                        nc.scalar.activation(sq[:], h_sb[:, kc, t0:t1], SQ)
                        nc.tensor.matmul(pss[:], ones_sb[:], sq[:],
                                         start=(kc == 0), stop=(kc == 15))
                    rms = nrm.tile([128, 512], F32, name="rms", tag="rms")
                    nc.scalar.activation(rms[:], pss[:], SQRT,
                                         scale=1.0 / H, bias=eps_sb[:])
                    rinv = nrm.tile([128, 512], F32, name="rinv", tag="rinv")
                    nc.vector.reciprocal(rinv[:], rms[:])
                    for kc in range(16):
                        nc.vector.tensor_mul(h_sb[:, kc, t0:t1],
                                             h_sb[:, kc, t0:t1], rinv[:])
            with ExitStack() as pA2:
                wp = pA2.enter_context(tc.tile_pool(name="wp", bufs=3))
                mps = pA2.enter_context(tc.tile_pool(name="mps", bufs=2,
                                                     space="PSUM"))
                for slot in range(6):
                    wts = []
                    for ex, wsrc in ((0, wqkv0), (1, wqkv1)):
                        wt = wp.tile([128, 16, 128], BF16,
                                     name=f"wq{ex}{slot}", tag=f"wq{ex}")
                        nc.sync.dma_start(
                            wt[:], r128(wsrc.ap()[:, slot * 128:slot * 128 + 128]))
                        wts.append(wt)
                    for t0, t1 in _chunks(0, S, 512):
                        sg = [x for x in _segs(t0, t1, b0, b1, b2) if x[2]]
                        if not sg:
                            continue
                        need = sorted({x for _, _, ex in sg for x in ex})
                        pss_ = {}
                        for x in need:
                            ps = mps.tile([128, 512], F32, name=f"qps{x}",
                                          tag=f"qps{x}")
                            for kc in range(16):
                                nc.tensor.matmul(ps[:], wts[x][:, kc, :],
                                                 h_sb[:, kc, t0:t1],
                                                 start=(kc == 0), stop=(kc == 15))
                            pss_[x] = ps
                        for s, e, ex in sg:
                            if len(ex) == 1:
                                nc.vector.tensor_copy(qkv_sb[:, slot, s:e],
                                                      pss_[ex[0]][:, s - t0:e - t0])
                            else:
                                nc.vector.tensor_add(qkv_sb[:, slot, s:e],
                                                     pss_[0][:, s - t0:e - t0],
                                                     pss_[1][:, s - t0:e - t0])
                    if b2 < S:
                        nc.vector.memset(qkv_sb[:, slot, b2:S], 0.0)
                # rope on q,k
                for slot in range(4):
                    for t0, t1 in _chunks(0, S, 512):
                        rp = mps.tile([128, 512], F32, name="rps", tag="qps0")
                        nc.tensor.matmul(rp[:], rot_sb[:],
                                         qkv_sb[:, slot, t0:t1],
                                         start=True, stop=True)
                        c1 = scrp.tile([128, 512], F32, name="ropec", tag="ropec")
                        nc.vector.tensor_mul(c1[:], qkv_sb[:, slot, t0:t1],
                                             cos_sb[:, t0:t1])
                        s1 = scrp.tile([128, 512], F32, name="ropes", tag="ropes")
                        nc.vector.tensor_mul(s1[:], rp[:], sin_sb[:, t0:t1])
                        nc.vector.tensor_add(qkv_sb[:, slot, t0:t1],
                                             c1[:], s1[:])
                # v -> token-major via PE transpose
                for hh in range(2):
                    for tt in range(16):
                        tp = mps.tile([128, 512], BF16, name="tps", tag="qps0")
                        nc.tensor.transpose(
                            tp[:, :128],
                            qkv_sb[:, 4 + hh, tt * 128:tt * 128 + 128],
                            ident[:])
                        nc.vector.tensor_copy(v_sb[:, tt, hh * 128:hh * 128 + 128],
                                              tp[:, :128])
        # ===== phase B: self-attention, block-sparse, pipelined =====
        with ExitStack() as pB:
            ap_ = pB.enter_context(tc.tile_pool(name="ap", bufs=3))
            mp_ = pB.enter_context(tc.tile_pool(name="mp", bufs=2))
            aps = pB.enter_context(tc.tile_pool(name="aps", bufs=2, space="PSUM"))
            accp = pB.enter_context(tc.tile_pool(name="accp", bufs=1, space="PSUM"))
            ctxo = pB.enter_context(tc.tile_pool(name="ctxo", bufs=3))
            for ci, (t0, t1) in enumerate(_chunks(0, S, 512)):
                live = pattern[ci]
                nlv = len(live)
                pss_ = [accp.tile([128, 512], F32, name=f"pssum{h}",
                                  tag=f"pssum{h}") for h in range(2)]
                psc_ = [accp.tile([128, 512], F32, name=f"psctx{h}",
                                  tag=f"psctx{h}") for h in range(2)]
                scs = [None, None]
                prs = [None, None]
                for j in range(nlv + 1):
                    if j < nlv:
                        kt, _ = live[j]
                        sc = aps.tile([128, 1024], F32, name="sc", tag="sc")
                        for hh in range(2):
                            nc.tensor.matmul(
                                sc[:, hh * 512:hh * 512 + 512],
                                qkv_sb[:, 2 + hh, kt * 128:kt * 128 + 128],
                                qkv_sb[:, hh, t0:t1], start=True, stop=True)
                        scs[j % 2] = sc
                    if j >= 1:
                        kt, slot = live[j - 1]
                        sc = scs[(j - 1) % 2]
                        pr = ap_.tile([128, 1024], BF16, name="pr", tag="pr")
                        nc.scalar.activation(pr[:], sc[:], EXP, scale=SC)
                        if slot >= 0:
                            mt_ = mp_.tile([128, 1024], BF16, name="mt", tag="mt")
                            nc.sync.dma_start(mt_[:], maskm.ap()[:, slot, :])
                            nc.vector.tensor_mul(pr[:], pr[:], mt_[:])
                        for hh in range(2):
                            ph = pr[:, hh * 512:hh * 512 + 512]
                            nc.tensor.matmul(pss_[hh][:], ones_bf[:], ph,
                                             start=(j == 1), stop=(j == nlv))
                            nc.tensor.matmul(
                                psc_[hh][:],
                                v_sb[:, kt, hh * 128:hh * 128 + 128],
                                ph, start=(j == 1), stop=(j == nlv))
                ctxb = ctxo.tile([128, 2, 512], BF16, name="ctxb", tag="ctxb")
                for hh in range(2):
                    rc = ap_.tile([128, 512], F32, name="rc", tag="rc")
                    nc.vector.reciprocal(rc[:], pss_[hh][:])
                    nc.vector.tensor_mul(ctxb[:, hh, :], psc_[hh][:], rc[:])
                for hf in range(2):
                    jb = (t0 + hf * 256) // 256
                    dst = ctx_bnc[jb * 256:(jb + 1) * 256, :]
                    nc.sync.dma_start(
                        dst.rearrange("(c p) n -> p c n", p=128),
                        ctxb[:, :, hf * 256:hf * 256 + 256])
        pAB.close()
        nc.gpsimd.collective_compute(
            "AllToAll", mybir.AluOpType.bypass,
            replica_groups=[list(range(NC_))],
            ins=[ctx_bnc.opt()], outs=[ctx_all.opt()])

        # ===== phase C: token-parallel dense + residual + rmsnorm =====
        hold = top.enter_context(ExitStack())
        hp2 = hold.enter_context(tc.tile_pool(name="hp2", bufs=1))
        h1_sb = hp2.tile([128, 16, 256], F32)
        h1n_sb = hp2.tile([128, 16, 256], BF16)
        h2_sb = hp2.tile([128, 16, 256], F32)
        h2n_sb = hp2.tile([128, 16, 256], BF16)
        with ExitStack() as pC:
            cp = pC.enter_context(tc.tile_pool(name="cp", bufs=1))
            re_sb = cp.tile([128, 16, 256], F32)
            nc.sync.dma_start(re_sb[:], r128(resid.ap()))
            ctx_sb = cp.tile([128, 16, 256], BF16)
            nc.sync.dma_start(ctx_sb[:], r128(ctx_all[:]))
            wdp = pC.enter_context(tc.tile_pool(name="wdp", bufs=3))
            dps = pC.enter_context(tc.tile_pool(name="dps", bufs=2, space="PSUM"))
            for mt in range(16):
                wt = wdp.tile([128, 16, 128], BF16, name="wdt", tag="wdt")
                nc.sync.dma_start(wt[:], r128(wdT.ap()[mt]))
                ps = dps.tile([128, 256], F32, name="dp", tag="dp")
                for kc in range(16):
                    nc.tensor.matmul(ps[:], wt[:, kc, :], ctx_sb[:, kc, :],
                                     start=(kc == 0), stop=(kc == 15))
                nc.vector.tensor_add(h1_sb[:, mt, :], ps[:], re_sb[:, mt, :])
            pss = dps.tile([128, 256], F32, name="np1", tag="dp")
            for kc in range(16):
                sq = scrp.tile([128, 256], F32R, name="sq1", tag="sq1")
                nc.scalar.activation(sq[:], h1_sb[:, kc, :], SQ)
                nc.tensor.matmul(pss[:], ones_sb[:], sq[:],
                                 start=(kc == 0), stop=(kc == 15))
            rms = scrp.tile([128, 256], F32, name="rms1", tag="rms1")
            nc.scalar.activation(rms[:], pss[:], SQRT, scale=1.0 / H,
                                 bias=eps_sb[:])
            rinv = cp.tile([128, 256], F32)
            nc.vector.reciprocal(rinv[:], rms[:])
            for kc in range(16):
                nc.vector.tensor_mul(h1n_sb[:, kc, :], h1_sb[:, kc, :], rinv[:])

            # cq projection (token-parallel): [CC, 256]
            cqp = pC.enter_context(tc.tile_pool(name="cqp", bufs=1))
            cq_sb = cqp.tile([128, 8, 256], BF16)
            for mt in range(8):
                wt = wdp.tile([128, 16, 128], BF16, name="wcqt", tag="wdt")
                nc.sync.dma_start(wt[:], r128(wcqT.ap()[mt]))
                ps = dps.tile([128, 256], F32, name="cqp", tag="dp")
                for kc in range(16):
                    nc.tensor.matmul(ps[:], wt[:, kc, :], h1n_sb[:, kc, :],
                                     start=(kc == 0), stop=(kc == 15))
                nc.vector.tensor_copy(cq_sb[:, mt, :], ps[:])

            # ===== phase D: cross attention (16 heads, E keys) =====
            with ExitStack() as pD:
                kp = pD.enter_context(tc.tile_pool(name="kp", bufs=1))
                k_sb = kp.tile([128, 8, E], BF16)
                v2_sb = kp.tile([128, 16, 16, 65], BF16)
                for r in range(NC_):
                    blk = kv_all[r * 2 * CC:(r + 1) * 2 * CC, :]
                    nc.sync.dma_start(k_sb[:, :, r * 256:r * 256 + 256],
                                      r128(blk[0:CC, :]))
                    vblk = blk[CC:2 * CC, :].rearrange("(p x) n -> p (x n)",
                                                       p=256)
                    for i2 in range(2):
                        nc.sync.dma_start(
                            v2_sb[:, 2 * r + i2, :, 0:64],
                            vblk[i2 * 128:i2 * 128 + 128, :].rearrange(
                                "p (h d) -> p h d", h=16))
                nc.vector.memset(v2_sb[:, :, :, 64:65], 1.0)
                cap = pD.enter_context(tc.tile_pool(name="cap", bufs=3))
                caps = pD.enter_context(tc.tile_pool(name="caps", bufs=2,
                                                     space="PSUM"))
                cacc = pD.enter_context(tc.tile_pool(name="cacc", bufs=2,
                                                     space="PSUM"))
                cctx_sb = cqp.tile([128, 8, 256], BF16)
                for h in range(NH):
                    kch, koff = h // 2, 64 * (h % 2)
                    psctx = cacc.tile([65, 256], F32, name="cpc", tag="cpc")
                    scs2 = [None, None]
                    for j in range(9):
                        if j < 8:
                            sc = caps.tile([128, 512], F32, name="csc",
                                           tag="csc")
                            for q in range(2):
                                kt = 2 * j + q
                                nc.tensor.matmul(
                                    sc[:, q * 256:q * 256 + 256],
                                    k_sb[koff:koff + 64, kch,
                                         kt * 128:kt * 128 + 128],
                                    cq_sb[koff:koff + 64, kch, :],
                                    start=True, stop=True)
                            scs2[j % 2] = sc
                        if j >= 1:
                            sc = scs2[(j - 1) % 2]
                            pr = cap.tile([128, 512], BF16, name="cpr",
                                          tag="cpr")
                            nc.scalar.activation(pr[:], sc[:], EXP, scale=CSC)
                            for q in range(2):
                                kt = 2 * (j - 1) + q
                                nc.tensor.matmul(
                                    psctx[:],
                                    v2_sb[:, kt, h, :],
                                    pr[:, q * 256:q * 256 + 256],
                                    start=(kt == 0), stop=(kt == 15))
                    den_r = cap.tile([1, 256], F32R, name="crd", tag="crd")
                    nc.scalar.copy(den_r[:], psctx[64:65, :])
                    bc = caps.tile([64, 256], F32, name="cbc", tag="cbc")
                    nc.tensor.matmul(bc[:], ones_sb[0:1, 0:64], den_r[:],
                                     start=True, stop=True)
                    rc = cap.tile([64, 256], F32, name="crc", tag="crc")
                    nc.vector.reciprocal(rc[:], bc[:])
                    nc.vector.tensor_mul(cctx_sb[koff:koff + 64, kch, :],
                                         psctx[0:64, :], rc[:])
                # cdense + residual -> h2, rmsnorm -> h2n
                for mt in range(16):
                    wt = wdp.tile([128, 8, 128], BF16, name="wcdt", tag="wcdt")
                    nc.sync.dma_start(wt[:], r128(wcdT.ap()[mt]))
                    ps = dps.tile([128, 256], F32, name="cdp", tag="dp")
                    for kc in range(8):
                        nc.tensor.matmul(ps[:], wt[:, kc, :],
                                         cctx_sb[:, kc, :],
                                         start=(kc == 0), stop=(kc == 7))
                    nc.vector.tensor_add(h2_sb[:, mt, :], ps[:],
                                         h1_sb[:, mt, :])
                pss2 = dps.tile([128, 256], F32, name="np2", tag="dp")
                for kc in range(16):
                    sq = scrp.tile([128, 256], F32R, name="sq2", tag="sq1")
                    nc.scalar.activation(sq[:], h2_sb[:, kc, :], SQ)
                    nc.tensor.matmul(pss2[:], ones_sb[:], sq[:],
                                     start=(kc == 0), stop=(kc == 15))
                rms2 = scrp.tile([128, 256], F32, name="rms2", tag="rms1")
                nc.scalar.activation(rms2[:], pss2[:], SQRT, scale=1.0 / H,
                                     bias=eps_sb[:])
                rinv2 = cp.tile([128, 256], F32)
                nc.vector.reciprocal(rinv2[:], rms2[:])
                for kc in range(16):
                    nc.vector.tensor_mul(h2n_sb[:, kc, :], h2_sb[:, kc, :],
                                         rinv2[:])
        # ===== phase F: token-parallel MLP (one expert, full I) =====
        with ExitStack() as pF:
            fw = pF.enter_context(tc.tile_pool(name="fw", bufs=3))
            fps = pF.enter_context(tc.tile_pool(name="fps", bufs=2, space="PSUM"))
            fpd = pF.enter_context(tc.tile_pool(name="fpd", bufs=2, space="PSUM"))
            fac = pF.enter_context(tc.tile_pool(name="fac", bufs=1))
            fout = pF.enter_context(tc.tile_pool(name="fout", bufs=3))
            act = fac.tile([128, NPI, 256], BF16)
            for pi in range(NPI):
                gwt = fw.tile([128, 16, 256], BF16, name="guw", tag="guw")
                nc.sync.dma_start(gwt[:], r128(wguI.ap()[pi]))
                pg = fps.tile([128, 512], F32, name="pg", tag="pg")
                for kc in range(16):
                    nc.tensor.matmul(pg[:, 0:256], gwt[:, kc, 0:128],
                                     h2n_sb[:, kc, :],
                                     start=(kc == 0), stop=(kc == 15))
                for kc in range(16):
                    nc.tensor.matmul(pg[:, 256:512], gwt[:, kc, 128:256],
                                     h2n_sb[:, kc, :],
                                     start=(kc == 0), stop=(kc == 15))
                gs = scrp.tile([128, 256], F32, name="gs", tag="gs")
                nc.scalar.activation(gs[:], pg[:, 0:256], SILU)
                nc.vector.tensor_mul(act[:, pi, :], gs[:], pg[:, 256:512])
            for mt in range(16):
                dwt = fw.tile([128, NPI, 128], BF16, name="dnw", tag="dnw")
                nc.sync.dma_start(dwt[:], r128(wdnT.ap()[mt]))
                pd = fpd.tile([128, 256], F32, name="pd", tag="pd")
                for kc in range(NPI):
                    nc.tensor.matmul(pd[:], dwt[:, kc, :], act[:, kc, :],
                                     start=(kc == 0), stop=(kc == NPI - 1))
                ot = fout.tile([128, 256], F32, name="fot", tag="fot")
                nc.vector.tensor_add(ot[:], pd[:], h2_sb[:, mt, :])
                nc.sync.dma_start(y.ap()[mt * 128:mt * 128 + 128, :], ot[:])
        hold.close()
    nc.compile()
    return nc


_CACHE = {}


def _prep_common(inputs):
    import ml_dtypes
    vm = np.asarray(inputs["vision_token_ids"]).astype(bool)
    lm = np.asarray(inputs["language_token_ids"]).astype(bool)
    g0 = np.where(vm & ~lm)[0]; g1 = np.where(vm & lm)[0]
    g2 = np.where(~vm & lm)[0]; g3 = np.where(~vm & ~lm)[0]
    perm = np.concatenate([g0, g1, g2, g3])
    b0 = len(g0); b1 = b0 + len(g1); b2 = b1 + len(g2)
    return perm, b0, b1, b2


def kernel(**inputs):
    import ml_dtypes
    perm, b0, b1, b2 = _prep_common(inputs)
    fast = (b0 == b1) and (b2 == S) and (b1 % 256 == 0)
    if not fast:
        return _kernel_general(inputs, perm, b0, b1, b2)

    f32 = lambda x: np.ascontiguousarray(np.asarray(x, np.float32))
    bf = lambda x: np.ascontiguousarray(np.asarray(x).astype(ml_dtypes.bfloat16))
    pos = np.asarray(inputs["positions"]).astype(np.float32)
    half = HD // 2
    inv_freq = 1.0 / (ROPE_BASE ** (np.arange(half, dtype=np.float32) / half))
    fr = pos[:, None] * inv_freq[None, :]
    cos2 = np.concatenate([np.cos(fr)] * 2, 1).T[:, perm]
    sin2 = np.concatenate([np.sin(fr)] * 2, 1).T[:, perm]
    rot = np.zeros((HD, HD), np.float32)
    rot[np.arange(half), np.arange(half) + half] = -1.0
    rot[np.arange(half) + half, np.arange(half)] = 1.0
    op = np.asarray(inputs["positions"])[perm]

    # block-sparsity pattern + multiplicative masks for partial tiles
    vis = op[None, :] >= op[:, None]          # [key, query]
    pattern = []
    slots = []
    for ci, (t0, t1) in enumerate(_chunks(0, S, 512)):
        lst = []
        for kt in range(16):
            blk = vis[kt * 128:kt * 128 + 128, t0:t1]
            if not blk.any():
                continue
            if blk.all():
                lst.append((kt, -1))
            else:
                lst.append((kt, len(slots)))
                slots.append(blk)
        pattern.append(tuple(lst))
    pattern = tuple(pattern)
    nslot = len(slots)
    if nslot:
        mk = np.stack(slots).astype(np.float32)      # [n, 128, 512]
        maskm = np.concatenate([mk, mk], axis=2)     # [n, 128, 1024]
        maskm = bf(maskm.transpose(1, 0, 2))         # [128, n, 1024]
    else:
        maskm = np.zeros((128, 1, 1024), ml_dtypes.bfloat16)

    wln_in = f32(inputs["w_ln_in"])[:, None]
    wln_pa = f32(inputs["w_ln_post_attn"])[:, None]
    wln_pc = f32(inputs["w_ln_post_cross"])[:, None]
    wqkv = [f32(inputs["w_vis_qkv"]) * wln_in, f32(inputs["w_lang_qkv"]) * wln_in]
    wd = [f32(inputs["w_vis_dense"]), f32(inputs["w_lang_dense"])]
    wgu = [f32(inputs["w_vis_gate_up"]) * wln_pc,
           f32(inputs["w_lang_gate_up"]) * wln_pc]
    wdn = [f32(inputs["w_vis_down"]), f32(inputs["w_lang_down"])]
    wkvf = f32(inputs["w_cross_kv"])
    hTp = f32(inputs["hidden_states"]).T[:, perm].copy()
    encT = bf(f32(inputs["encoder_embeds"]).T)

    def mtblocks(w, nb):  # w [K, nb*128] -> [nb, K, 128]
        return np.ascontiguousarray(
            w.reshape(w.shape[0], nb, 128).transpose(1, 0, 2))

    # per-expert big weights (shared across cores of the same expert)
    wdT_e = [bf(mtblocks(wd[e], 16)) for e in range(2)]
    wguI_e = []
    for e in range(2):
        g = wgu[e][:, :I].reshape(H, NPI, 128)
        u = wgu[e][:, I:].reshape(H, NPI, 128)
        blk = np.concatenate([g, u], axis=2)         # [H, NPI, 256]
        wguI_e.append(bf(np.ascontiguousarray(blk.transpose(1, 0, 2))))
    wdnT_e = [bf(mtblocks(wdn[e], 16)) for e in range(2)]
    wcqT = bf(mtblocks(f32(inputs["w_cross_q"]) * wln_pa, 8))
    wcdT = bf(mtblocks(f32(inputs["w_cross_dense"]), 16))
    wkb = bf(wkvf[:, :CC]); wvb = bf(wkvf[:, CC:])
    hTb = bf(hTp)
    cos2b = bf(cos2); sin2b = bf(sin2); rotb = bf(rot.T)
    onesr = np.ones((128, 128), np.float32)
    onesb = np.ones((128, 128), ml_dtypes.bfloat16)

    key = (b0, b1, b2, pattern)
    if _CACHE.get("key") != key:
        _CACHE.clear()
        _CACHE["key"] = key
        _CACHE["nc"] = build_fast(b0, b1, b2, pattern, nslot)
    nc = _CACHE["nc"]

    in_maps = []
    for c in range(NC_):
        qs = slice(256 * c, 256 * c + 256)
        ex = 0 if 256 * (c + 1) <= b1 else 1
        m = dict(
            hT=hTb, resid=hTp[:, qs].copy(),
            wqkv0=bf(np.concatenate([wqkv[0][:, 256 * c:256 * c + 256],
                                     wqkv[0][:, H:][:, qs],
                                     wqkv[0][:, 2 * H:][:, qs]], 1)),
            wqkv1=bf(np.concatenate([wqkv[1][:, qs],
                                     wqkv[1][:, H:][:, qs],
                                     wqkv[1][:, 2 * H:][:, qs]], 1)),
            cos2=cos2b, sin2=sin2b, rotT=rotb,
            onesr=onesr, onesb=onesb, maskm=maskm,
            encsl=np.ascontiguousarray(encT[:, qs]),
            wk=wkb, wvv=wvb,
            wdT=wdT_e[ex], wcqT=wcqT, wcdT=wcdT,
            wguI=wguI_e[ex], wdnT=wdnT_e[ex],
        )
        in_maps.append(m)

    trace = bool(int(os.environ.get("KTRACE", "0")))
    res = run_bass_kernel_spmd(nc, in_maps, core_ids=list(range(NC_)),
                               trace=trace)
    kernel.last_exec_ns = res.exec_time_ns
    out = np.empty((S, H), np.float32)
    for c in range(NC_):
        out[perm[256 * c:256 * c + 256], :] = res.results[c]["y"].T
    return out


# ============ general fallback path (original kernel) ================

def build_general(b0, b1, b2):
    nc = bacc.Bacc("TRN2", target_bir_lowering=False, debug=False,
                   num_devices=NC_)
    din = lambda n, sh, dt: nc.dram_tensor(n, sh, dt, kind="ExternalInput")
    hT = din("hT", [H, S], BF16)
    wqkv0 = din("wqkv0", [H, 768], BF16)
    wqkv1 = din("wqkv1", [H, 768], BF16)
    wd0 = din("wd0", [256, H], F32R)
    wd1 = din("wd1", [256, H], F32R)
    cos2 = din("cos2", [128, S], BF16)
    sin2 = din("sin2", [128, S], BF16)
    rotT = din("rotT", [128, 128], BF16)
    onesr = din("onesr", [128, 128], F32R)
    onesb = din("onesb", [128, 128], BF16)
    zeros = din("zeros", [128, 512], F32R)
    maskneg = din("maskneg", [S, S], BF16)
    resid = din("resid", [H, 256], F32R)
    encT = din("encT", [CH, E], BF16)
    wk = din("wk", [CH, CC], BF16)
    wvv = din("wvv", [CH, CC], BF16)
    wcq = din("wcq", [H, CC], F32R)
    wcd = din("wcd", [CC, H], F32R)
    wgu0 = din("wgu0", [H, 2 * IS], BF16)
    wgu1 = din("wgu1", [H, 2 * IS], BF16)
    wdn0 = din("wdn0", [ISP, H], BF16)
    wdn1 = din("wdn1", [ISP, H], BF16)
    y = nc.dram_tensor("y", [H, S], F32, kind="ExternalOutput")

    SC = 1.0 / float(np.sqrt(HD))
    CSC = 1.0 / float(np.sqrt(CHD))
    EXP = mybir.ActivationFunctionType.Exp
    SQ = mybir.ActivationFunctionType.Square
    SQRT = mybir.ActivationFunctionType.Sqrt
    SILU = mybir.ActivationFunctionType.Silu
    r128 = lambda ap: ap.rearrange("(c p) n -> p c n", p=128)

    with tile.TileContext(nc) as tc, ExitStack() as top:
        const = top.enter_context(tc.tile_pool(name="const", bufs=1))
        ones_sb = const.tile([128, 128], F32R)
        nc.sync.dma_start(ones_sb[:], onesr.ap()[:])
        ones_bf = const.tile([128, 128], BF16)
        nc.sync.dma_start(ones_bf[:], onesb.ap()[:])
        rot_sb = const.tile([128, 128], BF16)
        nc.sync.dma_start(rot_sb[:], rotT.ap()[:])
        from concourse.masks import make_identity
        ident = const.tile([128, 128], BF16)
        make_identity(nc, ident[:])
        cos_sb = const.tile([128, S], BF16)
        nc.sync.dma_start(cos_sb[:], cos2.ap()[:])
        sin_sb = const.tile([128, S], BF16)
        nc.sync.dma_start(sin_sb[:], sin2.ap()[:])
        zer_sb = const.tile([128, 512], F32R)
        nc.sync.dma_start(zer_sb[:], zeros.ap()[:])
        eps_sb = const.tile([128, 1], F32)
        nc.vector.memset(eps_sb[:], EPS)

        dram = top.enter_context(tc.tile_pool(name="dram", bufs=1, space="DRAM"))
        bounce = dram.tile([NC_ * H, 256], F32)
        rs_out = dram.tile([H, 256], F32)
        h2n_bnc = dram.tile([H, 256], BF16)
        h2n_all = dram.tile([NC_ * H, 256], BF16, addr_space="Shared")
        h2out = nc.dram_tensor("h2out", [H, 256], F32, kind="ExternalOutput")

        scrp = top.enter_context(tc.tile_pool(name="scr", bufs=2))

        def vwrite(op, dst, a, bb):
            if DVE_F32R:
                op(dst, a, bb)
            else:
                scr = scrp.tile([dst.shape[0], dst.shape[-1]], F32,
                                name="vscr", tag="vscr")
                op(scr[:], a, bb)
                nc.scalar.copy(dst, scr[:])

        pABC = top.enter_context(ExitStack())
        qkp = pABC.enter_context(tc.tile_pool(name="qkp", bufs=1))
        qkv_sb = qkp.tile([128, 6, S], BF16)      # q0 q1 k0 k1 v0 v1
        v_sb = qkp.tile([128, 16, 256], BF16)     # token-major v
        ctxp = pABC.enter_context(tc.tile_pool(name="ctxp", bufs=1))
        ctx_sb = ctxp.tile([128, 2, S], F32R)

        # ===== phase A: h load + rmsnorm + QKV + rope + vT =====
        with ExitStack() as pA:
            hp = pA.enter_context(tc.tile_pool(name="hp", bufs=1))
            h_sb = hp.tile([128, 16, S], BF16)
            nc.sync.dma_start(h_sb[:], r128(hT.ap()))
            with ExitStack() as pA1:
                nrm = pA1.enter_context(tc.tile_pool(name="nrm", bufs=2))
                nps = pA1.enter_context(tc.tile_pool(name="nps", bufs=2,
                                                     space="PSUM"))
                for t0, t1 in _chunks(0, S, 512):
                    pss = nps.tile([128, 512], F32, name="pss", tag="pss")
                    for kc in range(16):
                        sq = nrm.tile([128, 512], F32R, name="sq", tag="sq")
                        nc.scalar.activation(sq[:], h_sb[:, kc, t0:t1], SQ)
                        nc.tensor.matmul(pss[:], ones_sb[:], sq[:],
                                         start=(kc == 0), stop=(kc == 15))
                    rms = nrm.tile([128, 512], F32, name="rms", tag="rms")
                    nc.scalar.activation(rms[:], pss[:], SQRT,
                                         scale=1.0 / H, bias=eps_sb[:])
                    rinv = nrm.tile([128, 512], F32, name="rinv", tag="rinv")
                    nc.vector.reciprocal(rinv[:], rms[:])
                    for kc in range(16):
                        nc.vector.tensor_mul(h_sb[:, kc, t0:t1],
                                             h_sb[:, kc, t0:t1], rinv[:])
            with ExitStack() as pA2:
                wp = pA2.enter_context(tc.tile_pool(name="wp", bufs=3))
                mps = pA2.enter_context(tc.tile_pool(name="mps", bufs=2,
                                                     space="PSUM"))
                for slot in range(6):
                    wts = []
                    for ex, wsrc in ((0, wqkv0), (1, wqkv1)):
                        wt = wp.tile([128, 16, 128], BF16,
                                     name=f"wq{ex}{slot}", tag=f"wq{ex}")
                        nc.sync.dma_start(
                            wt[:], r128(wsrc.ap()[:, slot * 128:slot * 128 + 128]))
                        wts.append(wt)
                    for t0, t1 in _chunks(0, S, 512):
                        sg = [x for x in _segs(t0, t1, b0, b1, b2) if x[2]]
                        if not sg:
                            continue
                        need = sorted({x for _, _, ex in sg for x in ex})
                        pss_ = {}
                        for x in need:
                            ps = mps.tile([128, 512], F32, name=f"qps{x}",
                                          tag=f"qps{x}")
                            for kc in range(16):
                                nc.tensor.matmul(ps[:], wts[x][:, kc, :],
                                                 h_sb[:, kc, t0:t1],
                                                 start=(kc == 0), stop=(kc == 15))
                            pss_[x] = ps
                        for s, e, ex in sg:
                            if len(ex) == 1:
                                nc.vector.tensor_copy(qkv_sb[:, slot, s:e],
                                                      pss_[ex[0]][:, s - t0:e - t0])
                            else:
                                nc.vector.tensor_add(qkv_sb[:, slot, s:e],
                                                     pss_[0][:, s - t0:e - t0],
                                                     pss_[1][:, s - t0:e - t0])
                    if b2 < S:
                        nc.vector.memset(qkv_sb[:, slot, b2:S], 0.0)
                # rope on q,k
                for slot in range(4):
                    for t0, t1 in _chunks(0, S, 512):
                        rp = mps.tile([128, 512], F32, name="rps", tag="qps")
                        nc.tensor.matmul(rp[:], rot_sb[:],
                                         qkv_sb[:, slot, t0:t1],
                                         start=True, stop=True)
                        c1 = scrp.tile([128, 512], F32, name="ropec", tag="ropec")
                        nc.vector.tensor_mul(c1[:], qkv_sb[:, slot, t0:t1],
                                             cos_sb[:, t0:t1])
                        s1 = scrp.tile([128, 512], F32, name="ropes", tag="ropes")
                        nc.vector.tensor_mul(s1[:], rp[:], sin_sb[:, t0:t1])
                        nc.vector.tensor_add(qkv_sb[:, slot, t0:t1],
                                             c1[:], s1[:])
                # v -> token-major via PE transpose
                for hh in range(2):
                    for tt in range(16):
                        tp = mps.tile([128, 512], BF16, name="tps", tag="qps")
                        nc.tensor.transpose(
                            tp[:, :128],
                            qkv_sb[:, 4 + hh, tt * 128:tt * 128 + 128],
                            ident[:])
                        nc.vector.tensor_copy(v_sb[:, tt, hh * 128:hh * 128 + 128],
                                              tp[:, :128])
        # ===== phase B: self-attention (perm order) =====
        with ExitStack() as pB:
            ap_ = pB.enter_context(tc.tile_pool(name="ap", bufs=3))
            aps = pB.enter_context(tc.tile_pool(name="aps", bufs=2, space="PSUM"))
            accp = pB.enter_context(tc.tile_pool(name="accp", bufs=1, space="PSUM"))
            for t0, t1 in _chunks(0, S, 512):
                pss_ = [accp.tile([128, 512], F32, name=f"pssum{h}", tag=f"pssum{h}")
                        for h in range(2)]
                psc_ = [accp.tile([128, 512], F32, name=f"psctx{h}", tag=f"psctx{h}")
                        for h in range(2)]
                for kt in range(16):
                    mt_ = ap_.tile([128, 512], BF16, name="mt", tag="mt")
                    nc.sync.dma_start(
                        mt_[:], maskneg.ap()[kt * 128:kt * 128 + 128, t0:t1])
                    for hh in range(2):
                        sc = aps.tile([128, 512], F32, name="sc", tag="sc")
                        nc.tensor.matmul(
                            sc[:], qkv_sb[:, 2 + hh, kt * 128:kt * 128 + 128],
                            qkv_sb[:, hh, t0:t1], start=True, stop=True)
                        nc.vector.tensor_add(sc[:], sc[:], mt_[:])
                        pr = ap_.tile([128, 512], BF16, name="pr", tag="pr")
                        nc.scalar.activation(pr[:], sc[:], EXP, scale=SC)
                        nc.tensor.matmul(pss_[hh][:], ones_bf[:], pr[:],
                                         start=(kt == 0), stop=(kt == 15))
                        nc.tensor.matmul(
                            psc_[hh][:], v_sb[:, kt, hh * 128:hh * 128 + 128],
                            pr[:], start=(kt == 0), stop=(kt == 15))
                for hh in range(2):
                    rc = ap_.tile([128, 512], F32, name="rc", tag="rc")
                    nc.vector.reciprocal(rc[:], pss_[hh][:])
                    vwrite(nc.vector.tensor_mul, ctx_sb[:, hh, t0:t1],
                           psc_[hh][:], rc[:])
        # ===== phase C: dense (routed) -> bounce -> RS =====
        with ExitStack() as pC:
            dwp = pC.enter_context(tc.tile_pool(name="dwp", bufs=1))
            dps = pC.enter_context(tc.tile_pool(name="dps", bufs=2, space="PSUM"))
            dop = pC.enter_context(tc.tile_pool(name="dop", bufs=4))
            dwts = []
            for ex, wsrc in ((0, wd0), (1, wd1)):
                dwt = dwp.tile([128, 2, H], F32R, name=f"dw{ex}", tag=f"dw{ex}")
                nc.sync.dma_start(dwt[:], r128(wsrc.ap()))
                dwts.append(dwt)
            for tt in range(8):
                t0, t1 = tt * 256, tt * 256 + 256
                sg = _segs(t0, t1, b0, b1, b2)
                live = [x for x in sg if x[2]]
                for mt in range(16):
                    ot = dop.tile([128, 256], F32, name="dot", tag="dot")
                    if live:
                        need = sorted({x for _, _, ex in live for x in ex})
                        pss_ = {}
                        for x in need:
                            ps = dps.tile([128, 256], F32, name=f"dpst{x}",
                                          tag=f"dpst{x}")
                            for kc in range(2):
                                nc.tensor.matmul(
                                    ps[:],
                                    dwts[x][:, kc, mt * 128:mt * 128 + 128],
                                    ctx_sb[:, kc, t0:t1],
                                    start=(kc == 0), stop=(kc == 1))
                            pss_[x] = ps
                        for s, e, ex in sg:
                            if len(ex) == 2:
                                nc.vector.tensor_add(ot[:, s - t0:e - t0],
                                                     pss_[0][:, s - t0:e - t0],
                                                     pss_[1][:, s - t0:e - t0])
                            elif ex:
                                nc.vector.tensor_copy(ot[:, s - t0:e - t0],
                                                      pss_[ex[0]][:, s - t0:e - t0])
                            else:
                                nc.vector.memset(ot[:, s - t0:e - t0], 0.0)
                    else:
                        nc.vector.memset(ot[:], 0.0)
                    nc.sync.dma_start(
                        bounce[tt * H + mt * 128: tt * H + mt * 128 + 128, :],
                        ot[:])
        pABC.close()
        nc.gpsimd.collective_compute(
            "ReduceScatter", mybir.AluOpType.add,
            replica_groups=[list(range(NC_))],
            ins=[bounce.opt()], outs=[rs_out.opt()])

        # ===== phase D: cross attention (token-parallel) =====
        with ExitStack() as pD:
            dp = pD.enter_context(tc.tile_pool(name="dp", bufs=1))
            dps2 = pD.enter_context(tc.tile_pool(name="dps2", bufs=2, space="PSUM"))
            h1_sb = dp.tile([128, 16, 256], F32R)
            cq_sb = dp.tile([128, 8, 256], BF16)
            cctx_sb = dp.tile([128, 8, 256], F32R)
            with ExitStack() as pD1:
                d1 = pD1.enter_context(tc.tile_pool(name="d1", bufs=1))
                rs_sb = d1.tile([128, 16, 256], F32)
                nc.sync.dma_start(rs_sb[:], r128(rs_out[:]))
                re_sb = d1.tile([128, 16, 256], F32R)
                nc.sync.dma_start(re_sb[:], r128(resid.ap()))
                for kc in range(16):
                    vwrite(nc.vector.tensor_add, h1_sb[:, kc, :],
                           rs_sb[:, kc, :], re_sb[:, kc, :].bitcast(F32))
                pss = dps2.tile([128, 256], F32, name="psd", tag="psd")
                for kc in range(16):
                    sq = scrp.tile([128, 256], F32R, name="sqd", tag="sqd")
                    nc.scalar.activation(sq[:], h1_sb[:, kc, :].bitcast(F32), SQ)
                    nc.tensor.matmul(pss[:], ones_sb[:], sq[:],
                                     start=(kc == 0), stop=(kc == 15))
                rms = scrp.tile([128, 256], F32, name="rmsd", tag="rmsd")
                nc.scalar.activation(rms[:], pss[:], SQRT, scale=1.0 / H, bias=eps_sb[:])
                rinv = d1.tile([128, 256], F32)
                nc.vector.reciprocal(rinv[:], rms[:])
                h1n_sb = d1.tile([128, 16, 256], F32R)
                for kc in range(16):
                    vwrite(nc.vector.tensor_mul, h1n_sb[:, kc, :],
                           h1_sb[:, kc, :].bitcast(F32), rinv[:])
                for mt in range(8):
                    wcq_t = d1.tile([128, 16, 128], F32R, name="wcqt", tag="wcqt",
                                    bufs=2)
                    nc.sync.dma_start(
                        wcq_t[:], r128(wcq.ap()[:, mt * 128:mt * 128 + 128]))
                    ps = dps2.tile([128, 256], F32, name="cqp", tag="psd")
                    for kc in range(16):
                        nc.tensor.matmul(ps[:],
                                         wcq_t[:, kc, :],
                                         h1n_sb[:, kc, :],
                                         start=(kc == 0), stop=(kc == 15))
                    nc.vector.tensor_copy(cq_sb[:, mt, :], ps[:])
            with ExitStack() as pD2:
                kp = pD2.enter_context(tc.tile_pool(name="kp", bufs=1))
                k_sb = kp.tile([128, 8, E], BF16)
                v_sb2 = kp.tile([128, 16, CC], BF16)
                with ExitStack() as pD2e:
                    ep = pD2e.enter_context(tc.tile_pool(name="ep", bufs=1))
                    enc_sb = ep.tile([128, 8, E], BF16)
                    nc.sync.dma_start(enc_sb[:], r128(encT.ap()))
                    wk_sb = ep.tile([128, 8, CC], BF16)
                    nc.sync.dma_start(wk_sb[:], r128(wk.ap()))
                    wv_sb = ep.tile([128, 8, CC], BF16)
                    nc.sync.dma_start(wv_sb[:], r128(wvv.ap()))
                    for mt in range(8):
                        for n0, n1 in _chunks(0, E, 512):
                            ps = dps2.tile([128, 512], F32, name="kps", tag="kps")
                            for kc in range(8):
                                nc.tensor.matmul(
                                    ps[:], wk_sb[:, kc, mt * 128:mt * 128 + 128],
                                    enc_sb[:, kc, n0:n1],
                                    start=(kc == 0), stop=(kc == 7))
                            nc.vector.tensor_copy(k_sb[:, mt, n0:n1], ps[:])
                    for tt in range(16):
                        for n0, n1 in _chunks(0, CC, 512):
                            ps = dps2.tile([128, 512], F32, name="vps", tag="kps")
                            for kc in range(8):
                                nc.tensor.matmul(
                                    ps[:], enc_sb[:, kc, tt * 128:tt * 128 + 128],
                                    wv_sb[:, kc, n0:n1],
                                    start=(kc == 0), stop=(kc == 7))
                            nc.vector.tensor_copy(v_sb2[:, tt, n0:n1], ps[:])
                with ExitStack() as pD3:
                    cap = pD3.enter_context(tc.tile_pool(name="cap", bufs=3))
                    caps = pD3.enter_context(tc.tile_pool(name="caps", bufs=2,
                                                          space="PSUM"))
                    cacc = pD3.enter_context(tc.tile_pool(name="cacc", bufs=1,
                                                          space="PSUM"))
                    for h in range(NH):
                        kch, koff = h // 2, 64 * (h % 2)
                        pssum = cacc.tile([128, 256], F32, name="cps", tag="cps")
                        psctx = cacc.tile([64, 256], F32, name="cpc", tag="cpc")
                        for kt in range(16):
                            sc = caps.tile([128, 256], F32, name="csc", tag="csc")
                            nc.tensor.matmul(
                                sc[:],
                                k_sb[koff:koff + 64, kch, kt * 128:kt * 128 + 128],
                                cq_sb[koff:koff + 64, kch, :],
                                start=True, stop=True)
                            pr = cap.tile([128, 256], BF16, name="cpr", tag="cpr")
                            nc.scalar.activation(pr[:], sc[:], EXP, scale=CSC)
                            nc.tensor.matmul(pssum[:], ones_bf[:], pr[:],
                                             start=(kt == 0), stop=(kt == 15))
                            nc.tensor.matmul(psctx[:],
                                             v_sb2[:, kt, 64 * h:64 * h + 64],
                                             pr[:], start=(kt == 0), stop=(kt == 15))
                        rc = cap.tile([64, 256], F32, name="crc", tag="crc")
                        nc.vector.reciprocal(rc[:], pssum[:64, :])
                        vwrite(nc.vector.tensor_mul,
                               cctx_sb[koff:koff + 64, kch, :], psctx[:], rc[:])
            # cdense + residual -> h2, rmsnorm -> h2n -> AG
            with ExitStack() as pD4:
                d4 = pD4.enter_context(tc.tile_pool(name="d4", bufs=1))
                h2_sb = d4.tile([128, 16, 256], F32)
                h2n_sb = d4.tile([128, 16, 256], BF16)
                wcd_sb = d4.tile([128, 8, H], F32R)
                nc.sync.dma_start(wcd_sb[:], r128(wcd.ap()))
                for mt in range(16):
                    ps = dps2.tile([128, 256], F32, name="cdp", tag="psd")
                    for kc in range(8):
                        nc.tensor.matmul(ps[:],
                                         wcd_sb[:, kc, mt * 128:mt * 128 + 128],
                                         cctx_sb[:, kc, :],
                                         start=(kc == 0), stop=(kc == 7))
                    nc.vector.tensor_add(h2_sb[:, mt, :], ps[:],
                                         h1_sb[:, mt, :].bitcast(F32))
                pss2 = dps2.tile([128, 256], F32, name="psd2", tag="psd")
                for kc in range(16):
                    sq = scrp.tile([128, 256], F32R, name="sqd2", tag="sqd")
                    nc.scalar.activation(sq[:], h2_sb[:, kc, :], SQ)
                    nc.tensor.matmul(pss2[:], ones_sb[:], sq[:],
                                     start=(kc == 0), stop=(kc == 15))
                rms2 = scrp.tile([128, 256], F32, name="rmsd2", tag="rmsd")
                nc.scalar.activation(rms2[:], pss2[:], SQRT,
                                     scale=1.0 / H, bias=eps_sb[:])
                rinv2 = d4.tile([128, 256], F32)
                nc.vector.reciprocal(rinv2[:], rms2[:])
                for kc in range(16):
                    nc.vector.tensor_mul(h2n_sb[:, kc, :],
                                         h2_sb[:, kc, :], rinv2[:])
                nc.sync.dma_start(r128(h2n_bnc[:]), h2n_sb[:])
                nc.sync.dma_start(r128(h2out.ap()), h2_sb[:])
            nc.gpsimd.collective_compute(
                "AllGather", mybir.AluOpType.bypass,
                replica_groups=[list(range(NC_))],
                ins=[h2n_bnc.opt()], outs=[h2n_all.opt()])
        # ===== phase F: MLP (routed by expert ranges, bf16) =====
        with ExitStack() as pF:
            fp = pF.enter_context(tc.tile_pool(name="fp", bufs=1))
            hn_sb = fp.tile([128, 16, S], BF16)
            for r in range(NC_):
                nc.sync.dma_start(hn_sb[:, :, r * 256:r * 256 + 256],
                                  r128(h2n_all[r * H:(r + 1) * H, :]))
            fw = pF.enter_context(tc.tile_pool(name="fw", bufs=1))
            fps = pF.enter_context(tc.tile_pool(name="fps", bufs=1, space="PSUM"))
            fpd = pF.enter_context(tc.tile_pool(name="fpd", bufs=2, space="PSUM"))
            fac = pF.enter_context(tc.tile_pool(name="fac", bufs=2))
            fout = pF.enter_context(tc.tile_pool(name="fout", bufs=4))
            for ex, (lo, hi) in ((0, (0, b1)), (1, (b1, S))):
                gsrc = (wgu0, wgu1)[ex]
                dsrc = (wdn0, wdn1)[ex]
                dn_t = fw.tile([128, 6, H], BF16, name=f"dn{ex}", tag="dn")
                nc.sync.dma_start(dn_t[:], r128(dsrc.ap()))
                gwts = []
                for pi in range(6):
                    gw = 128 if pi < 5 else 48
                    gwt = fw.tile([128, 16, 256], BF16,
                                  name=f"guw{ex}{pi}", tag=f"guw{pi}")
                    nc.sync.dma_start(
                        gwt[:, :, :2 * gw],
                        r128(gsrc.ap()[:, pi * 256:pi * 256 + 2 * gw]))
                    gwts.append(gwt)
                for a0 in range(0, S, 512):
                    c0, c1 = max(a0, lo), min(a0 + 512, hi)
                    if c0 >= c1:
                        continue
                    t0_, W = a0, 512
                    eo, ew = c0 - a0, c1 - c0
                    act = fac.tile([128, 6, 512], BF16, name="act", tag="act")
                    for pi in range(6):
                        gw = 128 if pi < 5 else 48
                        gwt = gwts[pi]
                        pg = fps.tile([128, 512], F32, name="pg", tag="pg")
                        pu = fps.tile([128, 512], F32, name="pu", tag="pu")
                        for kc in range(16):
                            nc.tensor.matmul(pg[:gw, :W], gwt[:, kc, :gw],
                                             hn_sb[:, kc, t0_:t0_ + 512],
                                             start=(kc == 0), stop=(kc == 15))
                            nc.tensor.matmul(pu[:gw, :W], gwt[:, kc, gw:2 * gw],
                                             hn_sb[:, kc, t0_:t0_ + 512],
                                             start=(kc == 0), stop=(kc == 15))
                        gs = scrp.tile([128, 512], F32, name="gs", tag="gs")
                        nc.scalar.activation(gs[:gw, :W], pg[:gw, :W], SILU)
                        nc.vector.tensor_mul(act[:gw, pi, :W],
                                             gs[:gw, :W], pu[:gw, :W])
                    for mt in range(16):
                        pd = fpd.tile([128, 512], F32, name="pd", tag="pd")
                        for pi in range(6):
                            kw = 128 if pi < 5 else 48
                            nc.tensor.matmul(
                                pd[:, :W],
                                dn_t[:kw, pi, mt * 128:mt * 128 + 128],
                                act[:kw, pi, :W],
                                start=(pi == 0), stop=(pi == 5))
                        ot = fout.tile([128, 512], F32, name="fot", tag="fot")
                        nc.vector.tensor_copy(ot[:, eo:eo + ew], pd[:, eo:eo + ew])
                        nc.sync.dma_start(
                            y.ap()[mt * 128:mt * 128 + 128, c0:c1],
                            ot[:, eo:eo + ew])
    nc.compile()
    return nc


def _kernel_general(inputs, perm, b0, b1, b2):
    import ml_dtypes
    f32 = lambda x: np.ascontiguousarray(np.asarray(x, np.float32))
    bf = lambda x: np.ascontiguousarray(np.asarray(x).astype(ml_dtypes.bfloat16))
    pos = np.asarray(inputs["positions"]).astype(np.float32)
    half = HD // 2
    inv_freq = 1.0 / (ROPE_BASE ** (np.arange(half, dtype=np.float32) / half))
    fr = pos[:, None] * inv_freq[None, :]
    cos2 = np.concatenate([np.cos(fr)] * 2, 1).T[:, perm]
    sin2 = np.concatenate([np.sin(fr)] * 2, 1).T[:, perm]
    rot = np.zeros((HD, HD), np.float32)
    rot[np.arange(half), np.arange(half) + half] = -1.0
    rot[np.arange(half) + half, np.arange(half)] = 1.0
    op = np.asarray(inputs["positions"])[perm]
    maskneg = np.where(op[None, :] >= op[:, None], 0.0, -30000.0)

    wln_in = f32(inputs["w_ln_in"])[:, None]
    wln_pa = f32(inputs["w_ln_post_attn"])[:, None]
    wln_pc = f32(inputs["w_ln_post_cross"])[:, None]
    wqkv = [f32(inputs["w_vis_qkv"]) * wln_in, f32(inputs["w_lang_qkv"]) * wln_in]
    wd = [f32(inputs["w_vis_dense"]), f32(inputs["w_lang_dense"])]
    wgu = [f32(inputs["w_vis_gate_up"]) * wln_pc,
           f32(inputs["w_lang_gate_up"]) * wln_pc]
    wdn = [f32(inputs["w_vis_down"]), f32(inputs["w_lang_down"])]
    wkvf = f32(inputs["w_cross_kv"])
    hTp = f32(inputs["hidden_states"]).T[:, perm].copy()

    def interleave(w):  # w [H, 2*IS] = [gate | up]
        cols = []
        for i in range(5):
            cols.append(w[:, 128 * i:128 * i + 128])
            cols.append(w[:, IS + 128 * i:IS + 128 * i + 128])
        cols.append(w[:, 640:IS]); cols.append(w[:, IS + 640:2 * IS])
        return np.ascontiguousarray(np.concatenate(cols, 1))

    key = ("general", b0, b1, b2)
    if _CACHE.get("key") != key:
        _CACHE.clear()
        _CACHE["key"] = key
        _CACHE["nc"] = build_general(b0, b1, b2)
    nc = _CACHE["nc"]

    in_maps = []
    for c in range(NC_):
        qs = slice(256 * c, 256 * c + 256)
        m = dict(
            hT=bf(hTp),
            wqkv0=bf(np.concatenate([wqkv[0][:, qs], wqkv[0][:, H:][:, qs],
                                     wqkv[0][:, 2 * H:][:, qs]], 1)),
            wqkv1=bf(np.concatenate([wqkv[1][:, qs], wqkv[1][:, H:][:, qs],
                                     wqkv[1][:, 2 * H:][:, qs]], 1)),
            wd0=wd[0][qs].copy(), wd1=wd[1][qs].copy(),
            cos2=bf(cos2), sin2=bf(sin2), rotT=bf(rot.T),
            onesr=np.ones((128, 128), np.float32),
            onesb=np.ones((128, 128), ml_dtypes.bfloat16),
            zeros=np.zeros((128, 512), np.float32),
            maskneg=bf(maskneg), resid=hTp[:, qs].copy(),
            encT=bf(f32(inputs["encoder_embeds"]).T),
            wk=bf(wkvf[:, :CC]), wvv=bf(wkvf[:, CC:]),
            wcq=(f32(inputs["w_cross_q"]) * wln_pa).copy(),
            wcd=f32(inputs["w_cross_dense"]),
            wgu0=bf(interleave(np.concatenate(
                [wgu[0][:, IS * c:IS * c + IS],
                 wgu[0][:, I + IS * c:I + IS * c + IS]], 1))),
            wgu1=bf(interleave(np.concatenate(
                [wgu[1][:, IS * c:IS * c + IS],
                 wgu[1][:, I + IS * c:I + IS * c + IS]], 1))),
            wdn0=bf(np.concatenate([wdn[0][IS * c:IS * c + IS],
                                    np.zeros((ISP - IS, H), np.float32)], 0)),
            wdn1=bf(np.concatenate([wdn[1][IS * c:IS * c + IS],
                                    np.zeros((ISP - IS, H), np.float32)], 0)),
        )
        in_maps.append(m)

    trace = bool(int(os.environ.get("KTRACE", "0")))
    res = run_bass_kernel_spmd(nc, in_maps, core_ids=list(range(NC_)),
                               trace=trace)
    kernel.last_exec_ns = res.exec_time_ns
    tot = res.results[0]["y"].astype(np.float64)
    for c in range(1, NC_):
        tot += res.results[c]["y"]
    for c in range(NC_):
        tot[:, 256 * c:256 * c + 256] += res.results[c]["h2out"]
    out = np.empty((S, H), np.float32)
    out[perm, :] = tot.T.astype(np.float32)
    return out


# revision 15
# speedup vs baseline: 1.5567x; 1.0283x over previous
"""Trainium2 Bass kernel for nn_CogAgentDecoderLayer (8-core SPMD).

Fast path (disjoint vis/lang masks, expert boundary % 256 == 0):
feature-major activations [feat, tok] in permuted token order.
Self-attn head-parallel (2 heads/core, block-sparse causal), then an
AllToAll re-shards ctx token-parallel (256 tok/core); dense, cross-attn
and MLP all run token-parallel with full weights streamed from HBM
(each core uses one expert's weights, chosen host-side). Cross-attn KV
is computed enc-token-sharded and AllGathered early (overlapped with
self-attention). No post-MLP collective: each core emits its final
[H, 256] f32 output slice. bf16 everywhere except residual trunk (f32).

General fallback (any masks): original head/intermediate-parallel
kernel with ReduceScatter + AllGather.
"""
import os
import numpy as np
from contextlib import ExitStack
from concourse import bacc, tile, mybir
from concourse.bass_utils import run_bass_kernel_spmd

NC_ = 8
S, E, H, NH, HD = 2048, 2048, 2048, 16, 128
CH, CC, CHD = 1024, 1024, 64
I = 5504
IS = I // NC_          # 688
ISP = 768              # padded to 6*128 (general path)
NPI = I // 128         # 43 (fast path)
EPS = 1e-5
ROPE_BASE = 10000.0
F32 = mybir.dt.float32
F32R = mybir.dt.float32r
BF16 = mybir.dt.bfloat16
DVE_F32R = True        # DVE may write fp32r tiles directly


def _segs(lo, hi, b0, b1, b2):
    pts = sorted({lo, hi, *[b for b in (b0, b1, b2) if lo < b < hi]})
    out = []
    for s, e in zip(pts, pts[1:]):
        ex = []
        if s < b1:
            ex.append(0)
        if b0 <= s < b2:
            ex.append(1)
        out.append((s, e, ex))
    return out


def _chunks(lo, hi, w):
    out = []
    while lo < hi:
        out.append((lo, min(lo + w, hi)))
        lo += w
    return out


def build_fast(b0, b1, b2, pattern, nslot):
    """pattern: per 512-chunk tuple of (kt, slot) with slot=-1 for
    fully-visible key tiles, else index into maskm."""
    nc = bacc.Bacc("TRN2", target_bir_lowering=False, debug=False,
                   num_devices=NC_)
    din = lambda n, sh, dt: nc.dram_tensor(n, sh, dt, kind="ExternalInput")
    hT = din("hT", [H, S], BF16)
    resid = din("resid", [H, 256], F32)
    wqkv0 = din("wqkv0", [H, 768], BF16)
    wqkv1 = din("wqkv1", [H, 768], BF16)
    cos2 = din("cos2", [128, S], BF16)
    sin2 = din("sin2", [128, S], BF16)
    rotT = din("rotT", [128, 128], BF16)
    onesr = din("onesr", [128, 128], F32R)
    onesb = din("onesb", [128, 128], BF16)
    maskm = din("maskm", [128, max(nslot, 1), 512], BF16)
    encsl = din("encsl", [CH, 256], BF16)
    wk = din("wk", [CH, CC], BF16)
    wvv = din("wvv", [CH, CC], BF16)
    wdT = din("wdT", [16, H, 128], BF16)
    wcqT = din("wcqT", [8, H, 128], BF16)
    wcdT = din("wcdT", [16, CC, 128], BF16)
    wguI = din("wguI", [NPI, H, 256], BF16)
    wdnT = din("wdnT", [16, I, 128], BF16)
    y = nc.dram_tensor("y", [H, 256], F32, kind="ExternalOutput")

    SC = 1.0 / float(np.sqrt(HD))
    CSC = 1.0 / float(np.sqrt(CHD))
    EXP = mybir.ActivationFunctionType.Exp
    SQ = mybir.ActivationFunctionType.Square
    SQRT = mybir.ActivationFunctionType.Sqrt
    SILU = mybir.ActivationFunctionType.Silu
    r128 = lambda ap: ap.rearrange("(c p) n -> p c n", p=128)

    with tile.TileContext(nc) as tc, ExitStack() as top:
        const = top.enter_context(tc.tile_pool(name="const", bufs=1))
        ones_sb = const.tile([128, 128], F32R)
        nc.sync.dma_start(ones_sb[:], onesr.ap()[:])
        ones_bf = const.tile([128, 128], BF16)
        nc.sync.dma_start(ones_bf[:], onesb.ap()[:])
        rot_sb = const.tile([128, 128], BF16)
        nc.sync.dma_start(rot_sb[:], rotT.ap()[:])
        from concourse.masks import make_identity
        ident = const.tile([128, 128], BF16)
        make_identity(nc, ident[:])
        cos_sb = const.tile([128, S], BF16)
        nc.sync.dma_start(cos_sb[:], cos2.ap()[:])
        sin_sb = const.tile([128, S], BF16)
        nc.sync.dma_start(sin_sb[:], sin2.ap()[:])
        eps_sb = const.tile([128, 1], F32)
        nc.vector.memset(eps_sb[:], EPS)

        dram = top.enter_context(tc.tile_pool(name="dram", bufs=1, space="DRAM"))
        kv_bnc = dram.tile([2 * CC, 256], BF16)
        kv_all = dram.tile([NC_ * 2 * CC, 256], BF16, addr_space="Shared")
        ctx_bnc = dram.tile([NC_ * 256, 256], BF16)
        ctx_all = dram.tile([NC_ * 256, 256], BF16)

        scrp = top.enter_context(tc.tile_pool(name="scr", bufs=2))

        pAB = top.enter_context(ExitStack())
        qkp = pAB.enter_context(tc.tile_pool(name="qkp", bufs=1))
        qkv_sb = qkp.tile([128, 6, S], BF16)      # q0 q1 k0 k1 v0 v1
        v_sb = qkp.tile([128, 16, 256], BF16)     # token-major v
        mk_sb = qkp.tile([128, max(nslot, 1), 512], BF16)

        # PE warmup while initial DMAs land
        with ExitStack() as pW:
            wps = pW.enter_context(tc.tile_pool(name="wps", bufs=2,
                                                space="PSUM"))
            for i in range(60):
                wp_ = wps.tile([128, 256], F32, name="wrm", tag="wrm")
                nc.tensor.matmul(wp_[:], ones_bf[:], cos_sb[:, 0:256],
                                 start=True, stop=True)

        # ===== phase 0: cross-KV for this core's 256 enc tokens -> AG ====
        hpool = pAB.enter_context(tc.tile_pool(name="hpool", bufs=2))
        htiles = {}

        def h_load(ci):
            t0 = ci * 512
            ht = hpool.tile([128, 16, 512], BF16, name="hc", tag="hc")
            nc.sync.dma_start(ht[:], r128(hT.ap())[:, :, t0:t0 + 512])
            htiles[ci] = ht

        with ExitStack() as p0:
            ep = p0.enter_context(tc.tile_pool(name="ep", bufs=1))
            enc_sb = ep.tile([128, 8, 256], BF16)
            nc.sync.dma_start(enc_sb[:], r128(encsl.ap()))
            wk_sb = ep.tile([128, 8, CC], BF16)
            nc.sync.dma_start(wk_sb[:], r128(wk.ap()))
            wv_sb = ep.tile([128, 8, CC], BF16)
            nc.sync.dma_start(wv_sb[:], r128(wvv.ap()))
            h_load(0)
            h_load(1)
            kps = p0.enter_context(tc.tile_pool(name="kps", bufs=2,
                                                space="PSUM"))
            kout = p0.enter_context(tc.tile_pool(name="kout", bufs=3))
            for mt in range(8):
                ps = kps.tile([128, 256], F32, name="kp", tag="kp")
                for kc in range(8):
                    nc.tensor.matmul(ps[:],
                                     wk_sb[:, kc, mt * 128:mt * 128 + 128],
                                     enc_sb[:, kc, :],
                                     start=(kc == 0), stop=(kc == 7))
                ko = kout.tile([128, 256], BF16, name="ko", tag="ko")
                nc.vector.tensor_copy(ko[:], ps[:])
                nc.sync.dma_start(kv_bnc[mt * 128:mt * 128 + 128, :], ko[:])
            vdst = kv_bnc[CC:2 * CC, :].rearrange("(p x) n -> p (x n)", p=256)
            for et in range(2):
                for hf in range(2):
                    ps = kps.tile([128, 512], F32, name="vp", tag="kp")
                    for kc in range(8):
                        nc.tensor.matmul(
                            ps[:], enc_sb[:, kc, et * 128:et * 128 + 128],
                            wv_sb[:, kc, hf * 512:hf * 512 + 512],
                            start=(kc == 0), stop=(kc == 7))
                    vo = kout.tile([128, 512], BF16, name="vo", tag="ko")
                    nc.vector.tensor_copy(vo[:], ps[:])
                    nc.sync.dma_start(
                        vdst[et * 128:et * 128 + 128,
                             hf * 512:hf * 512 + 512], vo[:])
        nc.gpsimd.collective_compute(
            "AllGather", mybir.AluOpType.bypass,
            replica_groups=[list(range(NC_))],
            ins=[kv_bnc.opt()], outs=[kv_all.opt()])

        # ===== phase A (chunk-major): rmsnorm + QKV + rope + vT =====
        pA = pAB.enter_context(ExitStack())
        wp = pA.enter_context(tc.tile_pool(name="wp", bufs=1))
        wts = {}
        for slot in range(6):
            for ex, wsrc in ((0, wqkv0), (1, wqkv1)):
                wt = wp.tile([128, 16, 128], BF16, name=f"wq{ex}{slot}",
                             tag=f"wq{ex}{slot}")
                nc.sync.dma_start(
                    wt[:], r128(wsrc.ap()[:, slot * 128:slot * 128 + 128]))
                wts[(ex, slot)] = wt
        nc.sync.dma_start(mk_sb[:], maskm.ap()[:])
        nrm = pA.enter_context(tc.tile_pool(name="nrm", bufs=2))
        mps = pA.enter_context(tc.tile_pool(name="mps", bufs=2, space="PSUM"))
        for ci, (t0, t1) in enumerate(_chunks(0, S, 512)):
            ht = htiles.pop(ci)
            if ci + 2 < 4:
                h_load(ci + 2)
            pss = mps.tile([128, 512], F32, name="pss", tag="qpsA")
            for kc in range(16):
                sq = nrm.tile([128, 512], F32R, name="sq", tag="sq")
                nc.scalar.activation(sq[:], ht[:, kc, :], SQ)
                nc.tensor.matmul(pss[:], ones_sb[:], sq[:],
                                 start=(kc == 0), stop=(kc == 15))
            rms = nrm.tile([128, 512], F32, name="rms", tag="rms")
            nc.scalar.activation(rms[:], pss[:], SQRT,
                                 scale=1.0 / H, bias=eps_sb[:])
            rinv = nrm.tile([128, 512], F32, name="rinv", tag="rinv")
            nc.vector.reciprocal(rinv[:], rms[:])
            for kc in range(16):
                nc.vector.tensor_mul(ht[:, kc, :], ht[:, kc, :], rinv[:])
            for slot in range(6):
                sg = [x for x in _segs(t0, t1, b0, b1, b2) if x[2]]
                need = sorted({x for _, _, ex in sg for x in ex})
                pss_ = {}
                for x in need:
                    ps = mps.tile([128, 512], F32, name=f"qps{x}",
                                  tag=f"qps{x % 2}")
                    for kc in range(16):
                        nc.tensor.matmul(ps[:], wts[(x, slot)][:, kc, :],
                                         ht[:, kc, :],
                                         start=(kc == 0), stop=(kc == 15))
                    pss_[x] = ps
                for s, e, ex in sg:
                    if len(ex) == 1:
                        nc.vector.tensor_copy(qkv_sb[:, slot, s:e],
                                              pss_[ex[0]][:, s - t0:e - t0])
                    else:
                        nc.vector.tensor_add(qkv_sb[:, slot, s:e],
                                             pss_[0][:, s - t0:e - t0],
                                             pss_[1][:, s - t0:e - t0])
                if b2 < S:
                    nc.vector.memset(qkv_sb[:, slot, b2:S], 0.0)
            # rope on q,k for this chunk
            for slot in range(4):
                rp = mps.tile([128, 512], F32, name="rps", tag="qpsA")
                nc.tensor.matmul(rp[:], rot_sb[:], qkv_sb[:, slot, t0:t1],
                                 start=True, stop=True)
                c1 = scrp.tile([128, 512], F32, name="ropec", tag="ropec")
                nc.vector.tensor_mul(c1[:], qkv_sb[:, slot, t0:t1],
                                     cos_sb[:, t0:t1])
                s1 = scrp.tile([128, 512], F32, name="ropes", tag="ropes")
                nc.vector.tensor_mul(s1[:], rp[:], sin_sb[:, t0:t1])
                nc.vector.tensor_add(qkv_sb[:, slot, t0:t1], c1[:], s1[:])
            # v -> token-major via PE transpose (this chunk's key tiles)
            for hh in range(2):
                for tt in range(4 * ci, 4 * ci + 4):
                    tp = mps.tile([128, 512], BF16, name="tps", tag="qpsA")
                    nc.tensor.transpose(
                        tp[:, :128],
                        qkv_sb[:, 4 + hh, tt * 128:tt * 128 + 128],
                        ident[:])
                    nc.vector.tensor_copy(v_sb[:, tt, hh * 128:hh * 128 + 128],
                                          tp[:, :128])
        pA.close()
        # ===== phase B: self-attention, block-sparse, 2-deep pipeline ====
        with ExitStack() as pB:
            ap_ = pB.enter_context(tc.tile_pool(name="ap", bufs=4))
            aps = pB.enter_context(tc.tile_pool(name="aps", bufs=3, space="PSUM"))
            accp = pB.enter_context(tc.tile_pool(name="accp", bufs=1, space="PSUM"))
            ctxo = pB.enter_context(tc.tile_pool(name="ctxo", bufs=3))
            for ci, (t0, t1) in enumerate(_chunks(0, S, 512)):
                live = pattern[ci]
                nlv = len(live)
                for hh in range(2):
                    pss = accp.tile([128, 512], F32, name="pssum",
                                    tag=f"pssum{hh}")
                    psc = accp.tile([128, 512], F32, name="psctx",
                                    tag=f"psctx{hh}")
                    scs = [None, None, None]
                    prs = [None, None, None]
                    for j in range(nlv + 2):
                        if j < nlv:
                            kt, _ = live[j]
                            sc = aps.tile([128, 512], F32, name="sc", tag="sc")
                            nc.tensor.matmul(
                                sc[:],
                                qkv_sb[:, 2 + hh, kt * 128:kt * 128 + 128],
                                qkv_sb[:, hh, t0:t1], start=True, stop=True)
                            scs[j % 3] = sc
                        if j >= 1 and j - 1 < nlv:
                            kt, slot = live[j - 1]
                            pr = ap_.tile([128, 512], BF16, name="pr", tag="pr")
                            nc.scalar.activation(pr[:], scs[(j - 1) % 3][:],
                                                 EXP, scale=SC)
                            if slot >= 0:
                                nc.vector.tensor_mul(pr[:], pr[:],
                                                     mk_sb[:, slot, :])
                            prs[(j - 1) % 3] = pr
                        if j >= 2:
                            kt, slot = live[j - 2]
                            pr = prs[(j - 2) % 3]
                            nc.tensor.matmul(pss[:], ones_bf[:], pr[:],
                                             start=(j == 2), stop=(j == nlv + 1))
                            nc.tensor.matmul(
                                psc[:], v_sb[:, kt, hh * 128:hh * 128 + 128],
                                pr[:], start=(j == 2), stop=(j == nlv + 1))
                    ctxb = ctxo.tile([128, 512], BF16, name="ctxb",
                                     tag=f"ctxb{hh}")
                    rc = ap_.tile([128, 512], F32, name="rc", tag="rc")
                    nc.vector.reciprocal(rc[:], pss[:])
                    nc.vector.tensor_mul(ctxb[:], psc[:], rc[:])
                    for hf in range(2):
                        jb = (t0 + hf * 256) // 256
                        dst = ctx_bnc[jb * 256:(jb + 1) * 256, :]
                        nc.sync.dma_start(
                            dst.rearrange("(c p) n -> p c n",
                                          p=128)[:, hh, :],
                            ctxb[:, hf * 256:hf * 256 + 256])
        pAB.close()
        nc.gpsimd.collective_compute(
            "AllToAll", mybir.AluOpType.bypass,
            replica_groups=[list(range(NC_))],
            ins=[ctx_bnc.opt()], outs=[ctx_all.opt()])
        # keep the PE clock warm while the AllToAll flies
        with ExitStack() as pW2:
            wps2 = pW2.enter_context(tc.tile_pool(name="wps2", bufs=2,
                                                  space="PSUM"))
            for i in range(250):
                wp_ = wps2.tile([128, 256], F32, name="wrm2", tag="wrm2")
                nc.tensor.matmul(wp_[:], ones_bf[:], cos_sb[:, 0:256],
                                 start=True, stop=True)

        # ===== phase C: token-parallel dense + residual + rmsnorm =====
        hold = top.enter_context(ExitStack())
        hp2 = hold.enter_context(tc.tile_pool(name="hp2", bufs=1))
        h1_sb = hp2.tile([128, 16, 256], F32)
        h1n_sb = hp2.tile([128, 16, 256], BF16)
        h2_sb = hp2.tile([128, 16, 256], F32)
        h2n_sb = hp2.tile([128, 16, 256], BF16)
        with ExitStack() as pC:
            cp = pC.enter_context(tc.tile_pool(name="cp", bufs=1))
            re_sb = cp.tile([128, 16, 256], F32)
            nc.sync.dma_start(re_sb[:], r128(resid.ap()))
            ctx_sb = cp.tile([128, 16, 256], BF16)
            nc.sync.dma_start(ctx_sb[:], r128(ctx_all[:]))
            wdp = pC.enter_context(tc.tile_pool(name="wdp", bufs=3))
            dps = pC.enter_context(tc.tile_pool(name="dps", bufs=2, space="PSUM"))
            for mt in range(16):
                wt = wdp.tile([128, 16, 128], BF16, name="wdt", tag="wdt")
                nc.sync.dma_start(wt[:], r128(wdT.ap()[mt]))
                ps = dps.tile([128, 256], F32, name="dp", tag="dp")
                for kc in range(16):
                    nc.tensor.matmul(ps[:], wt[:, kc, :], ctx_sb[:, kc, :],
                                     start=(kc == 0), stop=(kc == 15))
                nc.vector.tensor_add(h1_sb[:, mt, :], ps[:], re_sb[:, mt, :])
            pss = dps.tile([128, 256], F32, name="np1", tag="dp")
            for kc in range(16):
                sq = scrp.tile([128, 256], F32R, name="sq1", tag="sq1")
                nc.scalar.activation(sq[:], h1_sb[:, kc, :], SQ)
                nc.tensor.matmul(pss[:], ones_sb[:], sq[:],
                                 start=(kc == 0), stop=(kc == 15))
            rms = scrp.tile([128, 256], F32, name="rms1", tag="rms1")
            nc.scalar.activation(rms[:], pss[:], SQRT, scale=1.0 / H,
                                 bias=eps_sb[:])
            rinv = cp.tile([128, 256], F32)
            nc.vector.reciprocal(rinv[:], rms[:])
            for kc in range(16):
                nc.vector.tensor_mul(h1n_sb[:, kc, :], h1_sb[:, kc, :], rinv[:])

            # cq projection (token-parallel): [CC, 256]
            cqp = pC.enter_context(tc.tile_pool(name="cqp", bufs=1))
            cq_sb = cqp.tile([128, 8, 256], BF16)
            for mt in range(8):
                wt = wdp.tile([128, 16, 128], BF16, name="wcqt", tag="wdt")
                nc.sync.dma_start(wt[:], r128(wcqT.ap()[mt]))
                ps = dps.tile([128, 256], F32, name="cqp", tag="dp")
                for kc in range(16):
                    nc.tensor.matmul(ps[:], wt[:, kc, :], h1n_sb[:, kc, :],
                                     start=(kc == 0), stop=(kc == 15))
                nc.vector.tensor_copy(cq_sb[:, mt, :], ps[:])

            # ===== phase D: cross attention (16 heads, E keys) =====
            with ExitStack() as pD:
                kp = pD.enter_context(tc.tile_pool(name="kp", bufs=1))
                k_sb = kp.tile([128, 8, E], BF16)
                v2_sb = kp.tile([128, 16, 16, 65], BF16)
                for r in range(NC_):
                    blk = kv_all[r * 2 * CC:(r + 1) * 2 * CC, :]
                    nc.sync.dma_start(k_sb[:, :, r * 256:r * 256 + 256],
                                      r128(blk[0:CC, :]))
                    vblk = blk[CC:2 * CC, :].rearrange("(p x) n -> p (x n)",
                                                       p=256)
                    for i2 in range(2):
                        nc.sync.dma_start(
                            v2_sb[:, 2 * r + i2, :, 0:64],
                            vblk[i2 * 128:i2 * 128 + 128, :].rearrange(
                                "p (h d) -> p h d", h=16))
                nc.vector.memset(v2_sb[:, :, :, 64:65], 1.0)
                cap = pD.enter_context(tc.tile_pool(name="cap", bufs=4))
                caps = pD.enter_context(tc.tile_pool(name="caps", bufs=3,
                                                     space="PSUM"))
                cbp = pD.enter_context(tc.tile_pool(name="cbp", bufs=1,
                                                    space="PSUM"))
                cacc = pD.enter_context(tc.tile_pool(name="cacc", bufs=2,
                                                     space="PSUM"))
                cctx_sb = cqp.tile([128, 8, 256], BF16)
                for h in range(NH):
                    kch, koff = h // 2, 64 * (h % 2)
                    psctx = cacc.tile([65, 256], F32, name="cpc", tag="cpc")
                    scs2 = [None] * 3
                    prs2 = [None] * 3
                    for j in range(10):
                        if j < 8:
                            sc = caps.tile([128, 512], F32, name="csc",
                                           tag="csc")
                            for q in range(2):
                                kt = 2 * j + q
                                nc.tensor.matmul(
                                    sc[:, q * 256:q * 256 + 256],
                                    k_sb[koff:koff + 64, kch,
                                         kt * 128:kt * 128 + 128],
                                    cq_sb[koff:koff + 64, kch, :],
                                    start=True, stop=True)
                            scs2[j % 3] = sc
                        if j >= 1 and j - 1 < 8:
                            pr = cap.tile([128, 512], BF16, name="cpr",
                                          tag="cpr")
                            nc.scalar.activation(pr[:], scs2[(j - 1) % 3][:],
                                                 EXP, scale=CSC)
                            prs2[(j - 1) % 3] = pr
                        if j >= 2:
                            pr = prs2[(j - 2) % 3]
                            for q in range(2):
                                kt = 2 * (j - 2) + q
                                nc.tensor.matmul(
                                    psctx[:],
                                    v2_sb[:, kt, h, :],
                                    pr[:, q * 256:q * 256 + 256],
                                    start=(kt == 0), stop=(kt == 15))
                    den_r = cap.tile([1, 256], F32R, name="crd", tag="crd")
                    nc.scalar.copy(den_r[:], psctx[64:65, :])
                    bc = cbp.tile([64, 256], F32, name="cbc", tag="cbc")
                    nc.tensor.matmul(bc[:], ones_sb[0:1, 0:64], den_r[:],
                                     start=True, stop=True)
                    rc = cap.tile([64, 256], F32, name="crc", tag="crc")
                    nc.vector.reciprocal(rc[:], bc[:])
                    nc.vector.tensor_mul(cctx_sb[koff:koff + 64, kch, :],
                                         psctx[0:64, :], rc[:])
                # cdense + residual -> h2, rmsnorm -> h2n
                for mt in range(16):
                    wt = wdp.tile([128, 8, 128], BF16, name="wcdt", tag="wcdt")
                    nc.sync.dma_start(wt[:], r128(wcdT.ap()[mt]))
                    ps = dps.tile([128, 256], F32, name="cdp", tag="dp")
                    for kc in range(8):
                        nc.tensor.matmul(ps[:], wt[:, kc, :],
                                         cctx_sb[:, kc, :],
                                         start=(kc == 0), stop=(kc == 7))
                    nc.vector.tensor_add(h2_sb[:, mt, :], ps[:],
                                         h1_sb[:, mt, :])
                pss2 = dps.tile([128, 256], F32, name="np2", tag="dp")
                for kc in range(16):
                    sq = scrp.tile([128, 256], F32R, name="sq2", tag="sq1")
                    nc.scalar.activation(sq[:], h2_sb[:, kc, :], SQ)
                    nc.tensor.matmul(pss2[:], ones_sb[:], sq[:],
                                     start=(kc == 0), stop=(kc == 15))
                rms2 = scrp.tile([128, 256], F32, name="rms2", tag="rms1")
                nc.scalar.activation(rms2[:], pss2[:], SQRT, scale=1.0 / H,
                                     bias=eps_sb[:])
                rinv2 = cp.tile([128, 256], F32)
                nc.vector.reciprocal(rinv2[:], rms2[:])
                for kc in range(16):
                    nc.vector.tensor_mul(h2n_sb[:, kc, :], h2_sb[:, kc, :],
                                         rinv2[:])
        # ===== phase F: token-parallel MLP (one expert, full I) =====
        with ExitStack() as pF:
            fw = pF.enter_context(tc.tile_pool(name="fw", bufs=3))
            fps = pF.enter_context(tc.tile_pool(name="fps", bufs=2, space="PSUM"))
            fpd = pF.enter_context(tc.tile_pool(name="fpd", bufs=2, space="PSUM"))
            fac = pF.enter_context(tc.tile_pool(name="fac", bufs=1))
            fout = pF.enter_context(tc.tile_pool(name="fout", bufs=3))
            act = fac.tile([128, NPI, 256], BF16)
            for pi in range(NPI):
                gwt = fw.tile([128, 16, 256], BF16, name="guw", tag="guw")
                nc.sync.dma_start(gwt[:], r128(wguI.ap()[pi]))
                pg = fps.tile([128, 512], F32, name="pg", tag="pg")
                for kc in range(16):
                    nc.tensor.matmul(pg[:, 0:256], gwt[:, kc, 0:128],
                                     h2n_sb[:, kc, :],
                                     start=(kc == 0), stop=(kc == 15))
                for kc in range(16):
                    nc.tensor.matmul(pg[:, 256:512], gwt[:, kc, 128:256],
                                     h2n_sb[:, kc, :],
                                     start=(kc == 0), stop=(kc == 15))
                gs = scrp.tile([128, 256], F32, name="gs", tag="gs")
                nc.scalar.activation(gs[:], pg[:, 0:256], SILU)
                nc.vector.tensor_mul(act[:, pi, :], gs[:], pg[:, 256:512])
            for mt in range(16):
                dwt = fw.tile([128, NPI, 128], BF16, name="dnw", tag="dnw")
                nc.sync.dma_start(dwt[:], r128(wdnT.ap()[mt]))
                pd = fpd.tile([128, 256], F32, name="pd", tag="pd")
                for kc in range(NPI):
                    nc.tensor.matmul(pd[:], dwt[:, kc, :], act[:, kc, :],
                                     start=(kc == 0), stop=(kc == NPI - 1))
                ot = fout.tile([128, 256], F32, name="fot", tag="fot")
                nc.vector.tensor_add(ot[:], pd[:], h2_sb[:, mt, :])
                nc.sync.dma_start(y.ap()[mt * 128:mt * 128 + 128, :], ot[:])
        hold.close()
    nc.compile()
    return nc


_CACHE = {}


def _prep_common(inputs):
    import ml_dtypes
    vm = np.asarray(inputs["vision_token_ids"]).astype(bool)
    lm = np.asarray(inputs["language_token_ids"]).astype(bool)
    g0 = np.where(vm & ~lm)[0]; g1 = np.where(vm & lm)[0]
    g2 = np.where(~vm & lm)[0]; g3 = np.where(~vm & ~lm)[0]
    perm = np.concatenate([g0, g1, g2, g3])
    b0 = len(g0); b1 = b0 + len(g1); b2 = b1 + len(g2)
    return perm, b0, b1, b2


def kernel(**inputs):
    import ml_dtypes
    perm, b0, b1, b2 = _prep_common(inputs)
    fast = (b0 == b1) and (b2 == S) and (b1 % 256 == 0)
    if not fast:
        return _kernel_general(inputs, perm, b0, b1, b2)

    f32 = lambda x: np.ascontiguousarray(np.asarray(x, np.float32))
    bf = lambda x: np.ascontiguousarray(np.asarray(x).astype(ml_dtypes.bfloat16))
    pos = np.asarray(inputs["positions"]).astype(np.float32)
    half = HD // 2
    inv_freq = 1.0 / (ROPE_BASE ** (np.arange(half, dtype=np.float32) / half))
    fr = pos[:, None] * inv_freq[None, :]
    cos2 = np.concatenate([np.cos(fr)] * 2, 1).T[:, perm]
    sin2 = np.concatenate([np.sin(fr)] * 2, 1).T[:, perm]
    rot = np.zeros((HD, HD), np.float32)
    rot[np.arange(half), np.arange(half) + half] = -1.0
    rot[np.arange(half) + half, np.arange(half)] = 1.0
    op = np.asarray(inputs["positions"])[perm]

    # block-sparsity pattern + multiplicative masks for partial tiles
    vis = op[None, :] >= op[:, None]          # [key, query]
    pattern = []
    slots = []
    for ci, (t0, t1) in enumerate(_chunks(0, S, 512)):
        lst = []
        for kt in range(16):
            blk = vis[kt * 128:kt * 128 + 128, t0:t1]
            if not blk.any():
                continue
            if blk.all():
                lst.append((kt, -1))
            else:
                lst.append((kt, len(slots)))
                slots.append(blk)
        pattern.append(tuple(lst))
    pattern = tuple(pattern)
    nslot = len(slots)
    if nslot:
        mk = np.stack(slots).astype(np.float32)      # [n, 128, 512]
        maskm = bf(mk.transpose(1, 0, 2))            # [128, n, 512]
    else:
        maskm = np.zeros((128, 1, 512), ml_dtypes.bfloat16)

    wln_in = f32(inputs["w_ln_in"])[:, None]
    wln_pa = f32(inputs["w_ln_post_attn"])[:, None]
    wln_pc = f32(inputs["w_ln_post_cross"])[:, None]
    wqkv = [f32(inputs["w_vis_qkv"]) * wln_in, f32(inputs["w_lang_qkv"]) * wln_in]
    wd = [f32(inputs["w_vis_dense"]), f32(inputs["w_lang_dense"])]
    wgu = [f32(inputs["w_vis_gate_up"]) * wln_pc,
           f32(inputs["w_lang_gate_up"]) * wln_pc]
    wdn = [f32(inputs["w_vis_down"]), f32(inputs["w_lang_down"])]
    wkvf = f32(inputs["w_cross_kv"])
    hTp = f32(inputs["hidden_states"]).T[:, perm].copy()
    encT = bf(f32(inputs["encoder_embeds"]).T)

    def mtblocks(w, nb):  # w [K, nb*128] -> [nb, K, 128]
        return np.ascontiguousarray(
            w.reshape(w.shape[0], nb, 128).transpose(1, 0, 2))

    # per-expert big weights (shared across cores of the same expert)
    wdT_e = [bf(mtblocks(wd[e], 16)) for e in range(2)]
    wguI_e = []
    for e in range(2):
        g = wgu[e][:, :I].reshape(H, NPI, 128)
        u = wgu[e][:, I:].reshape(H, NPI, 128)
        blk = np.concatenate([g, u], axis=2)         # [H, NPI, 256]
        wguI_e.append(bf(np.ascontiguousarray(blk.transpose(1, 0, 2))))
    wdnT_e = [bf(mtblocks(wdn[e], 16)) for e in range(2)]
    wcqT = bf(mtblocks(f32(inputs["w_cross_q"]) * wln_pa, 8))
    wcdT = bf(mtblocks(f32(inputs["w_cross_dense"]), 16))
    wkb = bf(wkvf[:, :CC]); wvb = bf(wkvf[:, CC:])
    hTb = bf(hTp)
    cos2b = bf(cos2); sin2b = bf(sin2); rotb = bf(rot.T)
    onesr = np.ones((128, 128), np.float32)
    onesb = np.ones((128, 128), ml_dtypes.bfloat16)

    key = (b0, b1, b2, pattern)
    if _CACHE.get("key") != key:
        _CACHE.clear()
        _CACHE["key"] = key
        _CACHE["nc"] = build_fast(b0, b1, b2, pattern, nslot)
    nc = _CACHE["nc"]

    in_maps = []
    for c in range(NC_):
        qs = slice(256 * c, 256 * c + 256)
        ex = 0 if 256 * (c + 1) <= b1 else 1
        m = dict(
            hT=hTb, resid=hTp[:, qs].copy(),
            wqkv0=bf(np.concatenate([wqkv[0][:, 256 * c:256 * c + 256],
                                     wqkv[0][:, H:][:, qs],
                                     wqkv[0][:, 2 * H:][:, qs]], 1)),
            wqkv1=bf(np.concatenate([wqkv[1][:, qs],
                                     wqkv[1][:, H:][:, qs],
                                     wqkv[1][:, 2 * H:][:, qs]], 1)),
            cos2=cos2b, sin2=sin2b, rotT=rotb,
            onesr=onesr, onesb=onesb, maskm=maskm,
            encsl=np.ascontiguousarray(encT[:, qs]),
            wk=wkb, wvv=wvb,
            wdT=wdT_e[ex], wcqT=wcqT, wcdT=wcdT,
            wguI=wguI_e[ex], wdnT=wdnT_e[ex],
        )
        in_maps.append(m)

    trace = bool(int(os.environ.get("KTRACE", "0")))
    res = run_bass_kernel_spmd(nc, in_maps, core_ids=list(range(NC_)),
                               trace=trace)
    kernel.last_exec_ns = res.exec_time_ns
    out = np.empty((S, H), np.float32)
    for c in range(NC_):
        out[perm[256 * c:256 * c + 256], :] = res.results[c]["y"].T
    return out


# ============ general fallback path (original kernel) ================

def build_general(b0, b1, b2):
    nc = bacc.Bacc("TRN2", target_bir_lowering=False, debug=False,
                   num_devices=NC_)
    din = lambda n, sh, dt: nc.dram_tensor(n, sh, dt, kind="ExternalInput")
    hT = din("hT", [H, S], BF16)
    wqkv0 = din("wqkv0", [H, 768], BF16)
    wqkv1 = din("wqkv1", [H, 768], BF16)
    wd0 = din("wd0", [256, H], F32R)
    wd1 = din("wd1", [256, H], F32R)
    cos2 = din("cos2", [128, S], BF16)
    sin2 = din("sin2", [128, S], BF16)
    rotT = din("rotT", [128, 128], BF16)
    onesr = din("onesr", [128, 128], F32R)
    onesb = din("onesb", [128, 128], BF16)
    zeros = din("zeros", [128, 512], F32R)
    maskneg = din("maskneg", [S, S], BF16)
    resid = din("resid", [H, 256], F32R)
    encT = din("encT", [CH, E], BF16)
    wk = din("wk", [CH, CC], BF16)
    wvv = din("wvv", [CH, CC], BF16)
    wcq = din("wcq", [H, CC], F32R)
    wcd = din("wcd", [CC, H], F32R)
    wgu0 = din("wgu0", [H, 2 * IS], BF16)
    wgu1 = din("wgu1", [H, 2 * IS], BF16)
    wdn0 = din("wdn0", [ISP, H], BF16)
    wdn1 = din("wdn1", [ISP, H], BF16)
    y = nc.dram_tensor("y", [H, S], F32, kind="ExternalOutput")

    SC = 1.0 / float(np.sqrt(HD))
    CSC = 1.0 / float(np.sqrt(CHD))
    EXP = mybir.ActivationFunctionType.Exp
    SQ = mybir.ActivationFunctionType.Square
    SQRT = mybir.ActivationFunctionType.Sqrt
    SILU = mybir.ActivationFunctionType.Silu
    r128 = lambda ap: ap.rearrange("(c p) n -> p c n", p=128)

    with tile.TileContext(nc) as tc, ExitStack() as top:
        const = top.enter_context(tc.tile_pool(name="const", bufs=1))
        ones_sb = const.tile([128, 128], F32R)
        nc.sync.dma_start(ones_sb[:], onesr.ap()[:])
        ones_bf = const.tile([128, 128], BF16)
        nc.sync.dma_start(ones_bf[:], onesb.ap()[:])
        rot_sb = const.tile([128, 128], BF16)
        nc.sync.dma_start(rot_sb[:], rotT.ap()[:])
        from concourse.masks import make_identity
        ident = const.tile([128, 128], BF16)
        make_identity(nc, ident[:])
        cos_sb = const.tile([128, S], BF16)
        nc.sync.dma_start(cos_sb[:], cos2.ap()[:])
        sin_sb = const.tile([128, S], BF16)
        nc.sync.dma_start(sin_sb[:], sin2.ap()[:])
        zer_sb = const.tile([128, 512], F32R)
        nc.sync.dma_start(zer_sb[:], zeros.ap()[:])
        eps_sb = const.tile([128, 1], F32)
        nc.vector.memset(eps_sb[:], EPS)

        dram = top.enter_context(tc.tile_pool(name="dram", bufs=1, space="DRAM"))
        bounce = dram.tile([NC_ * H, 256], F32)
        rs_out = dram.tile([H, 256], F32)
        h2n_bnc = dram.tile([H, 256], BF16)
        h2n_all = dram.tile([NC_ * H, 256], BF16, addr_space="Shared")
        h2out = nc.dram_tensor("h2out", [H, 256], F32, kind="ExternalOutput")

        scrp = top.enter_context(tc.tile_pool(name="scr", bufs=2))

        def vwrite(op, dst, a, bb):
            if DVE_F32R:
                op(dst, a, bb)
            else:
                scr = scrp.tile([dst.shape[0], dst.shape[-1]], F32,
                                name="vscr", tag="vscr")
                op(scr[:], a, bb)
                nc.scalar.copy(dst, scr[:])

        pABC = top.enter_context(ExitStack())
        qkp = pABC.enter_context(tc.tile_pool(name="qkp", bufs=1))
        qkv_sb = qkp.tile([128, 6, S], BF16)      # q0 q1 k0 k1 v0 v1
        v_sb = qkp.tile([128, 16, 256], BF16)     # token-major v
        ctxp = pABC.enter_context(tc.tile_pool(name="ctxp", bufs=1))
        ctx_sb = ctxp.tile([128, 2, S], F32R)

        # ===== phase A: h load + rmsnorm + QKV + rope + vT =====
        with ExitStack() as pA:
            hp = pA.enter_context(tc.tile_pool(name="hp", bufs=1))
            h_sb = hp.tile([128, 16, S], BF16)
            nc.sync.dma_start(h_sb[:], r128(hT.ap()))
            with ExitStack() as pA1:
                nrm = pA1.enter_context(tc.tile_pool(name="nrm", bufs=2))
                nps = pA1.enter_context(tc.tile_pool(name="nps", bufs=2,
                                                     space="PSUM"))
                for t0, t1 in _chunks(0, S, 512):
                    pss = nps.tile([128, 512], F32, name="pss", tag="pss")
                    for kc in range(16):
                        sq = nrm.tile([128, 512], F32R, name="sq", tag="sq")
                        nc.scalar.activation(sq[:], h_sb[:, kc, t0:t1], SQ)
                        nc.tensor.matmul(pss[:], ones_sb[:], sq[:],
                                         start=(kc == 0), stop=(kc == 15))
                    rms = nrm.tile([128, 512], F32, name="rms", tag="rms")
                    nc.scalar.activation(rms[:], pss[:], SQRT,
                                         scale=1.0 / H, bias=eps_sb[:])
                    rinv = nrm.tile([128, 512], F32, name="rinv", tag="rinv")
                    nc.vector.reciprocal(rinv[:], rms[:])
                    for kc in range(16):
                        nc.vector.tensor_mul(h_sb[:, kc, t0:t1],
                                             h_sb[:, kc, t0:t1], rinv[:])
            with ExitStack() as pA2:
                wp = pA2.enter_context(tc.tile_pool(name="wp", bufs=3))
                mps = pA2.enter_context(tc.tile_pool(name="mps", bufs=2,
                                                     space="PSUM"))
                for slot in range(6):
                    wts = []
                    for ex, wsrc in ((0, wqkv0), (1, wqkv1)):
                        wt = wp.tile([128, 16, 128], BF16,
                                     name=f"wq{ex}{slot}", tag=f"wq{ex}")
                        nc.sync.dma_start(
                            wt[:], r128(wsrc.ap()[:, slot * 128:slot * 128 + 128]))
                        wts.append(wt)
                    for t0, t1 in _chunks(0, S, 512):
                        sg = [x for x in _segs(t0, t1, b0, b1, b2) if x[2]]
                        if not sg:
                            continue
                        need = sorted({x for _, _, ex in sg for x in ex})
                        pss_ = {}
                        for x in need:
                            ps = mps.tile([128, 512], F32, name=f"qps{x}",
                                          tag=f"qps{x}")
                            for kc in range(16):
                                nc.tensor.matmul(ps[:], wts[x][:, kc, :],
                                                 h_sb[:, kc, t0:t1],
                                                 start=(kc == 0), stop=(kc == 15))
                            pss_[x] = ps
                        for s, e, ex in sg:
                            if len(ex) == 1:
                                nc.vector.tensor_copy(qkv_sb[:, slot, s:e],
                                                      pss_[ex[0]][:, s - t0:e - t0])
                            else:
                                nc.vector.tensor_add(qkv_sb[:, slot, s:e],
                                                     pss_[0][:, s - t0:e - t0],
                                                     pss_[1][:, s - t0:e - t0])
                    if b2 < S:
                        nc.vector.memset(qkv_sb[:, slot, b2:S], 0.0)
                # rope on q,k
                for slot in range(4):
                    for t0, t1 in _chunks(0, S, 512):
                        rp = mps.tile([128, 512], F32, name="rps", tag="qps")
                        nc.tensor.matmul(rp[:], rot_sb[:],
                                         qkv_sb[:, slot, t0:t1],
                                         start=True, stop=True)
                        c1 = scrp.tile([128, 512], F32, name="ropec", tag="ropec")
                        nc.vector.tensor_mul(c1[:], qkv_sb[:, slot, t0:t1],
                                             cos_sb[:, t0:t1])
                        s1 = scrp.tile([128, 512], F32, name="ropes", tag="ropes")
                        nc.vector.tensor_mul(s1[:], rp[:], sin_sb[:, t0:t1])
                        nc.vector.tensor_add(qkv_sb[:, slot, t0:t1],
                                             c1[:], s1[:])
                # v -> token-major via PE transpose
                for hh in range(2):
                    for tt in range(16):
                        tp = mps.tile([128, 512], BF16, name="tps", tag="qps")
                        nc.tensor.transpose(
                            tp[:, :128],
                            qkv_sb[:, 4 + hh, tt * 128:tt * 128 + 128],
                            ident[:])
                        nc.vector.tensor_copy(v_sb[:, tt, hh * 128:hh * 128 + 128],
                                              tp[:, :128])
        # ===== phase B: self-attention (perm order) =====
        with ExitStack() as pB:
            ap_ = pB.enter_context(tc.tile_pool(name="ap", bufs=3))
            aps = pB.enter_context(tc.tile_pool(name="aps", bufs=2, space="PSUM"))
            accp = pB.enter_context(tc.tile_pool(name="accp", bufs=1, space="PSUM"))
            for t0, t1 in _chunks(0, S, 512):
                pss_ = [accp.tile([128, 512], F32, name=f"pssum{h}", tag=f"pssum{h}")
                        for h in range(2)]
                psc_ = [accp.tile([128, 512], F32, name=f"psctx{h}", tag=f"psctx{h}")
                        for h in range(2)]
                for kt in range(16):
                    mt_ = ap_.tile([128, 512], BF16, name="mt", tag="mt")
                    nc.sync.dma_start(
                        mt_[:], maskneg.ap()[kt * 128:kt * 128 + 128, t0:t1])
                    for hh in range(2):
                        sc = aps.tile([128, 512], F32, name="sc", tag="sc")
                        nc.tensor.matmul(
                            sc[:], qkv_sb[:, 2 + hh, kt * 128:kt * 128 + 128],
                            qkv_sb[:, hh, t0:t1], start=True, stop=True)
                        nc.vector.tensor_add(sc[:], sc[:], mt_[:])
                        pr = ap_.tile([128, 512], BF16, name="pr", tag="pr")
                        nc.scalar.activation(pr[:], sc[:], EXP, scale=SC)
                        nc.tensor.matmul(pss_[hh][:], ones_bf[:], pr[:],
                                         start=(kt == 0), stop=(kt == 15))
                        nc.tensor.matmul(
                            psc_[hh][:], v_sb[:, kt, hh * 128:hh * 128 + 128],
                            pr[:], start=(kt == 0), stop=(kt == 15))
                for hh in range(2):
                    rc = ap_.tile([128, 512], F32, name="rc", tag="rc")
                    nc.vector.reciprocal(rc[:], pss_[hh][:])
                    vwrite(nc.vector.tensor_mul, ctx_sb[:, hh, t0:t1],
                           psc_[hh][:], rc[:])
        # ===== phase C: dense (routed) -> bounce -> RS =====
        with ExitStack() as pC:
            dwp = pC.enter_context(tc.tile_pool(name="dwp", bufs=1))
            dps = pC.enter_context(tc.tile_pool(name="dps", bufs=2, space="PSUM"))
            dop = pC.enter_context(tc.tile_pool(name="dop", bufs=4))
            dwts = []
            for ex, wsrc in ((0, wd0), (1, wd1)):
                dwt = dwp.tile([128, 2, H], F32R, name=f"dw{ex}", tag=f"dw{ex}")
                nc.sync.dma_start(dwt[:], r128(wsrc.ap()))
                dwts.append(dwt)
            for tt in range(8):
                t0, t1 = tt * 256, tt * 256 + 256
                sg = _segs(t0, t1, b0, b1, b2)
                live = [x for x in sg if x[2]]
                for mt in range(16):
                    ot = dop.tile([128, 256], F32, name="dot", tag="dot")
                    if live:
                        need = sorted({x for _, _, ex in live for x in ex})
                        pss_ = {}
                        for x in need:
                            ps = dps.tile([128, 256], F32, name=f"dpst{x}",
                                          tag=f"dpst{x}")
                            for kc in range(2):
                                nc.tensor.matmul(
                                    ps[:],
                                    dwts[x][:, kc, mt * 128:mt * 128 + 128],
                                    ctx_sb[:, kc, t0:t1],
                                    start=(kc == 0), stop=(kc == 1))
                            pss_[x] = ps
                        for s, e, ex in sg:
                            if len(ex) == 2:
                                nc.vector.tensor_add(ot[:, s - t0:e - t0],
                                                     pss_[0][:, s - t0:e - t0],
                                                     pss_[1][:, s - t0:e - t0])
                            elif ex:
                                nc.vector.tensor_copy(ot[:, s - t0:e - t0],
                                                      pss_[ex[0]][:, s - t0:e - t0])
                            else:
                                nc.vector.memset(ot[:, s - t0:e - t0], 0.0)
                    else:
                        nc.vector.memset(ot[:], 0.0)
                    nc.sync.dma_start(
                        bounce[tt * H + mt * 128: tt * H + mt * 128 + 128, :],
                        ot[:])
        pABC.close()
        nc.gpsimd.collective_compute(
            "ReduceScatter", mybir.AluOpType.add,
            replica_groups=[list(range(NC_))],
            ins=[bounce.opt()], outs=[rs_out.opt()])

        # ===== phase D: cross attention (token-parallel) =====
        with ExitStack() as pD:
            dp = pD.enter_context(tc.tile_pool(name="dp", bufs=1))
            dps2 = pD.enter_context(tc.tile_pool(name="dps2", bufs=2, space="PSUM"))
            h1_sb = dp.tile([128, 16, 256], F32R)
            cq_sb = dp.tile([128, 8, 256], BF16)
            cctx_sb = dp.tile([128, 8, 256], F32R)
            with ExitStack() as pD1:
                d1 = pD1.enter_context(tc.tile_pool(name="d1", bufs=1))
                rs_sb = d1.tile([128, 16, 256], F32)
                nc.sync.dma_start(rs_sb[:], r128(rs_out[:]))
                re_sb = d1.tile([128, 16, 256], F32R)
                nc.sync.dma_start(re_sb[:], r128(resid.ap()))
                for kc in range(16):
                    vwrite(nc.vector.tensor_add, h1_sb[:, kc, :],
                           rs_sb[:, kc, :], re_sb[:, kc, :].bitcast(F32))
                pss = dps2.tile([128, 256], F32, name="psd", tag="psd")
                for kc in range(16):
                    sq = scrp.tile([128, 256], F32R, name="sqd", tag="sqd")
                    nc.scalar.activation(sq[:], h1_sb[:, kc, :].bitcast(F32), SQ)
                    nc.tensor.matmul(pss[:], ones_sb[:], sq[:],
                                     start=(kc == 0), stop=(kc == 15))
                rms = scrp.tile([128, 256], F32, name="rmsd", tag="rmsd")
                nc.scalar.activation(rms[:], pss[:], SQRT, scale=1.0 / H, bias=eps_sb[:])
                rinv = d1.tile([128, 256], F32)
                nc.vector.reciprocal(rinv[:], rms[:])
                h1n_sb = d1.tile([128, 16, 256], F32R)
                for kc in range(16):
                    vwrite(nc.vector.tensor_mul, h1n_sb[:, kc, :],
                           h1_sb[:, kc, :].bitcast(F32), rinv[:])
                for mt in range(8):
                    wcq_t = d1.tile([128, 16, 128], F32R, name="wcqt", tag="wcqt",
                                    bufs=2)
                    nc.sync.dma_start(
                        wcq_t[:], r128(wcq.ap()[:, mt * 128:mt * 128 + 128]))
                    ps = dps2.tile([128, 256], F32, name="cqp", tag="psd")
                    for kc in range(16):
                        nc.tensor.matmul(ps[:],
                                         wcq_t[:, kc, :],
                                         h1n_sb[:, kc, :],
                                         start=(kc == 0), stop=(kc == 15))
                    nc.vector.tensor_copy(cq_sb[:, mt, :], ps[:])
            with ExitStack() as pD2:
                kp = pD2.enter_context(tc.tile_pool(name="kp", bufs=1))
                k_sb = kp.tile([128, 8, E], BF16)
                v_sb2 = kp.tile([128, 16, CC], BF16)
                with ExitStack() as pD2e:
                    ep = pD2e.enter_context(tc.tile_pool(name="ep", bufs=1))
                    enc_sb = ep.tile([128, 8, E], BF16)
                    nc.sync.dma_start(enc_sb[:], r128(encT.ap()))
                    wk_sb = ep.tile([128, 8, CC], BF16)
                    nc.sync.dma_start(wk_sb[:], r128(wk.ap()))
                    wv_sb = ep.tile([128, 8, CC], BF16)
                    nc.sync.dma_start(wv_sb[:], r128(wvv.ap()))
                    for mt in range(8):
                        for n0, n1 in _chunks(0, E, 512):
                            ps = dps2.tile([128, 512], F32, name="kps", tag="kps")
                            for kc in range(8):
                                nc.tensor.matmul(
                                    ps[:], wk_sb[:, kc, mt * 128:mt * 128 + 128],
                                    enc_sb[:, kc, n0:n1],
                                    start=(kc == 0), stop=(kc == 7))
                            nc.vector.tensor_copy(k_sb[:, mt, n0:n1], ps[:])
                    for tt in range(16):
                        for n0, n1 in _chunks(0, CC, 512):
                            ps = dps2.tile([128, 512], F32, name="vps", tag="kps")
                            for kc in range(8):
                                nc.tensor.matmul(
                                    ps[:], enc_sb[:, kc, tt * 128:tt * 128 + 128],
                                    wv_sb[:, kc, n0:n1],
                                    start=(kc == 0), stop=(kc == 7))
                            nc.vector.tensor_copy(v_sb2[:, tt, n0:n1], ps[:])
                with ExitStack() as pD3:
                    cap = pD3.enter_context(tc.tile_pool(name="cap", bufs=3))
                    caps = pD3.enter_context(tc.tile_pool(name="caps", bufs=2,
                                                          space="PSUM"))
                    cacc = pD3.enter_context(tc.tile_pool(name="cacc", bufs=1,
                                                          space="PSUM"))
                    for h in range(NH):
                        kch, koff = h // 2, 64 * (h % 2)
                        pssum = cacc.tile([128, 256], F32, name="cps", tag="cps")
                        psctx = cacc.tile([64, 256], F32, name="cpc", tag="cpc")
                        for kt in range(16):
                            sc = caps.tile([128, 256], F32, name="csc", tag="csc")
                            nc.tensor.matmul(
                                sc[:],
                                k_sb[koff:koff + 64, kch, kt * 128:kt * 128 + 128],
                                cq_sb[koff:koff + 64, kch, :],
                                start=True, stop=True)
                            pr = cap.tile([128, 256], BF16, name="cpr", tag="cpr")
                            nc.scalar.activation(pr[:], sc[:], EXP, scale=CSC)
                            nc.tensor.matmul(pssum[:], ones_bf[:], pr[:],
                                             start=(kt == 0), stop=(kt == 15))
                            nc.tensor.matmul(psctx[:],
                                             v_sb2[:, kt, 64 * h:64 * h + 64],
                                             pr[:], start=(kt == 0), stop=(kt == 15))
                        rc = cap.tile([64, 256], F32, name="crc", tag="crc")
                        nc.vector.reciprocal(rc[:], pssum[:64, :])
                        vwrite(nc.vector.tensor_mul,
                               cctx_sb[koff:koff + 64, kch, :], psctx[:], rc[:])
            # cdense + residual -> h2, rmsnorm -> h2n -> AG
            with ExitStack() as pD4:
                d4 = pD4.enter_context(tc.tile_pool(name="d4", bufs=1))
                h2_sb = d4.tile([128, 16, 256], F32)
                h2n_sb = d4.tile([128, 16, 256], BF16)
                wcd_sb = d4.tile([128, 8, H], F32R)
                nc.sync.dma_start(wcd_sb[:], r128(wcd.ap()))
                for mt in range(16):
                    ps = dps2.tile([128, 256], F32, name="cdp", tag="psd")
                    for kc in range(8):
                        nc.tensor.matmul(ps[:],
                                         wcd_sb[:, kc, mt * 128:mt * 128 + 128],
                                         cctx_sb[:, kc, :],
                                         start=(kc == 0), stop=(kc == 7))
                    nc.vector.tensor_add(h2_sb[:, mt, :], ps[:],
                                         h1_sb[:, mt, :].bitcast(F32))
                pss2 = dps2.tile([128, 256], F32, name="psd2", tag="psd")
                for kc in range(16):
                    sq = scrp.tile([128, 256], F32R, name="sqd2", tag="sqd")
                    nc.scalar.activation(sq[:], h2_sb[:, kc, :], SQ)
                    nc.tensor.matmul(pss2[:], ones_sb[:], sq[:],
                                     start=(kc == 0), stop=(kc == 15))
                rms2 = scrp.tile([128, 256], F32, name="rmsd2", tag="rmsd")
                nc.scalar.activation(rms2[:], pss2[:], SQRT,
                                     scale=1.0 / H, bias=eps_sb[:])
                rinv2 = d4.tile([128, 256], F32)
                nc.vector.reciprocal(rinv2[:], rms2[:])
                for kc in range(16):
                    nc.vector.tensor_mul(h2n_sb[:, kc, :],
                                         h2_sb[:, kc, :], rinv2[:])
                nc.sync.dma_start(r128(h2n_bnc[:]), h2n_sb[:])
                nc.sync.dma_start(r128(h2out.ap()), h2_sb[:])
            nc.gpsimd.collective_compute(
                "AllGather", mybir.AluOpType.bypass,
                replica_groups=[list(range(NC_))],
                ins=[h2n_bnc.opt()], outs=[h2n_all.opt()])
        # ===== phase F: MLP (routed by expert ranges, bf16) =====
        with ExitStack() as pF:
            fp = pF.enter_context(tc.tile_pool(name="fp", bufs=1))
            hn_sb = fp.tile([128, 16, S], BF16)
            for r in range(NC_):
                nc.sync.dma_start(hn_sb[:, :, r * 256:r * 256 + 256],
                                  r128(h2n_all[r * H:(r + 1) * H, :]))
            fw = pF.enter_context(tc.tile_pool(name="fw", bufs=1))
            fps = pF.enter_context(tc.tile_pool(name="fps", bufs=1, space="PSUM"))
            fpd = pF.enter_context(tc.tile_pool(name="fpd", bufs=2, space="PSUM"))
            fac = pF.enter_context(tc.tile_pool(name="fac", bufs=2))
            fout = pF.enter_context(tc.tile_pool(name="fout", bufs=4))
            for ex, (lo, hi) in ((0, (0, b1)), (1, (b1, S))):
                gsrc = (wgu0, wgu1)[ex]
                dsrc = (wdn0, wdn1)[ex]
                dn_t = fw.tile([128, 6, H], BF16, name=f"dn{ex}", tag="dn")
                nc.sync.dma_start(dn_t[:], r128(dsrc.ap()))
                gwts = []
                for pi in range(6):
                    gw = 128 if pi < 5 else 48
                    gwt = fw.tile([128, 16, 256], BF16,
                                  name=f"guw{ex}{pi}", tag=f"guw{pi}")
                    nc.sync.dma_start(
                        gwt[:, :, :2 * gw],
                        r128(gsrc.ap()[:, pi * 256:pi * 256 + 2 * gw]))
                    gwts.append(gwt)
                for a0 in range(0, S, 512):
                    c0, c1 = max(a0, lo), min(a0 + 512, hi)
                    if c0 >= c1:
                        continue
                    t0_, W = a0, 512
                    eo, ew = c0 - a0, c1 - c0
                    act = fac.tile([128, 6, 512], BF16, name="act", tag="act")
                    for pi in range(6):
                        gw = 128 if pi < 5 else 48
                        gwt = gwts[pi]
                        pg = fps.tile([128, 512], F32, name="pg", tag="pg")
                        pu = fps.tile([128, 512], F32, name="pu", tag="pu")
                        for kc in range(16):
                            nc.tensor.matmul(pg[:gw, :W], gwt[:, kc, :gw],
                                             hn_sb[:, kc, t0_:t0_ + 512],
                                             start=(kc == 0), stop=(kc == 15))
                            nc.tensor.matmul(pu[:gw, :W], gwt[:, kc, gw:2 * gw],
                                             hn_sb[:, kc, t0_:t0_ + 512],
                                             start=(kc == 0), stop=(kc == 15))
                        gs = scrp.tile([128, 512], F32, name="gs", tag="gs")
                        nc.scalar.activation(gs[:gw, :W], pg[:gw, :W], SILU)
                        nc.vector.tensor_mul(act[:gw, pi, :W],
                                             gs[:gw, :W], pu[:gw, :W])
                    for mt in range(16):
                        pd = fpd.tile([128, 512], F32, name="pd", tag="pd")
                        for pi in range(6):
                            kw = 128 if pi < 5 else 48
                            nc.tensor.matmul(
                                pd[:, :W],
                                dn_t[:kw, pi, mt * 128:mt * 128 + 128],
                                act[:kw, pi, :W],
                                start=(pi == 0), stop=(pi == 5))
                        ot = fout.tile([128, 512], F32, name="fot", tag="fot")
                        nc.vector.tensor_copy(ot[:, eo:eo + ew], pd[:, eo:eo + ew])
                        nc.sync.dma_start(
                            y.ap()[mt * 128:mt * 128 + 128, c0:c1],
                            ot[:, eo:eo + ew])
    nc.compile()
    return nc


def _kernel_general(inputs, perm, b0, b1, b2):
    import ml_dtypes
    f32 = lambda x: np.ascontiguousarray(np.asarray(x, np.float32))
    bf = lambda x: np.ascontiguousarray(np.asarray(x).astype(ml_dtypes.bfloat16))
    pos = np.asarray(inputs["positions"]).astype(np.float32)
    half = HD // 2
    inv_freq = 1.0 / (ROPE_BASE ** (np.arange(half, dtype=np.float32) / half))
    fr = pos[:, None] * inv_freq[None, :]
    cos2 = np.concatenate([np.cos(fr)] * 2, 1).T[:, perm]
    sin2 = np.concatenate([np.sin(fr)] * 2, 1).T[:, perm]
    rot = np.zeros((HD, HD), np.float32)
    rot[np.arange(half), np.arange(half) + half] = -1.0
    rot[np.arange(half) + half, np.arange(half)] = 1.0
    op = np.asarray(inputs["positions"])[perm]
    maskneg = np.where(op[None, :] >= op[:, None], 0.0, -30000.0)

    wln_in = f32(inputs["w_ln_in"])[:, None]
    wln_pa = f32(inputs["w_ln_post_attn"])[:, None]
    wln_pc = f32(inputs["w_ln_post_cross"])[:, None]
    wqkv = [f32(inputs["w_vis_qkv"]) * wln_in, f32(inputs["w_lang_qkv"]) * wln_in]
    wd = [f32(inputs["w_vis_dense"]), f32(inputs["w_lang_dense"])]
    wgu = [f32(inputs["w_vis_gate_up"]) * wln_pc,
           f32(inputs["w_lang_gate_up"]) * wln_pc]
    wdn = [f32(inputs["w_vis_down"]), f32(inputs["w_lang_down"])]
    wkvf = f32(inputs["w_cross_kv"])
    hTp = f32(inputs["hidden_states"]).T[:, perm].copy()

    def interleave(w):  # w [H, 2*IS] = [gate | up]
        cols = []
        for i in range(5):
            cols.append(w[:, 128 * i:128 * i + 128])
            cols.append(w[:, IS + 128 * i:IS + 128 * i + 128])
        cols.append(w[:, 640:IS]); cols.append(w[:, IS + 640:2 * IS])
        return np.ascontiguousarray(np.concatenate(cols, 1))

    key = ("general", b0, b1, b2)
    if _CACHE.get("key") != key:
        _CACHE.clear()
        _CACHE["key"] = key
        _CACHE["nc"] = build_general(b0, b1, b2)
    nc = _CACHE["nc"]

    in_maps = []
    for c in range(NC_):
        qs = slice(256 * c, 256 * c + 256)
        m = dict(
            hT=bf(hTp),
            wqkv0=bf(np.concatenate([wqkv[0][:, qs], wqkv[0][:, H:][:, qs],
                                     wqkv[0][:, 2 * H:][:, qs]], 1)),
            wqkv1=bf(np.concatenate([wqkv[1][:, qs], wqkv[1][:, H:][:, qs],
                                     wqkv[1][:, 2 * H:][:, qs]], 1)),
            wd0=wd[0][qs].copy(), wd1=wd[1][qs].copy(),
            cos2=bf(cos2), sin2=bf(sin2), rotT=bf(rot.T),
            onesr=np.ones((128, 128), np.float32),
            onesb=np.ones((128, 128), ml_dtypes.bfloat16),
            zeros=np.zeros((128, 512), np.float32),
            maskneg=bf(maskneg), resid=hTp[:, qs].copy(),
            encT=bf(f32(inputs["encoder_embeds"]).T),
            wk=bf(wkvf[:, :CC]), wvv=bf(wkvf[:, CC:]),
            wcq=(f32(inputs["w_cross_q"]) * wln_pa).copy(),
            wcd=f32(inputs["w_cross_dense"]),
            wgu0=bf(interleave(np.concatenate(
                [wgu[0][:, IS * c:IS * c + IS],
                 wgu[0][:, I + IS * c:I + IS * c + IS]], 1))),
            wgu1=bf(interleave(np.concatenate(
                [wgu[1][:, IS * c:IS * c + IS],
                 wgu[1][:, I + IS * c:I + IS * c + IS]], 1))),
            wdn0=bf(np.concatenate([wdn[0][IS * c:IS * c + IS],
                                    np.zeros((ISP - IS, H), np.float32)], 0)),
            wdn1=bf(np.concatenate([wdn[1][IS * c:IS * c + IS],
                                    np.zeros((ISP - IS, H), np.float32)], 0)),
        )
        in_maps.append(m)

    trace = bool(int(os.environ.get("KTRACE", "0")))
    res = run_bass_kernel_spmd(nc, in_maps, core_ids=list(range(NC_)),
                               trace=trace)
    kernel.last_exec_ns = res.exec_time_ns
    tot = res.results[0]["y"].astype(np.float64)
    for c in range(1, NC_):
        tot += res.results[c]["y"]
    for c in range(NC_):
        tot[:, 256 * c:256 * c + 256] += res.results[c]["h2out"]
    out = np.empty((S, H), np.float32)
    out[perm, :] = tot.T.astype(np.float32)
    return out


# revision 23
# speedup vs baseline: 1.6229x; 1.0425x over previous
"""Trainium2 Bass kernel for nn_CogAgentDecoderLayer (8-core SPMD).

Fast path (disjoint vis/lang masks, expert boundary % 256 == 0):
feature-major activations [feat, tok] in permuted token order.
Self-attn head-parallel (2 heads/core, block-sparse causal), then an
AllToAll re-shards ctx token-parallel (256 tok/core); dense, cross-attn
and MLP all run token-parallel with full weights streamed from HBM
(each core uses one expert's weights, chosen host-side). Cross-attn KV
is computed enc-token-sharded and AllGathered early (overlapped with
self-attention). No post-MLP collective: each core emits its final
[H, 256] f32 output slice. bf16 everywhere except residual trunk (f32).

General fallback (any masks): original head/intermediate-parallel
kernel with ReduceScatter + AllGather.
"""
import os
import numpy as np
from contextlib import ExitStack
from concourse import bacc, tile, mybir
from concourse.bass_utils import run_bass_kernel_spmd

NC_ = 8
S, E, H, NH, HD = 2048, 2048, 2048, 16, 128
CH, CC, CHD = 1024, 1024, 64
I = 5504
IS = I // NC_          # 688
ISP = 768              # padded to 6*128 (general path)
NPI = I // 128         # 43 (fast path)
EPS = 1e-5
ROPE_BASE = 10000.0
F32 = mybir.dt.float32
F32R = mybir.dt.float32r
BF16 = mybir.dt.bfloat16
DVE_F32R = True        # DVE may write fp32r tiles directly


def _segs(lo, hi, b0, b1, b2):
    pts = sorted({lo, hi, *[b for b in (b0, b1, b2) if lo < b < hi]})
    out = []
    for s, e in zip(pts, pts[1:]):
        ex = []
        if s < b1:
            ex.append(0)
        if b0 <= s < b2:
            ex.append(1)
        out.append((s, e, ex))
    return out


def _chunks(lo, hi, w):
    out = []
    while lo < hi:
        out.append((lo, min(lo + w, hi)))
        lo += w
    return out


def build_fast(b0, b1, b2, pattern, nslot):
    """pattern: per 512-chunk tuple of (kt, slot) with slot=-1 for
    fully-visible key tiles, else index into maskm."""
    nc = bacc.Bacc("TRN2", target_bir_lowering=False, debug=False,
                   num_devices=NC_)
    din = lambda n, sh, dt: nc.dram_tensor(n, sh, dt, kind="ExternalInput")
    hT = din("hT", [H, S], BF16)
    resid = din("resid", [H, 256], F32)
    wqkv0 = din("wqkv0", [H, 768], BF16)
    wqkv1 = din("wqkv1", [H, 768], BF16)
    cos2 = din("cos2", [128, S], BF16)
    sin2 = din("sin2", [128, S], BF16)
    rotT = din("rotT", [128, 128], BF16)
    onesr = din("onesr", [128, 128], F32R)
    onesb = din("onesb", [128, 128], BF16)
    maskm = din("maskm", [128, max(nslot, 1), 512], BF16)
    encsl = din("encsl", [CH, 256], BF16)
    wk = din("wk", [CH, CC], BF16)
    wvv = din("wvv", [CH, CC], BF16)
    wdT = din("wdT", [16, H, 128], BF16)
    wcqT = din("wcqT", [8, H, 128], BF16)
    wcdT = din("wcdT", [16, CC, 128], BF16)
    wguI = din("wguI", [NPI, H, 256], BF16)
    wdnT = din("wdnT", [16, I, 128], BF16)
    y = nc.dram_tensor("y", [H, 256], F32, kind="ExternalOutput")

    SC = 1.0 / float(np.sqrt(HD))
    CSC = 1.0 / float(np.sqrt(CHD))
    EXP = mybir.ActivationFunctionType.Exp
    SQ = mybir.ActivationFunctionType.Square
    SQRT = mybir.ActivationFunctionType.Sqrt
    SILU = mybir.ActivationFunctionType.Silu
    r128 = lambda ap: ap.rearrange("(c p) n -> p c n", p=128)

    with tile.TileContext(nc) as tc, ExitStack() as top:
        const = top.enter_context(tc.tile_pool(name="const", bufs=1))
        ones_sb = const.tile([128, 128], F32R)
        nc.sync.dma_start(ones_sb[:], onesr.ap()[:])
        ones_bf = const.tile([128, 128], BF16)
        nc.sync.dma_start(ones_bf[:], onesb.ap()[:])
        rot_sb = const.tile([128, 128], BF16)
        nc.sync.dma_start(rot_sb[:], rotT.ap()[:])
        from concourse.masks import make_identity
        ident = const.tile([128, 128], BF16)
        make_identity(nc, ident[:])
        cos_sb = const.tile([128, S], BF16)
        nc.sync.dma_start(cos_sb[:], cos2.ap()[:])
        sin_sb = const.tile([128, S], BF16)
        nc.sync.dma_start(sin_sb[:], sin2.ap()[:])
        eps_sb = const.tile([128, 1], F32)
        nc.vector.memset(eps_sb[:], EPS)

        dram = top.enter_context(tc.tile_pool(name="dram", bufs=1, space="DRAM"))
        kv_bnc = dram.tile([2 * CC, 256], BF16)
        kv_all = dram.tile([NC_ * 2 * CC, 256], BF16, addr_space="Shared")
        ctx_bnc = dram.tile([NC_ * 256, 256], BF16)
        ctx_all = dram.tile([NC_ * 256, 256], BF16)

        scrp = top.enter_context(tc.tile_pool(name="scr", bufs=2))

        pAB = top.enter_context(ExitStack())
        qkp = pAB.enter_context(tc.tile_pool(name="qkp", bufs=1))
        qkv_sb = qkp.tile([128, 6, S], BF16)      # q0 q1 k0 k1 v0 v1
        v_sb = qkp.tile([128, 16, 256], BF16)     # token-major v
        mk_sb = qkp.tile([128, max(nslot, 1), 512], BF16)

        # PE warmup while initial DMAs land
        with ExitStack() as pW:
            wps = pW.enter_context(tc.tile_pool(name="wps", bufs=2,
                                                space="PSUM"))
            for i in range(150):
                wp_ = wps.tile([128, 128], F32, name="wrm", tag="wrm")
                nc.tensor.matmul(wp_[:], ones_bf[:], ones_bf[:],
                                 start=True, stop=True)

        # ===== phase 0: cross-KV for this core's 256 enc tokens -> AG ====
        hpool = pAB.enter_context(tc.tile_pool(name="hpool", bufs=2))
        htiles = {}

        def h_load(ci):
            t0 = ci * 512
            ht = hpool.tile([128, 16, 512], BF16, name="hc", tag="hc")
            nc.sync.dma_start(ht[:], r128(hT.ap())[:, :, t0:t0 + 512])
            htiles[ci] = ht

        with ExitStack() as p0:
            ep = p0.enter_context(tc.tile_pool(name="ep", bufs=1))
            enc_sb = ep.tile([128, 8, 256], BF16)
            nc.sync.dma_start(enc_sb[:], r128(encsl.ap()))
            wk_sb = ep.tile([128, 8, CC], BF16)
            nc.sync.dma_start(wk_sb[:], r128(wk.ap()))
            wv_sb = ep.tile([128, 8, CC], BF16)
            nc.sync.dma_start(wv_sb[:], r128(wvv.ap()))
            h_load(0)
            h_load(1)
            kps = p0.enter_context(tc.tile_pool(name="kps", bufs=2,
                                                space="PSUM"))
            kout = p0.enter_context(tc.tile_pool(name="kout", bufs=3))
            for mt in range(8):
                ps = kps.tile([128, 256], F32, name="kp", tag="kp")
                for kc in range(8):
                    nc.tensor.matmul(ps[:],
                                     wk_sb[:, kc, mt * 128:mt * 128 + 128],
                                     enc_sb[:, kc, :],
                                     start=(kc == 0), stop=(kc == 7))
                ko = kout.tile([128, 256], BF16, name="ko", tag="ko")
                nc.vector.tensor_copy(ko[:], ps[:])
                nc.sync.dma_start(kv_bnc[mt * 128:mt * 128 + 128, :], ko[:])
            vdst = kv_bnc[CC:2 * CC, :].rearrange("(p x) n -> p (x n)", p=256)
            for et in range(2):
                for hf in range(2):
                    ps = kps.tile([128, 512], F32, name="vp", tag="kp")
                    for kc in range(8):
                        nc.tensor.matmul(
                            ps[:], enc_sb[:, kc, et * 128:et * 128 + 128],
                            wv_sb[:, kc, hf * 512:hf * 512 + 512],
                            start=(kc == 0), stop=(kc == 7))
                    vo = kout.tile([128, 512], BF16, name="vo", tag="ko")
                    nc.vector.tensor_copy(vo[:], ps[:])
                    nc.sync.dma_start(
                        vdst[et * 128:et * 128 + 128,
                             hf * 512:hf * 512 + 512], vo[:])
        nc.gpsimd.collective_compute(
            "AllGather", mybir.AluOpType.bypass,
            replica_groups=[list(range(NC_))],
            ins=[kv_bnc.opt()], outs=[kv_all.opt()])

        # ===== phase A (chunk-major): rmsnorm + QKV + rope + vT =====
        pA = pAB.enter_context(ExitStack())
        wp = pA.enter_context(tc.tile_pool(name="wp", bufs=1))
        wts = {}
        for slot in range(6):
            for ex, wsrc in ((0, wqkv0), (1, wqkv1)):
                wt = wp.tile([128, 16, 128], BF16, name=f"wq{ex}{slot}",
                             tag=f"wq{ex}{slot}")
                nc.sync.dma_start(
                    wt[:], r128(wsrc.ap()[:, slot * 128:slot * 128 + 128]))
                wts[(ex, slot)] = wt
        nc.sync.dma_start(mk_sb[:], maskm.ap()[:])
        nrm = pA.enter_context(tc.tile_pool(name="nrm", bufs=6))
        mps = pA.enter_context(tc.tile_pool(name="mps", bufs=2, space="PSUM"))
        for ci, (t0, t1) in enumerate(_chunks(0, S, 512)):
            ht = htiles.pop(ci)
            if ci + 2 < 4:
                h_load(ci + 2)
            sg = [x for x in _segs(t0, t1, b0, b1, b2) if x[2]]
            need = sorted({x for _, _, ex in sg for x in ex})

            def qkv_mm(slot):
                pss_ = {}
                for x in need:
                    ps = mps.tile([128, 512], F32, name=f"qps{x}",
                                  tag=f"qps{x % 2}")
                    for kc in range(16):
                        nc.tensor.matmul(ps[:], wts[(x, slot)][:, kc, :],
                                         ht[:, kc, :],
                                         start=(kc == 0), stop=(kc == 15))
                    pss_[x] = ps
                return pss_

            # QKV on raw h: rmsnorm's 1/rms folds into the psum->SBUF copy
            pss0 = qkv_mm(0)
            pssn = mps.tile([128, 512], F32, name="pss", tag="qpsA")
            for kc in range(16):
                sq = nrm.tile([128, 512], F32R, name="sq", tag="sq")
                nc.scalar.activation(sq[:], ht[:, kc, :], SQ)
                nc.tensor.matmul(pssn[:], ones_sb[:], sq[:],
                                 start=(kc == 0), stop=(kc == 15))
            rms = nrm.tile([128, 512], F32, name="rms", tag="rms")
            nc.scalar.activation(rms[:], pssn[:], SQRT,
                                 scale=1.0 / H, bias=eps_sb[:])
            rinv = nrm.tile([128, 512], F32, name="rinv", tag="rinv")
            nc.vector.reciprocal(rinv[:], rms[:])

            def qkv_store(slot, pss_):
                for s, e, ex in sg:
                    if len(ex) == 1:
                        nc.vector.tensor_mul(qkv_sb[:, slot, s:e],
                                             pss_[ex[0]][:, s - t0:e - t0],
                                             rinv[:, s - t0:e - t0])
                    else:
                        tmp = scrp.tile([128, 512], F32, name="qtmp",
                                        tag="qtmp")
                        nc.vector.tensor_add(tmp[:, :e - s],
                                             pss_[0][:, s - t0:e - t0],
                                             pss_[1][:, s - t0:e - t0])
                        nc.vector.tensor_mul(qkv_sb[:, slot, s:e],
                                             tmp[:, :e - s],
                                             rinv[:, s - t0:e - t0])
                if b2 < S:
                    nc.vector.memset(qkv_sb[:, slot, b2:S], 0.0)

            qkv_store(0, pss0)
            for slot in range(1, 6):
                qkv_store(slot, qkv_mm(slot))
            # rope on q,k for this chunk
            for slot in range(4):
                rp = mps.tile([128, 512], F32, name="rps", tag="qpsA")
                nc.tensor.matmul(rp[:], rot_sb[:], qkv_sb[:, slot, t0:t1],
                                 start=True, stop=True)
                c1 = scrp.tile([128, 512], F32, name="ropec", tag="ropec")
                nc.vector.tensor_mul(c1[:], qkv_sb[:, slot, t0:t1],
                                     cos_sb[:, t0:t1])
                s1 = scrp.tile([128, 512], F32, name="ropes", tag="ropes")
                nc.vector.tensor_mul(s1[:], rp[:], sin_sb[:, t0:t1])
                nc.vector.tensor_add(qkv_sb[:, slot, t0:t1], c1[:], s1[:])
            # v -> token-major via PE transpose (this chunk's key tiles)
            for hh in range(2):
                for tt in range(4 * ci, 4 * ci + 4):
                    tp = mps.tile([128, 512], BF16, name="tps", tag="qpsA")
                    nc.tensor.transpose(
                        tp[:, :128],
                        qkv_sb[:, 4 + hh, tt * 128:tt * 128 + 128],
                        ident[:])
                    nc.vector.tensor_copy(v_sb[:, tt, hh * 128:hh * 128 + 128],
                                          tp[:, :128])
        pA.close()
        # ===== phase B: self-attention, block-sparse, 2-deep pipeline ====
        with ExitStack() as pB:
            ap_ = pB.enter_context(tc.tile_pool(name="ap", bufs=4))
            aps = pB.enter_context(tc.tile_pool(name="aps", bufs=3, space="PSUM"))
            accp = pB.enter_context(tc.tile_pool(name="accp", bufs=1, space="PSUM"))
            ctxo = pB.enter_context(tc.tile_pool(name="ctxo", bufs=3))
            for ci, (t0, t1) in enumerate(_chunks(0, S, 512)):
                live = pattern[ci]
                nlv = len(live)
                for hh in range(2):
                    pss = accp.tile([128, 512], F32, name="pssum",
                                    tag=f"pssum{hh}")
                    psc = accp.tile([128, 512], F32, name="psctx",
                                    tag=f"psctx{hh}")
                    scs = [None, None, None]
                    prs = [None, None, None]
                    for j in range(nlv + 2):
                        if j < nlv:
                            kt, _ = live[j]
                            sc = aps.tile([128, 512], F32, name="sc", tag="sc")
                            nc.tensor.matmul(
                                sc[:],
                                qkv_sb[:, 2 + hh, kt * 128:kt * 128 + 128],
                                qkv_sb[:, hh, t0:t1], start=True, stop=True)
                            scs[j % 3] = sc
                        if j >= 1 and j - 1 < nlv:
                            kt, slot = live[j - 1]
                            pr = ap_.tile([128, 512], BF16, name="pr", tag="pr")
                            nc.scalar.activation(pr[:], scs[(j - 1) % 3][:],
                                                 EXP, scale=SC)
                            if slot >= 0:
                                nc.vector.tensor_mul(pr[:], pr[:],
                                                     mk_sb[:, slot, :])
                            prs[(j - 1) % 3] = pr
                        if j >= 2:
                            kt, slot = live[j - 2]
                            pr = prs[(j - 2) % 3]
                            nc.tensor.matmul(pss[:], ones_bf[:], pr[:],
                                             start=(j == 2), stop=(j == nlv + 1))
                            nc.tensor.matmul(
                                psc[:], v_sb[:, kt, hh * 128:hh * 128 + 128],
                                pr[:], start=(j == 2), stop=(j == nlv + 1))
                    ctxb = ctxo.tile([128, 512], BF16, name="ctxb",
                                     tag=f"ctxb{hh}")
                    rc = ap_.tile([128, 512], F32, name="rc", tag="rc")
                    nc.vector.reciprocal(rc[:], pss[:])
                    nc.vector.tensor_mul(ctxb[:], psc[:], rc[:])
                    for hf in range(2):
                        jb = (t0 + hf * 256) // 256
                        dst = ctx_bnc[jb * 256:(jb + 1) * 256, :]
                        nc.sync.dma_start(
                            dst.rearrange("(c p) n -> p c n",
                                          p=128)[:, hh, :],
                            ctxb[:, hf * 256:hf * 256 + 256])
        pAB.close()
        nc.gpsimd.collective_compute(
            "AllToAll", mybir.AluOpType.bypass,
            replica_groups=[list(range(NC_))],
            ins=[ctx_bnc.opt()], outs=[ctx_all.opt()])
        # keep the PE clock warm while the AllToAll flies
        with ExitStack() as pW2:
            wps2 = pW2.enter_context(tc.tile_pool(name="wps2", bufs=2,
                                                  space="PSUM"))
            for i in range(250):
                wp_ = wps2.tile([128, 256], F32, name="wrm2", tag="wrm2")
                nc.tensor.matmul(wp_[:], ones_bf[:], cos_sb[:, 0:256],
                                 start=True, stop=True)

        # ===== phase C: token-parallel dense + residual + rmsnorm =====
        hold = top.enter_context(ExitStack())
        hp2 = hold.enter_context(tc.tile_pool(name="hp2", bufs=1))
        h1_sb = hp2.tile([128, 16, 256], F32)
        h1n_sb = hp2.tile([128, 16, 256], BF16)
        h2_sb = hp2.tile([128, 16, 256], F32)
        h2n_sb = hp2.tile([128, 16, 256], BF16)
        with ExitStack() as pC:
            cp = pC.enter_context(tc.tile_pool(name="cp", bufs=1))
            re_sb = cp.tile([128, 16, 256], F32)
            nc.sync.dma_start(re_sb[:], r128(resid.ap()))
            ctx_sb = cp.tile([128, 16, 256], BF16)
            nc.sync.dma_start(ctx_sb[:], r128(ctx_all[:]))
            wdp = pC.enter_context(tc.tile_pool(name="wdp", bufs=3))
            dps = pC.enter_context(tc.tile_pool(name="dps", bufs=2, space="PSUM"))
            for mt in range(16):
                wt = wdp.tile([128, 16, 128], BF16, name="wdt", tag="wdt")
                nc.sync.dma_start(wt[:], r128(wdT.ap()[mt]))
                ps = dps.tile([128, 256], F32, name="dp", tag="dp")
                for kc in range(16):
                    nc.tensor.matmul(ps[:], wt[:, kc, :], ctx_sb[:, kc, :],
                                     start=(kc == 0), stop=(kc == 15))
                nc.vector.tensor_add(h1_sb[:, mt, :], ps[:], re_sb[:, mt, :])
            pss = dps.tile([128, 256], F32, name="np1", tag="dp")
            for kc in range(16):
                sq = scrp.tile([128, 256], F32R, name="sq1", tag="sq1")
                nc.scalar.activation(sq[:], h1_sb[:, kc, :], SQ)
                nc.tensor.matmul(pss[:], ones_sb[:], sq[:],
                                 start=(kc == 0), stop=(kc == 15))
            rms = scrp.tile([128, 256], F32, name="rms1", tag="rms1")
            nc.scalar.activation(rms[:], pss[:], SQRT, scale=1.0 / H,
                                 bias=eps_sb[:])
            rinv = cp.tile([128, 256], F32)
            nc.vector.reciprocal(rinv[:], rms[:])
            for kc in range(16):
                nc.vector.tensor_mul(h1n_sb[:, kc, :], h1_sb[:, kc, :], rinv[:])
            dmp = pC.enter_context(tc.tile_pool(name="dmp", bufs=1,
                                                space="PSUM"))
            for i in range(25):
                wd_ = dmp.tile([128, 256], F32, name="nwd", tag="nwd")
                nc.tensor.matmul(wd_[:], ones_bf[:], cos_sb[:, 0:256],
                                 start=True, stop=True)

            # cq projection (token-parallel): [CC, 256]
            cqp = pC.enter_context(tc.tile_pool(name="cqp", bufs=1))
            cq_sb = cqp.tile([128, 8, 256], BF16)
            for mt in range(8):
                wt = wdp.tile([128, 16, 128], BF16, name="wcqt", tag="wdt")
                nc.sync.dma_start(wt[:], r128(wcqT.ap()[mt]))
                ps = dps.tile([128, 256], F32, name="cqp", tag="dp")
                for kc in range(16):
                    nc.tensor.matmul(ps[:], wt[:, kc, :], h1n_sb[:, kc, :],
                                     start=(kc == 0), stop=(kc == 15))
                nc.vector.tensor_copy(cq_sb[:, mt, :], ps[:])

            # ===== phase D: cross attention (16 heads, E keys) =====
            with ExitStack() as pD:
                kp = pD.enter_context(tc.tile_pool(name="kp", bufs=1))
                k_sb = kp.tile([128, 8, E], BF16)
                v2_sb = kp.tile([128, 16, 16, 65], BF16)
                for r in range(NC_):
                    blk = kv_all[r * 2 * CC:(r + 1) * 2 * CC, :]
                    nc.sync.dma_start(k_sb[:, :, r * 256:r * 256 + 256],
                                      r128(blk[0:CC, :]))
                    vblk = blk[CC:2 * CC, :].rearrange("(p x) n -> p (x n)",
                                                       p=256)
                    for i2 in range(2):
                        nc.sync.dma_start(
                            v2_sb[:, 2 * r + i2, :, 0:64],
                            vblk[i2 * 128:i2 * 128 + 128, :].rearrange(
                                "p (h d) -> p h d", h=16))
                nc.vector.memset(v2_sb[:, :, :, 64:65], 1.0)
                cap = pD.enter_context(tc.tile_pool(name="cap", bufs=4))
                caps = pD.enter_context(tc.tile_pool(name="caps", bufs=2,
                                                     space="PSUM"))
                cbp = pD.enter_context(tc.tile_pool(name="cbp", bufs=1,
                                                    space="PSUM"))
                cacc = pD.enter_context(tc.tile_pool(name="cacc", bufs=2,
                                                     space="PSUM"))
                cctx_sb = cqp.tile([128, 8, 256], BF16)
                for h in range(NH):
                    kch, koff = h // 2, 64 * (h % 2)
                    psctx = cacc.tile([65, 256], F32, name="cpc", tag="cpc")
                    scs2 = [None] * 2
                    prs2 = [None] * 2
                    for j in range(10):
                        if j < 8:
                            sc = caps.tile([128, 512], F32, name="csc",
                                           tag="csc")
                            for q in range(2):
                                kt = 2 * j + q
                                nc.tensor.matmul(
                                    sc[:, q * 256:q * 256 + 256],
                                    k_sb[koff:koff + 64, kch,
                                         kt * 128:kt * 128 + 128],
                                    cq_sb[koff:koff + 64, kch, :],
                                    start=True, stop=True)
                            scs2[j % 2] = sc
                        # keep the PE activity monitor warm (scalar-bound loop)
                        wdum = dmp.tile([128, 256], F32, name="cwd", tag="nwd")
                        nc.tensor.matmul(wdum[:], ones_bf[:],
                                         cos_sb[:, 0:256],
                                         start=True, stop=True)
                        if j % 2 == 0:
                            nc.tensor.matmul(wdum[:], ones_bf[:],
                                             cos_sb[:, 256:512],
                                             start=True, stop=True)
                        if j >= 1 and j - 1 < 8:
                            pr = cap.tile([128, 512], BF16, name="cpr",
                                          tag="cpr")
                            nc.scalar.activation(pr[:], scs2[(j - 1) % 2][:],
                                                 EXP, scale=CSC)
                            prs2[(j - 1) % 2] = pr
                        if j >= 2:
                            pr = prs2[(j - 2) % 2]
                            for q in range(2):
                                kt = 2 * (j - 2) + q
                                nc.tensor.matmul(
                                    psctx[:],
                                    v2_sb[:, kt, h, :],
                                    pr[:, q * 256:q * 256 + 256],
                                    start=(kt == 0), stop=(kt == 15))
                    den_r = cap.tile([1, 256], F32R, name="crd", tag="crd")
                    nc.scalar.copy(den_r[:], psctx[64:65, :])
                    bc = cbp.tile([64, 256], F32, name="cbc", tag="cbc")
                    nc.tensor.matmul(bc[:], ones_sb[0:1, 0:64], den_r[:],
                                     start=True, stop=True)
                    rc = cap.tile([64, 256], F32, name="crc", tag="crc")
                    nc.vector.reciprocal(rc[:], bc[:])
                    nc.vector.tensor_mul(cctx_sb[koff:koff + 64, kch, :],
                                         psctx[0:64, :], rc[:])
                # cdense + residual -> h2, rmsnorm -> h2n
                for mt in range(16):
                    wt = wdp.tile([128, 8, 128], BF16, name="wcdt", tag="wcdt")
                    nc.sync.dma_start(wt[:], r128(wcdT.ap()[mt]))
                    ps = dps.tile([128, 256], F32, name="cdp", tag="dp")
                    for kc in range(8):
                        nc.tensor.matmul(ps[:], wt[:, kc, :],
                                         cctx_sb[:, kc, :],
                                         start=(kc == 0), stop=(kc == 7))
                    nc.vector.tensor_add(h2_sb[:, mt, :], ps[:],
                                         h1_sb[:, mt, :])
                pss2 = dps.tile([128, 256], F32, name="np2", tag="dp")
                for kc in range(16):
                    sq = scrp.tile([128, 256], F32R, name="sq2", tag="sq1")
                    nc.scalar.activation(sq[:], h2_sb[:, kc, :], SQ)
                    nc.tensor.matmul(pss2[:], ones_sb[:], sq[:],
                                     start=(kc == 0), stop=(kc == 15))
                rms2 = scrp.tile([128, 256], F32, name="rms2", tag="rms1")
                nc.scalar.activation(rms2[:], pss2[:], SQRT, scale=1.0 / H,
                                     bias=eps_sb[:])
                rinv2 = cp.tile([128, 256], F32)
                nc.vector.reciprocal(rinv2[:], rms2[:])
                for kc in range(16):
                    nc.vector.tensor_mul(h2n_sb[:, kc, :], h2_sb[:, kc, :],
                                         rinv2[:])
                for i in range(25):
                    wd_ = dmp.tile([128, 256], F32, name="nwd", tag="nwd")
                    nc.tensor.matmul(wd_[:], ones_bf[:], cos_sb[:, 0:256],
                                     start=True, stop=True)
        # ===== phase F: token-parallel MLP (one expert, full I) =====
        with ExitStack() as pF:
            fw = pF.enter_context(tc.tile_pool(name="fw", bufs=3))
            fdw = pF.enter_context(tc.tile_pool(name="fdw", bufs=4))
            fps = pF.enter_context(tc.tile_pool(name="fps", bufs=2, space="PSUM"))
            fpd = pF.enter_context(tc.tile_pool(name="fpd", bufs=2, space="PSUM"))
            fac = pF.enter_context(tc.tile_pool(name="fac", bufs=1))
            fout = pF.enter_context(tc.tile_pool(name="fout", bufs=3))
            act = fac.tile([128, NPI, 256], BF16)
            for pi in range(NPI):
                gwt = fw.tile([128, 16, 256], BF16, name="guw", tag="guw")
                nc.sync.dma_start(gwt[:], r128(wguI.ap()[pi]))
                pg = fps.tile([128, 512], F32, name="pg", tag="pg")
                for kc in range(16):
                    nc.tensor.matmul(pg[:, 0:256], gwt[:, kc, 0:128],
                                     h2n_sb[:, kc, :],
                                     start=(kc == 0), stop=(kc == 15))
                for kc in range(16):
                    nc.tensor.matmul(pg[:, 256:512], gwt[:, kc, 128:256],
                                     h2n_sb[:, kc, :],
                                     start=(kc == 0), stop=(kc == 15))
                gs = scrp.tile([128, 256], F32, name="gs", tag="gs")
                nc.scalar.activation(gs[:], pg[:, 0:256], SILU)
                nc.vector.tensor_mul(act[:, pi, :], gs[:], pg[:, 256:512])
            for mt in range(16):
                dwt = fdw.tile([128, NPI, 128], BF16, name="dnw", tag="dnw")
                nc.sync.dma_start(dwt[:], r128(wdnT.ap()[mt]))
                pd = fpd.tile([128, 256], F32, name="pd", tag="pd")
                for kc in range(NPI):
                    nc.tensor.matmul(pd[:], dwt[:, kc, :], act[:, kc, :],
                                     start=(kc == 0), stop=(kc == NPI - 1))
                ot = fout.tile([128, 256], F32, name="fot", tag="fot")
                nc.vector.tensor_add(ot[:], pd[:], h2_sb[:, mt, :])
                nc.sync.dma_start(y.ap()[mt * 128:mt * 128 + 128, :], ot[:])
        hold.close()
    nc.compile()
    return nc


_CACHE = {}


def _prep_common(inputs):
    import ml_dtypes
    vm = np.asarray(inputs["vision_token_ids"]).astype(bool)
    lm = np.asarray(inputs["language_token_ids"]).astype(bool)
    g0 = np.where(vm & ~lm)[0]; g1 = np.where(vm & lm)[0]
    g2 = np.where(~vm & lm)[0]; g3 = np.where(~vm & ~lm)[0]
    perm = np.concatenate([g0, g1, g2, g3])
    b0 = len(g0); b1 = b0 + len(g1); b2 = b1 + len(g2)
    return perm, b0, b1, b2


def kernel(**inputs):
    import ml_dtypes
    perm, b0, b1, b2 = _prep_common(inputs)
    fast = (b0 == b1) and (b2 == S) and (b1 % 256 == 0)
    if not fast:
        return _kernel_general(inputs, perm, b0, b1, b2)

    f32 = lambda x: np.ascontiguousarray(np.asarray(x, np.float32))
    bf = lambda x: np.ascontiguousarray(np.asarray(x).astype(ml_dtypes.bfloat16))
    pos = np.asarray(inputs["positions"]).astype(np.float32)
    half = HD // 2
    inv_freq = 1.0 / (ROPE_BASE ** (np.arange(half, dtype=np.float32) / half))
    fr = pos[:, None] * inv_freq[None, :]
    cos2 = np.concatenate([np.cos(fr)] * 2, 1).T[:, perm]
    sin2 = np.concatenate([np.sin(fr)] * 2, 1).T[:, perm]
    rot = np.zeros((HD, HD), np.float32)
    rot[np.arange(half), np.arange(half) + half] = -1.0
    rot[np.arange(half) + half, np.arange(half)] = 1.0
    op = np.asarray(inputs["positions"])[perm]

    # block-sparsity pattern + multiplicative masks for partial tiles
    vis = op[None, :] >= op[:, None]          # [key, query]
    pattern = []
    slots = []
    for ci, (t0, t1) in enumerate(_chunks(0, S, 512)):
        lst = []
        for kt in range(16):
            blk = vis[kt * 128:kt * 128 + 128, t0:t1]
            if not blk.any():
                continue
            if blk.all():
                lst.append((kt, -1))
            else:
                lst.append((kt, len(slots)))
                slots.append(blk)
        pattern.append(tuple(lst))
    pattern = tuple(pattern)
    nslot = len(slots)
    if nslot:
        mk = np.stack(slots).astype(np.float32)      # [n, 128, 512]
        maskm = bf(mk.transpose(1, 0, 2))            # [128, n, 512]
    else:
        maskm = np.zeros((128, 1, 512), ml_dtypes.bfloat16)

    wln_in = f32(inputs["w_ln_in"])[:, None]
    wln_pa = f32(inputs["w_ln_post_attn"])[:, None]
    wln_pc = f32(inputs["w_ln_post_cross"])[:, None]
    wqkv = [f32(inputs["w_vis_qkv"]) * wln_in, f32(inputs["w_lang_qkv"]) * wln_in]
    wd = [f32(inputs["w_vis_dense"]), f32(inputs["w_lang_dense"])]
    wgu = [f32(inputs["w_vis_gate_up"]) * wln_pc,
           f32(inputs["w_lang_gate_up"]) * wln_pc]
    wdn = [f32(inputs["w_vis_down"]), f32(inputs["w_lang_down"])]
    wkvf = f32(inputs["w_cross_kv"])
    hTp = f32(inputs["hidden_states"]).T[:, perm].copy()
    encT = bf(f32(inputs["encoder_embeds"]).T)

    def mtblocks(w, nb):  # w [K, nb*128] -> [nb, K, 128]
        return np.ascontiguousarray(
            w.reshape(w.shape[0], nb, 128).transpose(1, 0, 2))

    # per-expert big weights (shared across cores of the same expert)
    wdT_e = [bf(mtblocks(wd[e], 16)) for e in range(2)]
    wguI_e = []
    for e in range(2):
        g = wgu[e][:, :I].reshape(H, NPI, 128)
        u = wgu[e][:, I:].reshape(H, NPI, 128)
        blk = np.concatenate([g, u], axis=2)         # [H, NPI, 256]
        wguI_e.append(bf(np.ascontiguousarray(blk.transpose(1, 0, 2))))
    wdnT_e = [bf(mtblocks(wdn[e], 16)) for e in range(2)]
    wcqT = bf(mtblocks(f32(inputs["w_cross_q"]) * wln_pa, 8))
    wcdT = bf(mtblocks(f32(inputs["w_cross_dense"]), 16))
    wkb = bf(wkvf[:, :CC]); wvb = bf(wkvf[:, CC:])
    hTb = bf(hTp)
    cos2b = bf(cos2); sin2b = bf(sin2); rotb = bf(rot.T)
    onesr = np.ones((128, 128), np.float32)
    onesb = np.ones((128, 128), ml_dtypes.bfloat16)

    key = (b0, b1, b2, pattern)
    if _CACHE.get("key") != key:
        _CACHE.clear()
        _CACHE["key"] = key
        _CACHE["nc"] = build_fast(b0, b1, b2, pattern, nslot)
    nc = _CACHE["nc"]

    in_maps = []
    for c in range(NC_):
        qs = slice(256 * c, 256 * c + 256)
        ex = 0 if 256 * (c + 1) <= b1 else 1
        m = dict(
            hT=hTb, resid=hTp[:, qs].copy(),
            wqkv0=bf(np.concatenate([wqkv[0][:, 256 * c:256 * c + 256],
                                     wqkv[0][:, H:][:, qs],
                                     wqkv[0][:, 2 * H:][:, qs]], 1)),
            wqkv1=bf(np.concatenate([wqkv[1][:, qs],
                                     wqkv[1][:, H:][:, qs],
                                     wqkv[1][:, 2 * H:][:, qs]], 1)),
            cos2=cos2b, sin2=sin2b, rotT=rotb,
            onesr=onesr, onesb=onesb, maskm=maskm,
            encsl=np.ascontiguousarray(encT[:, qs]),
            wk=wkb, wvv=wvb,
            wdT=wdT_e[ex], wcqT=wcqT, wcdT=wcdT,
            wguI=wguI_e[ex], wdnT=wdnT_e[ex],
        )
        in_maps.append(m)

    trace = bool(int(os.environ.get("KTRACE", "0")))
    res = run_bass_kernel_spmd(nc, in_maps, core_ids=list(range(NC_)),
                               trace=trace)
    kernel.last_exec_ns = res.exec_time_ns
    out = np.empty((S, H), np.float32)
    for c in range(NC_):
        out[perm[256 * c:256 * c + 256], :] = res.results[c]["y"].T
    return out


# ============ general fallback path (original kernel) ================

def build_general(b0, b1, b2):
    nc = bacc.Bacc("TRN2", target_bir_lowering=False, debug=False,
                   num_devices=NC_)
    din = lambda n, sh, dt: nc.dram_tensor(n, sh, dt, kind="ExternalInput")
    hT = din("hT", [H, S], BF16)
    wqkv0 = din("wqkv0", [H, 768], BF16)
    wqkv1 = din("wqkv1", [H, 768], BF16)
    wd0 = din("wd0", [256, H], F32R)
    wd1 = din("wd1", [256, H], F32R)
    cos2 = din("cos2", [128, S], BF16)
    sin2 = din("sin2", [128, S], BF16)
    rotT = din("rotT", [128, 128], BF16)
    onesr = din("onesr", [128, 128], F32R)
    onesb = din("onesb", [128, 128], BF16)
    zeros = din("zeros", [128, 512], F32R)
    maskneg = din("maskneg", [S, S], BF16)
    resid = din("resid", [H, 256], F32R)
    encT = din("encT", [CH, E], BF16)
    wk = din("wk", [CH, CC], BF16)
    wvv = din("wvv", [CH, CC], BF16)
    wcq = din("wcq", [H, CC], F32R)
    wcd = din("wcd", [CC, H], F32R)
    wgu0 = din("wgu0", [H, 2 * IS], BF16)
    wgu1 = din("wgu1", [H, 2 * IS], BF16)
    wdn0 = din("wdn0", [ISP, H], BF16)
    wdn1 = din("wdn1", [ISP, H], BF16)
    y = nc.dram_tensor("y", [H, S], F32, kind="ExternalOutput")

    SC = 1.0 / float(np.sqrt(HD))
    CSC = 1.0 / float(np.sqrt(CHD))
    EXP = mybir.ActivationFunctionType.Exp
    SQ = mybir.ActivationFunctionType.Square
    SQRT = mybir.ActivationFunctionType.Sqrt
    SILU = mybir.ActivationFunctionType.Silu
    r128 = lambda ap: ap.rearrange("(c p) n -> p c n", p=128)

    with tile.TileContext(nc) as tc, ExitStack() as top:
        const = top.enter_context(tc.tile_pool(name="const", bufs=1))
        ones_sb = const.tile([128, 128], F32R)
        nc.sync.dma_start(ones_sb[:], onesr.ap()[:])
        ones_bf = const.tile([128, 128], BF16)
        nc.sync.dma_start(ones_bf[:], onesb.ap()[:])
        rot_sb = const.tile([128, 128], BF16)
        nc.sync.dma_start(rot_sb[:], rotT.ap()[:])
        from concourse.masks import make_identity
        ident = const.tile([128, 128], BF16)
        make_identity(nc, ident[:])
        cos_sb = const.tile([128, S], BF16)
        nc.sync.dma_start(cos_sb[:], cos2.ap()[:])
        sin_sb = const.tile([128, S], BF16)
        nc.sync.dma_start(sin_sb[:], sin2.ap()[:])
        zer_sb = const.tile([128, 512], F32R)
        nc.sync.dma_start(zer_sb[:], zeros.ap()[:])
        eps_sb = const.tile([128, 1], F32)
        nc.vector.memset(eps_sb[:], EPS)

        dram = top.enter_context(tc.tile_pool(name="dram", bufs=1, space="DRAM"))
        bounce = dram.tile([NC_ * H, 256], F32)
        rs_out = dram.tile([H, 256], F32)
        h2n_bnc = dram.tile([H, 256], BF16)
        h2n_all = dram.tile([NC_ * H, 256], BF16, addr_space="Shared")
        h2out = nc.dram_tensor("h2out", [H, 256], F32, kind="ExternalOutput")

        scrp = top.enter_context(tc.tile_pool(name="scr", bufs=2))

        def vwrite(op, dst, a, bb):
            if DVE_F32R:
                op(dst, a, bb)
            else:
                scr = scrp.tile([dst.shape[0], dst.shape[-1]], F32,
                                name="vscr", tag="vscr")
                op(scr[:], a, bb)
                nc.scalar.copy(dst, scr[:])

        pABC = top.enter_context(ExitStack())
        qkp = pABC.enter_context(tc.tile_pool(name="qkp", bufs=1))
        qkv_sb = qkp.tile([128, 6, S], BF16)      # q0 q1 k0 k1 v0 v1
        v_sb = qkp.tile([128, 16, 256], BF16)     # token-major v
        ctxp = pABC.enter_context(tc.tile_pool(name="ctxp", bufs=1))
        ctx_sb = ctxp.tile([128, 2, S], F32R)

        # ===== phase A: h load + rmsnorm + QKV + rope + vT =====
        with ExitStack() as pA:
            hp = pA.enter_context(tc.tile_pool(name="hp", bufs=1))
            h_sb = hp.tile([128, 16, S], BF16)
            nc.sync.dma_start(h_sb[:], r128(hT.ap()))
            with ExitStack() as pA1:
                nrm = pA1.enter_context(tc.tile_pool(name="nrm", bufs=2))
                nps = pA1.enter_context(tc.tile_pool(name="nps", bufs=2,
                                                     space="PSUM"))
                for t0, t1 in _chunks(0, S, 512):
                    pss = nps.tile([128, 512], F32, name="pss", tag="pss")
                    for kc in range(16):
                        sq = nrm.tile([128, 512], F32R, name="sq", tag="sq")
                        nc.scalar.activation(sq[:], h_sb[:, kc, t0:t1], SQ)
                        nc.tensor.matmul(pss[:], ones_sb[:], sq[:],
                                         start=(kc == 0), stop=(kc == 15))
                    rms = nrm.tile([128, 512], F32, name="rms", tag="rms")
                    nc.scalar.activation(rms[:], pss[:], SQRT,
                                         scale=1.0 / H, bias=eps_sb[:])
                    rinv = nrm.tile([128, 512], F32, name="rinv", tag="rinv")
                    nc.vector.reciprocal(rinv[:], rms[:])
                    for kc in range(16):
                        nc.vector.tensor_mul(h_sb[:, kc, t0:t1],
                                             h_sb[:, kc, t0:t1], rinv[:])
            with ExitStack() as pA2:
                wp = pA2.enter_context(tc.tile_pool(name="wp", bufs=3))
                mps = pA2.enter_context(tc.tile_pool(name="mps", bufs=2,
                                                     space="PSUM"))
                for slot in range(6):
                    wts = []
                    for ex, wsrc in ((0, wqkv0), (1, wqkv1)):
                        wt = wp.tile([128, 16, 128], BF16,
                                     name=f"wq{ex}{slot}", tag=f"wq{ex}")
                        nc.sync.dma_start(
                            wt[:], r128(wsrc.ap()[:, slot * 128:slot * 128 + 128]))
                        wts.append(wt)
                    for t0, t1 in _chunks(0, S, 512):
                        sg = [x for x in _segs(t0, t1, b0, b1, b2) if x[2]]
                        if not sg:
                            continue
                        need = sorted({x for _, _, ex in sg for x in ex})
                        pss_ = {}
                        for x in need:
                            ps = mps.tile([128, 512], F32, name=f"qps{x}",
                                          tag=f"qps{x}")
                            for kc in range(16):
                                nc.tensor.matmul(ps[:], wts[x][:, kc, :],
                                                 h_sb[:, kc, t0:t1],
                                                 start=(kc == 0), stop=(kc == 15))
                            pss_[x] = ps
                        for s, e, ex in sg:
                            if len(ex) == 1:
                                nc.vector.tensor_copy(qkv_sb[:, slot, s:e],
                                                      pss_[ex[0]][:, s - t0:e - t0])
                            else:
                                nc.vector.tensor_add(qkv_sb[:, slot, s:e],
                                                     pss_[0][:, s - t0:e - t0],
                                                     pss_[1][:, s - t0:e - t0])
                    if b2 < S:
                        nc.vector.memset(qkv_sb[:, slot, b2:S], 0.0)
                # rope on q,k
                for slot in range(4):
                    for t0, t1 in _chunks(0, S, 512):
                        rp = mps.tile([128, 512], F32, name="rps", tag="qps")
                        nc.tensor.matmul(rp[:], rot_sb[:],
                                         qkv_sb[:, slot, t0:t1],
                                         start=True, stop=True)
                        c1 = scrp.tile([128, 512], F32, name="ropec", tag="ropec")
                        nc.vector.tensor_mul(c1[:], qkv_sb[:, slot, t0:t1],
                                             cos_sb[:, t0:t1])
                        s1 = scrp.tile([128, 512], F32, name="ropes", tag="ropes")
                        nc.vector.tensor_mul(s1[:], rp[:], sin_sb[:, t0:t1])
                        nc.vector.tensor_add(qkv_sb[:, slot, t0:t1],
                                             c1[:], s1[:])
                # v -> token-major via PE transpose
                for hh in range(2):
                    for tt in range(16):
                        tp = mps.tile([128, 512], BF16, name="tps", tag="qps")
                        nc.tensor.transpose(
                            tp[:, :128],
                            qkv_sb[:, 4 + hh, tt * 128:tt * 128 + 128],
                            ident[:])
                        nc.vector.tensor_copy(v_sb[:, tt, hh * 128:hh * 128 + 128],
                                              tp[:, :128])
        # ===== phase B: self-attention (perm order) =====
        with ExitStack() as pB:
            ap_ = pB.enter_context(tc.tile_pool(name="ap", bufs=3))
            aps = pB.enter_context(tc.tile_pool(name="aps", bufs=2, space="PSUM"))
            accp = pB.enter_context(tc.tile_pool(name="accp", bufs=1, space="PSUM"))
            for t0, t1 in _chunks(0, S, 512):
                pss_ = [accp.tile([128, 512], F32, name=f"pssum{h}", tag=f"pssum{h}")
                        for h in range(2)]
                psc_ = [accp.tile([128, 512], F32, name=f"psctx{h}", tag=f"psctx{h}")
                        for h in range(2)]
                for kt in range(16):
                    mt_ = ap_.tile([128, 512], BF16, name="mt", tag="mt")
                    nc.sync.dma_start(
                        mt_[:], maskneg.ap()[kt * 128:kt * 128 + 128, t0:t1])
                    for hh in range(2):
                        sc = aps.tile([128, 512], F32, name="sc", tag="sc")
                        nc.tensor.matmul(
                            sc[:], qkv_sb[:, 2 + hh, kt * 128:kt * 128 + 128],
                            qkv_sb[:, hh, t0:t1], start=True, stop=True)
                        nc.vector.tensor_add(sc[:], sc[:], mt_[:])
                        pr = ap_.tile([128, 512], BF16, name="pr", tag="pr")
                        nc.scalar.activation(pr[:], sc[:], EXP, scale=SC)
                        nc.tensor.matmul(pss_[hh][:], ones_bf[:], pr[:],
                                         start=(kt == 0), stop=(kt == 15))
                        nc.tensor.matmul(
                            psc_[hh][:], v_sb[:, kt, hh * 128:hh * 128 + 128],
                            pr[:], start=(kt == 0), stop=(kt == 15))
                for hh in range(2):
                    rc = ap_.tile([128, 512], F32, name="rc", tag="rc")
                    nc.vector.reciprocal(rc[:], pss_[hh][:])
                    vwrite(nc.vector.tensor_mul, ctx_sb[:, hh, t0:t1],
                           psc_[hh][:], rc[:])
        # ===== phase C: dense (routed) -> bounce -> RS =====
        with ExitStack() as pC:
            dwp = pC.enter_context(tc.tile_pool(name="dwp", bufs=1))
            dps = pC.enter_context(tc.tile_pool(name="dps", bufs=2, space="PSUM"))
            dop = pC.enter_context(tc.tile_pool(name="dop", bufs=4))
            dwts = []
            for ex, wsrc in ((0, wd0), (1, wd1)):
                dwt = dwp.tile([128, 2, H], F32R, name=f"dw{ex}", tag=f"dw{ex}")
                nc.sync.dma_start(dwt[:], r128(wsrc.ap()))
                dwts.append(dwt)
            for tt in range(8):
                t0, t1 = tt * 256, tt * 256 + 256
                sg = _segs(t0, t1, b0, b1, b2)
                live = [x for x in sg if x[2]]
                for mt in range(16):
                    ot = dop.tile([128, 256], F32, name="dot", tag="dot")
                    if live:
                        need = sorted({x for _, _, ex in live for x in ex})
                        pss_ = {}
                        for x in need:
                            ps = dps.tile([128, 256], F32, name=f"dpst{x}",
                                          tag=f"dpst{x}")
                            for kc in range(2):
                                nc.tensor.matmul(
                                    ps[:],
                                    dwts[x][:, kc, mt * 128:mt * 128 + 128],
                                    ctx_sb[:, kc, t0:t1],
                                    start=(kc == 0), stop=(kc == 1))
                            pss_[x] = ps
                        for s, e, ex in sg:
                            if len(ex) == 2:
                                nc.vector.tensor_add(ot[:, s - t0:e - t0],
                                                     pss_[0][:, s - t0:e - t0],
                                                     pss_[1][:, s - t0:e - t0])
                            elif ex:
                                nc.vector.tensor_copy(ot[:, s - t0:e - t0],
                                                      pss_[ex[0]][:, s - t0:e - t0])
                            else:
                                nc.vector.memset(ot[:, s - t0:e - t0], 0.0)
                    else:
                        nc.vector.memset(ot[:], 0.0)
                    nc.sync.dma_start(
                        bounce[tt * H + mt * 128: tt * H + mt * 128 + 128, :],
                        ot[:])
        pABC.close()
        nc.gpsimd.collective_compute(
            "ReduceScatter", mybir.AluOpType.add,
            replica_groups=[list(range(NC_))],
            ins=[bounce.opt()], outs=[rs_out.opt()])

        # ===== phase D: cross attention (token-parallel) =====
        with ExitStack() as pD:
            dp = pD.enter_context(tc.tile_pool(name="dp", bufs=1))
            dps2 = pD.enter_context(tc.tile_pool(name="dps2", bufs=2, space="PSUM"))
            h1_sb = dp.tile([128, 16, 256], F32R)
            cq_sb = dp.tile([128, 8, 256], BF16)
            cctx_sb = dp.tile([128, 8, 256], F32R)
            with ExitStack() as pD1:
                d1 = pD1.enter_context(tc.tile_pool(name="d1", bufs=1))
                rs_sb = d1.tile([128, 16, 256], F32)
                nc.sync.dma_start(rs_sb[:], r128(rs_out[:]))
                re_sb = d1.tile([128, 16, 256], F32R)
                nc.sync.dma_start(re_sb[:], r128(resid.ap()))
                for kc in range(16):
                    vwrite(nc.vector.tensor_add, h1_sb[:, kc, :],
                           rs_sb[:, kc, :], re_sb[:, kc, :].bitcast(F32))
                pss = dps2.tile([128, 256], F32, name="psd", tag="psd")
                for kc in range(16):
                    sq = scrp.tile([128, 256], F32R, name="sqd", tag="sqd")
                    nc.scalar.activation(sq[:], h1_sb[:, kc, :].bitcast(F32), SQ)
                    nc.tensor.matmul(pss[:], ones_sb[:], sq[:],
                                     start=(kc == 0), stop=(kc == 15))
                rms = scrp.tile([128, 256], F32, name="rmsd", tag="rmsd")
                nc.scalar.activation(rms[:], pss[:], SQRT, scale=1.0 / H, bias=eps_sb[:])
                rinv = d1.tile([128, 256], F32)
                nc.vector.reciprocal(rinv[:], rms[:])
                h1n_sb = d1.tile([128, 16, 256], F32R)
                for kc in range(16):
                    vwrite(nc.vector.tensor_mul, h1n_sb[:, kc, :],
                           h1_sb[:, kc, :].bitcast(F32), rinv[:])
                for mt in range(8):
                    wcq_t = d1.tile([128, 16, 128], F32R, name="wcqt", tag="wcqt",
                                    bufs=2)
                    nc.sync.dma_start(
                        wcq_t[:], r128(wcq.ap()[:, mt * 128:mt * 128 + 128]))
                    ps = dps2.tile([128, 256], F32, name="cqp", tag="psd")
                    for kc in range(16):
                        nc.tensor.matmul(ps[:],
                                         wcq_t[:, kc, :],
                                         h1n_sb[:, kc, :],
                                         start=(kc == 0), stop=(kc == 15))
                    nc.vector.tensor_copy(cq_sb[:, mt, :], ps[:])
            with ExitStack() as pD2:
                kp = pD2.enter_context(tc.tile_pool(name="kp", bufs=1))
                k_sb = kp.tile([128, 8, E], BF16)
                v_sb2 = kp.tile([128, 16, CC], BF16)
                with ExitStack() as pD2e:
                    ep = pD2e.enter_context(tc.tile_pool(name="ep", bufs=1))
                    enc_sb = ep.tile([128, 8, E], BF16)
                    nc.sync.dma_start(enc_sb[:], r128(encT.ap()))
                    wk_sb = ep.tile([128, 8, CC], BF16)
                    nc.sync.dma_start(wk_sb[:], r128(wk.ap()))
                    wv_sb = ep.tile([128, 8, CC], BF16)
                    nc.sync.dma_start(wv_sb[:], r128(wvv.ap()))
                    for mt in range(8):
                        for n0, n1 in _chunks(0, E, 512):
                            ps = dps2.tile([128, 512], F32, name="kps", tag="kps")
                            for kc in range(8):
                                nc.tensor.matmul(
                                    ps[:], wk_sb[:, kc, mt * 128:mt * 128 + 128],
                                    enc_sb[:, kc, n0:n1],
                                    start=(kc == 0), stop=(kc == 7))
                            nc.vector.tensor_copy(k_sb[:, mt, n0:n1], ps[:])
                    for tt in range(16):
                        for n0, n1 in _chunks(0, CC, 512):
                            ps = dps2.tile([128, 512], F32, name="vps", tag="kps")
                            for kc in range(8):
                                nc.tensor.matmul(
                                    ps[:], enc_sb[:, kc, tt * 128:tt * 128 + 128],
                                    wv_sb[:, kc, n0:n1],
                                    start=(kc == 0), stop=(kc == 7))
                            nc.vector.tensor_copy(v_sb2[:, tt, n0:n1], ps[:])
                with ExitStack() as pD3:
                    cap = pD3.enter_context(tc.tile_pool(name="cap", bufs=3))
                    caps = pD3.enter_context(tc.tile_pool(name="caps", bufs=2,
                                                          space="PSUM"))
                    cacc = pD3.enter_context(tc.tile_pool(name="cacc", bufs=1,
                                                          space="PSUM"))
                    for h in range(NH):
                        kch, koff = h // 2, 64 * (h % 2)
                        pssum = cacc.tile([128, 256], F32, name="cps", tag="cps")
                        psctx = cacc.tile([64, 256], F32, name="cpc", tag="cpc")
                        for kt in range(16):
                            sc = caps.tile([128, 256], F32, name="csc", tag="csc")
                            nc.tensor.matmul(
                                sc[:],
                                k_sb[koff:koff + 64, kch, kt * 128:kt * 128 + 128],
                                cq_sb[koff:koff + 64, kch, :],
                                start=True, stop=True)
                            pr = cap.tile([128, 256], BF16, name="cpr", tag="cpr")
                            nc.scalar.activation(pr[:], sc[:], EXP, scale=CSC)
                            nc.tensor.matmul(pssum[:], ones_bf[:], pr[:],
                                             start=(kt == 0), stop=(kt == 15))
                            nc.tensor.matmul(psctx[:],
                                             v_sb2[:, kt, 64 * h:64 * h + 64],
                                             pr[:], start=(kt == 0), stop=(kt == 15))
                        rc = cap.tile([64, 256], F32, name="crc", tag="crc")
                        nc.vector.reciprocal(rc[:], pssum[:64, :])
                        vwrite(nc.vector.tensor_mul,
                               cctx_sb[koff:koff + 64, kch, :], psctx[:], rc[:])
            # cdense + residual -> h2, rmsnorm -> h2n -> AG
            with ExitStack() as pD4:
                d4 = pD4.enter_context(tc.tile_pool(name="d4", bufs=1))
                h2_sb = d4.tile([128, 16, 256], F32)
                h2n_sb = d4.tile([128, 16, 256], BF16)
                wcd_sb = d4.tile([128, 8, H], F32R)
                nc.sync.dma_start(wcd_sb[:], r128(wcd.ap()))
                for mt in range(16):
                    ps = dps2.tile([128, 256], F32, name="cdp", tag="psd")
                    for kc in range(8):
                        nc.tensor.matmul(ps[:],
                                         wcd_sb[:, kc, mt * 128:mt * 128 + 128],
                                         cctx_sb[:, kc, :],
                                         start=(kc == 0), stop=(kc == 7))
                    nc.vector.tensor_add(h2_sb[:, mt, :], ps[:],
                                         h1_sb[:, mt, :].bitcast(F32))
                pss2 = dps2.tile([128, 256], F32, name="psd2", tag="psd")
                for kc in range(16):
                    sq = scrp.tile([128, 256], F32R, name="sqd2", tag="sqd")
                    nc.scalar.activation(sq[:], h2_sb[:, kc, :], SQ)
                    nc.tensor.matmul(pss2[:], ones_sb[:], sq[:],
                                     start=(kc == 0), stop=(kc == 15))
                rms2 = scrp.tile([128, 256], F32, name="rmsd2", tag="rmsd")
                nc.scalar.activation(rms2[:], pss2[:], SQRT,
                                     scale=1.0 / H, bias=eps_sb[:])
                rinv2 = d4.tile([128, 256], F32)
                nc.vector.reciprocal(rinv2[:], rms2[:])
                for kc in range(16):
                    nc.vector.tensor_mul(h2n_sb[:, kc, :],
                                         h2_sb[:, kc, :], rinv2[:])
                nc.sync.dma_start(r128(h2n_bnc[:]), h2n_sb[:])
                nc.sync.dma_start(r128(h2out.ap()), h2_sb[:])
            nc.gpsimd.collective_compute(
                "AllGather", mybir.AluOpType.bypass,
                replica_groups=[list(range(NC_))],
                ins=[h2n_bnc.opt()], outs=[h2n_all.opt()])
        # ===== phase F: MLP (routed by expert ranges, bf16) =====
        with ExitStack() as pF:
            fp = pF.enter_context(tc.tile_pool(name="fp", bufs=1))
            hn_sb = fp.tile([128, 16, S], BF16)
            for r in range(NC_):
                nc.sync.dma_start(hn_sb[:, :, r * 256:r * 256 + 256],
                                  r128(h2n_all[r * H:(r + 1) * H, :]))
            fw = pF.enter_context(tc.tile_pool(name="fw", bufs=1))
            fps = pF.enter_context(tc.tile_pool(name="fps", bufs=1, space="PSUM"))
            fpd = pF.enter_context(tc.tile_pool(name="fpd", bufs=2, space="PSUM"))
            fac = pF.enter_context(tc.tile_pool(name="fac", bufs=2))
            fout = pF.enter_context(tc.tile_pool(name="fout", bufs=4))
            for ex, (lo, hi) in ((0, (0, b1)), (1, (b1, S))):
                gsrc = (wgu0, wgu1)[ex]
                dsrc = (wdn0, wdn1)[ex]
                dn_t = fw.tile([128, 6, H], BF16, name=f"dn{ex}", tag="dn")
                nc.sync.dma_start(dn_t[:], r128(dsrc.ap()))
                gwts = []
                for pi in range(6):
                    gw = 128 if pi < 5 else 48
                    gwt = fw.tile([128, 16, 256], BF16,
                                  name=f"guw{ex}{pi}", tag=f"guw{pi}")
                    nc.sync.dma_start(
                        gwt[:, :, :2 * gw],
                        r128(gsrc.ap()[:, pi * 256:pi * 256 + 2 * gw]))
                    gwts.append(gwt)
                for a0 in range(0, S, 512):
                    c0, c1 = max(a0, lo), min(a0 + 512, hi)
                    if c0 >= c1:
                        continue
                    t0_, W = a0, 512
                    eo, ew = c0 - a0, c1 - c0
                    act = fac.tile([128, 6, 512], BF16, name="act", tag="act")
                    for pi in range(6):
                        gw = 128 if pi < 5 else 48
                        gwt = gwts[pi]
                        pg = fps.tile([128, 512], F32, name="pg", tag="pg")
                        pu = fps.tile([128, 512], F32, name="pu", tag="pu")
                        for kc in range(16):
                            nc.tensor.matmul(pg[:gw, :W], gwt[:, kc, :gw],
                                             hn_sb[:, kc, t0_:t0_ + 512],
                                             start=(kc == 0), stop=(kc == 15))
                            nc.tensor.matmul(pu[:gw, :W], gwt[:, kc, gw:2 * gw],
                                             hn_sb[:, kc, t0_:t0_ + 512],
                                             start=(kc == 0), stop=(kc == 15))
                        gs = scrp.tile([128, 512], F32, name="gs", tag="gs")
                        nc.scalar.activation(gs[:gw, :W], pg[:gw, :W], SILU)
                        nc.vector.tensor_mul(act[:gw, pi, :W],
                                             gs[:gw, :W], pu[:gw, :W])
                    for mt in range(16):
                        pd = fpd.tile([128, 512], F32, name="pd", tag="pd")
                        for pi in range(6):
                            kw = 128 if pi < 5 else 48
                            nc.tensor.matmul(
                                pd[:, :W],
                                dn_t[:kw, pi, mt * 128:mt * 128 + 128],
                                act[:kw, pi, :W],
                                start=(pi == 0), stop=(pi == 5))
                        ot = fout.tile([128, 512], F32, name="fot", tag="fot")
                        nc.vector.tensor_copy(ot[:, eo:eo + ew], pd[:, eo:eo + ew])
                        nc.sync.dma_start(
                            y.ap()[mt * 128:mt * 128 + 128, c0:c1],
                            ot[:, eo:eo + ew])
    nc.compile()
    return nc


def _kernel_general(inputs, perm, b0, b1, b2):
    import ml_dtypes
    f32 = lambda x: np.ascontiguousarray(np.asarray(x, np.float32))
    bf = lambda x: np.ascontiguousarray(np.asarray(x).astype(ml_dtypes.bfloat16))
    pos = np.asarray(inputs["positions"]).astype(np.float32)
    half = HD // 2
    inv_freq = 1.0 / (ROPE_BASE ** (np.arange(half, dtype=np.float32) / half))
    fr = pos[:, None] * inv_freq[None, :]
    cos2 = np.concatenate([np.cos(fr)] * 2, 1).T[:, perm]
    sin2 = np.concatenate([np.sin(fr)] * 2, 1).T[:, perm]
    rot = np.zeros((HD, HD), np.float32)
    rot[np.arange(half), np.arange(half) + half] = -1.0
    rot[np.arange(half) + half, np.arange(half)] = 1.0
    op = np.asarray(inputs["positions"])[perm]
    maskneg = np.where(op[None, :] >= op[:, None], 0.0, -30000.0)

    wln_in = f32(inputs["w_ln_in"])[:, None]
    wln_pa = f32(inputs["w_ln_post_attn"])[:, None]
    wln_pc = f32(inputs["w_ln_post_cross"])[:, None]
    wqkv = [f32(inputs["w_vis_qkv"]) * wln_in, f32(inputs["w_lang_qkv"]) * wln_in]
    wd = [f32(inputs["w_vis_dense"]), f32(inputs["w_lang_dense"])]
    wgu = [f32(inputs["w_vis_gate_up"]) * wln_pc,
           f32(inputs["w_lang_gate_up"]) * wln_pc]
    wdn = [f32(inputs["w_vis_down"]), f32(inputs["w_lang_down"])]
    wkvf = f32(inputs["w_cross_kv"])
    hTp = f32(inputs["hidden_states"]).T[:, perm].copy()

    def interleave(w):  # w [H, 2*IS] = [gate | up]
        cols = []
        for i in range(5):
            cols.append(w[:, 128 * i:128 * i + 128])
            cols.append(w[:, IS + 128 * i:IS + 128 * i + 128])
        cols.append(w[:, 640:IS]); cols.append(w[:, IS + 640:2 * IS])
        return np.ascontiguousarray(np.concatenate(cols, 1))

    key = ("general", b0, b1, b2)
    if _CACHE.get("key") != key:
        _CACHE.clear()
        _CACHE["key"] = key
        _CACHE["nc"] = build_general(b0, b1, b2)
    nc = _CACHE["nc"]

    in_maps = []
    for c in range(NC_):
        qs = slice(256 * c, 256 * c + 256)
        m = dict(
            hT=bf(hTp),
            wqkv0=bf(np.concatenate([wqkv[0][:, qs], wqkv[0][:, H:][:, qs],
                                     wqkv[0][:, 2 * H:][:, qs]], 1)),
            wqkv1=bf(np.concatenate([wqkv[1][:, qs], wqkv[1][:, H:][:, qs],
                                     wqkv[1][:, 2 * H:][:, qs]], 1)),
            wd0=wd[0][qs].copy(), wd1=wd[1][qs].copy(),
            cos2=bf(cos2), sin2=bf(sin2), rotT=bf(rot.T),
            onesr=np.ones((128, 128), np.float32),
            onesb=np.ones((128, 128), ml_dtypes.bfloat16),
            zeros=np.zeros((128, 512), np.float32),
            maskneg=bf(maskneg), resid=hTp[:, qs].copy(),
            encT=bf(f32(inputs["encoder_embeds"]).T),
            wk=bf(wkvf[:, :CC]), wvv=bf(wkvf[:, CC:]),
            wcq=(f32(inputs["w_cross_q"]) * wln_pa).copy(),
            wcd=f32(inputs["w_cross_dense"]),
            wgu0=bf(interleave(np.concatenate(
                [wgu[0][:, IS * c:IS * c + IS],
                 wgu[0][:, I + IS * c:I + IS * c + IS]], 1))),
            wgu1=bf(interleave(np.concatenate(
                [wgu[1][:, IS * c:IS * c + IS],
                 wgu[1][:, I + IS * c:I + IS * c + IS]], 1))),
            wdn0=bf(np.concatenate([wdn[0][IS * c:IS * c + IS],
                                    np.zeros((ISP - IS, H), np.float32)], 0)),
            wdn1=bf(np.concatenate([wdn[1][IS * c:IS * c + IS],
                                    np.zeros((ISP - IS, H), np.float32)], 0)),
        )
        in_maps.append(m)

    trace = bool(int(os.environ.get("KTRACE", "0")))
    res = run_bass_kernel_spmd(nc, in_maps, core_ids=list(range(NC_)),
                               trace=trace)
    kernel.last_exec_ns = res.exec_time_ns
    tot = res.results[0]["y"].astype(np.float64)
    for c in range(1, NC_):
        tot += res.results[c]["y"]
    for c in range(NC_):
        tot[:, 256 * c:256 * c + 256] += res.results[c]["h2out"]
    out = np.empty((S, H), np.float32)
    out[perm, :] = tot.T.astype(np.float32)
    return out
